# revision 1
# baseline (speedup 1.0000x reference)
"""Multi-head causal self-attention (B=2, S=2048, E=1024, H=16, D=64) on 8
Trainium2 NeuronCores.

Sharding: batch x head-group. Core c handles batch (c // 4) and heads
[4*(c%4), 4*(c%4)+4). Each core computes QKV projection for its 4 heads,
causal flash-attention, and a partial output projection over its head
columns. Host sums the 4 partial outputs per batch and adds b_out.

v3 changes vs the original baseline:
  - warmup dummy matmuls ride out the DMA-gated startup so the PE p-state
    ramp (0.65 -> 1.2 -> 2.4 GHz) completes before real work arrives
  - normalization reads ctx PSUM directly (no staging copy); ctx psum slot
    slack comes from the deferred-ctx lag
  - proj/outproj PSUM gets bufs=2 (removes mm->copy serialization)
  - rel=1 score blocks skip their fully-masked first 128 columns
  - partial outputs stored bf16 (halves store DMA), host sums in fp64
  - tail outproj PSUM reuses the (idle by then) scores banks
"""

import sys

if "/opt/trn_rl_repo" not in sys.path:
    sys.path.insert(0, "/opt/trn_rl_repo")

import numpy as np
import ml_dtypes

import concourse.bacc as bacc
import concourse.mybir as mybir
import concourse.tile as tile

BF16 = mybir.dt.bfloat16
FP32 = mybir.dt.float32
F8 = mybir.dt.float8e4
DRMODE = mybir.MatmulPerfMode.DoubleRow

B, S, E = 2, 2048, 1024
H, DH = 16, 64
NCORES = 8
HPC = 4            # heads per core
M = HPC * DH       # 256 ctx columns per core
QC = 512           # q chunk (max wave width; also PSUM head stride)
KB = 128           # k block
SCALE = 1.0 / np.sqrt(DH)
NWARM = 64         # warmup dummy matmuls (128 cols each)
# q-chunks (q0, Q, grp). The last 512 splits 384+128 so the final
# norm/outproj tail is 4x smaller. grp = k-blocks per wave: the narrow final
# chunk processes 4 k-blocks per scores-PSUM tile / exp call, so its waves
# are not paced by per-call ACT overhead.
CHUNKS = [(0, 512, 1), (512, 512, 1), (1024, 512, 1),
          (1536, 384, 1), (1920, 128, 4)]
NCH = len(CHUNKS)


def _emit_kernel(tc, xT, wq, wk, wv, wo_d, x8, wk8, out):
    nc = tc.nc
    Exp = mybir.ActivationFunctionType.Exp

    with tc.tile_pool(name="res", bufs=1) as res, \
         tc.tile_pool(name="ps", bufs=1, space="PSUM") as ps, \
         tc.tile_pool(name="expp", bufs=16) as expp, \
         tc.tile_pool(name="scr", bufs=4) as scr, \
         tc.tile_pool(name="outb", bufs=2) as outb:

        # ---- resident SBUF tiles ----
        xt_all = res.tile([128, 8 * S], BF16, name="xt_all")
        xt = [xt_all[:, e * S:(e + 1) * S] for e in range(8)]
        xt_3d = xt_all.rearrange("p (e s) -> p e s", s=S)
        wqt = res.tile([128, 8 * M], BF16, name="wqt")
        wkt = res.tile([128, 8 * M], BF16, name="wkt")
        wvt = res.tile([128, 8 * M], BF16, name="wvt")
        wot = [res.tile([128, E], BF16, name=f"wot{i}") for i in range(2)]
        qTt = [res.tile([128, S], BF16, name=f"qTt{i}") for i in range(2)]
        kTt = [res.tile([128, S], BF16, name=f"kTt{i}") for i in range(2)]
        ctxT = [res.tile([128, S], BF16, name=f"ctxT{i}") for i in range(2)]
        # fp8 scores path (queries >= 512): one classic-layout fp8 staging
        # tile (free dims qk x mt x s) and one DoubleRow "pair" tile
        # [64, j x qk x mt x s] with head parity on partition halves {0,32}
        # and the two dh-32 k-tiles (j) in the free dim. Folding qk/mt into
        # free dims lets ONE rearrange DMA per (hh, j) re-stage a whole
        # chunk (HWDGE generation is a fixed 625ns per dma_start).
        qk8c = res.tile([128, 4 * S], F8, name="qk8c")
        qk8c4 = qk8c.rearrange("p (t m s) -> p t m s", t=2, m=2)
        qk8p = res.tile([64, 8 * S], F8, name="qk8p")
        qk8p5 = qk8p.rearrange("p (j t m s) -> p j t m s", j=2, t=2, m=2)
        # fp8 k-projection operands (host-packed e-pair layout): the
        # k-columns >= 512 (only ever consumed by the fp8 score path) are
        # projected with fp8 DoubleRow matmuls at 1/4 the PE cost
        x8t = res.tile([128, 8 * S], F8, name="x8t")
        x8_4d = x8t.rearrange("p (j t s) -> p j t s", j=4, t=2)
        wk8t = res.tile([128, 8 * M], F8, name="wk8t")
        wk8_4d = wk8t.rearrange("p (j t m) -> p j t m", j=4, t=2)
        # V with ones column: per (k-block kb, head h) a [128, 65] slab
        v1 = res.tile([128, (S // KB) * HPC * 65], BF16, name="v1")
        v1_3d = v1.rearrange("p (n c) -> p n c", c=65)
        mask = res.tile([128, 128], BF16, name="mask")
        warm_src = res.tile([128, 128], BF16, name="warm_src")

        # ---- warmup: dummy matmuls keep the PE busy (and its p-state
        # ramping) through the DMA-gated startup. They read a memset tile and
        # write a throwaway PSUM slot; the first real matmul enters a fully
        # ramped (2.4 GHz) engine.
        nc.gpsimd.memset(warm_src[:], 0.0)
        warm_ps = ps.tile([128, QC], FP32, tag="proj", bufs=2, name="warm_ps")
        for i in range(NWARM):
            nc.tensor.matmul(
                warm_ps[:, 0:128], lhsT=warm_src[:], rhs=warm_src[:],
                start=True, stop=True)

        # ---- input DMA: one batched transfer per tensor/chunk ----
        wqt_3d = wqt.rearrange("p (e m) -> p e m", m=M)
        wkt_3d = wkt.rearrange("p (e m) -> p e m", m=M)
        wvt_3d = wvt.rearrange("p (e m) -> p e m", m=M)
        xT_3d = xT.rearrange("(e p) s -> p e s", p=128)
        nc.sync.dma_start(wqt_3d[:], wq.rearrange("(e p) m -> p e m", p=128))
        nc.sync.dma_start(xt_3d[:, :, 0:256], xT_3d[:, :, 0:256])
        nc.sync.dma_start(wkt_3d[:], wk.rearrange("(e p) m -> p e m", p=128))
        nc.sync.dma_start(xt_3d[:, :, 256:QC], xT_3d[:, :, 256:QC])
        nc.sync.dma_start(wvt_3d[:], wv.rearrange("(e p) m -> p e m", p=128))
        x8_dram = x8.rearrange("p (j t s) -> p j t s", j=4, t=2)
        nc.sync.dma_start(wk8t[:], wk8)
        for chunk in range(1, S // QC):
            nc.sync.dma_start(
                x8_4d[:, :, :, chunk * QC:(chunk + 1) * QC],
                x8_dram[:, :, :, chunk * QC:(chunk + 1) * QC])
            nc.sync.dma_start(
                xt_3d[:, :, chunk * QC:(chunk + 1) * QC],
                xT_3d[:, :, chunk * QC:(chunk + 1) * QC])
        for i in range(2):
            nc.sync.dma_start(wot[i][:], wo_d[i * 128:(i + 1) * 128, :])

        # ---- constants ----
        nc.gpsimd.memset(v1[:], 1.0)  # data columns overwritten by V proj
        # stair mask: keep where k_local <= q_local (within a 128x128 block)
        nc.gpsimd.memset(mask[:], 1.0)
        nc.gpsimd.affine_select(
            out=mask[:], in_=mask[:],
            compare_op=mybir.AluOpType.is_ge,
            fill=0.0, base=0,
            pattern=[[1, 128]],
            channel_multiplier=-1,
        )

        # ---- emission helpers ----
        def stage_f8(ci, mt, kind, pqk, scale=None):
            # fp8 classic staging into the (qk, mt) slab of qk8c; the
            # partition rearrange into qk8p is a separate batched piece
            s0, Q, _ = CHUNKS[ci]
            t = 0 if kind == "q" else 1
            if scale is None:
                nc.vector.tensor_copy(qk8c4[:, t, mt, s0:s0 + Q], pqk[:, 0:Q])
            else:
                nc.vector.tensor_scalar_mul(qk8c4[:, t, mt, s0:s0 + Q],
                                            pqk[:, 0:Q], scale)

        def emit_rearrange(ci, k_only=False):
            # partition rearrange into the DoubleRow pair tile for chunk
            # ci's columns, all (qk, mt) slabs at once:
            # pt[32*hh + d%32, (d//32), t, m, s] = classic[64*hh + d, t, m, s]
            s0, Q, _ = CHUNKS[ci]
            t0 = 1 if k_only else 0
            for hh in range(2):
                for j in range(2):
                    nc.sync.dma_start(
                        qk8p5[32 * hh:32 * hh + 32, j, t0:2, :, s0:s0 + Q],
                        qk8c4[64 * hh + 32 * j: 64 * hh + 32 * j + 32,
                              t0:2, :, s0:s0 + Q])

        def stage_qk(ci, mt, kind, pqk):
            # chunk 0 queries score in bf16 (classic layout); all other
            # queries score in fp8 DoubleRow. k is needed in fp8 by every
            # fp8 chunk, and in bf16 only for chunk 0's k-blocks.
            s0, Q, _ = CHUNKS[ci]
            dstt = qTt if kind == "q" else kTt
            if ci == 0:
                nc.vector.tensor_copy(dstt[mt][:, s0:s0 + Q], pqk[:, 0:Q])
            if kind == "k" or ci >= 1:
                stage_f8(ci, mt, kind, pqk)

        def emit_proj_k8(ci, mt):
            # k-projection for fp8-only consumers via fp8 DoubleRow over
            # host-packed e-pairs: 1/4 the PE cost of the bf16 projection
            s0, Q, _ = CHUNKS[ci]
            pk = ps.tile([128, QC], FP32, tag="proj", bufs=2,
                         name=f"pk8_{ci}_{mt}")
            # a-piece OUTER: interleaving two DoubleRow accumulation groups
            # (j inner per region) miscomputes on hardware -- each region's
            # 4-instruction group must run contiguously
            for a in range(0, Q, 256):
                b = min(a + 256, Q)
                for j in range(4):
                    nc.tensor.matmul(
                        pk[:, a:b],
                        lhsT=wk8_4d[:, j, :, mt * 128:(mt + 1) * 128],
                        rhs=x8_4d[:, j, :, s0 + a: s0 + b],
                        start=(j == 0), stop=(j == 3),
                        perf_mode=DRMODE)
            # wk8 is host-scaled by 64 (w values ~0.02 sit in e4m3's
            # subnormal range, which the PE flushes to zero); undo here
            stage_f8(ci, mt, "k", pk, scale=1.0 / 64.0)

        def emit_proj_qk(ci, mt, wt, kind):
            s0, Q, _ = CHUNKS[ci]
            pqk = ps.tile([128, QC], FP32, tag="proj", bufs=2,
                          name=f"p{kind}_{ci}_{mt}")
            for e in range(8):
                nc.tensor.matmul(
                    pqk[:, 0:Q],
                    lhsT=wt[:, e * M + mt * 128: e * M + (mt + 1) * 128],
                    rhs=xt[e][:, s0:s0 + Q],
                    start=(e == 0), stop=(e == 7))
            stage_qk(ci, mt, kind, pqk)

        def emit_proj_qk_interleaved(ci, mt):
            # startup projection in column pieces matched to the split x
            # chunk-0 transfers: q[0:256] (after x cols 0:256 + wq),
            # k[0:128] (wave 0's only k-block), then the remainders as the
            # second x piece lands. Each region's accumulation group is
            # contiguous; staging copies go per piece so wave 0 isn't gated
            # on the full chunk.
            s0, Q, _ = CHUNKS[ci]
            pq = ps.tile([128, QC], FP32, tag="proj", bufs=2,
                         name=f"pqi_{ci}_{mt}")
            pk = ps.tile([128, QC], FP32, tag="proj", bufs=2,
                         name=f"pki_{ci}_{mt}")
            pieces = [(pq, wqt, 0, 256), (pk, wkt, 0, 128),
                      (pq, wqt, 256, Q), (pk, wkt, 128, Q)]
            for dst, wt, a, b in pieces:
                for e in range(8):
                    nc.tensor.matmul(
                        dst[:, a:b],
                        lhsT=wt[:, e * M + mt * 128: e * M + (mt + 1) * 128],
                        rhs=xt[e][:, s0 + a: s0 + b],
                        start=(e == 0), stop=(e == 7))
                tgt = qTt if wt is wqt else kTt
                nc.vector.tensor_copy(tgt[mt][:, s0 + a: s0 + b],
                                      dst[:, a:b])
            stage_f8(ci, mt, "k", pk)

        def emit_proj_v(sblk):
            pv = ps.tile([128, M], FP32, tag="proj", bufs=2, name=f"pv_{sblk}")
            for e in range(8):
                nc.tensor.matmul(
                    pv[:],
                    lhsT=xt[e][:, sblk * 128:(sblk + 1) * 128],
                    rhs=wvt[:, e * M:(e + 1) * M],
                    start=(e == 0), stop=(e == 7))
            nc.vector.tensor_copy(
                v1_3d[:, sblk * HPC:(sblk + 1) * HPC, 0:64],
                pv[:].rearrange("p (h c) -> p h c", c=64))

        def proj_qk_pieces(ci):
            pcs = []
            for mt in range(2):
                pcs.append(lambda mt=mt: emit_proj_qk(ci, mt, wqt, "q"))
                if ci == 0:
                    pcs.append(lambda mt=mt: emit_proj_qk(ci, mt, wkt, "k"))
                else:
                    pcs.append(lambda mt=mt: emit_proj_k8(ci, mt))
            return pcs

        def proj_v_pieces(blks):
            return [lambda sb=sb: emit_proj_v(sb) for sb in blks]

        ob_tiles = {}
        out_3d = out.rearrange("(q p) f -> p q f", p=128)

        def emit_outproj(ci, qq, fc, tail=False):
            q0, Q, _ = CHUNKS[ci]
            nqb = Q // 128
            qb = q0 // 128 + qq
            last = ci == NCH - 1
            if qq == 0 and fc == 0:
                ob_tiles[ci] = outb.tile([128, nqb * E], BF16, tag="ob",
                                         name=f"ob_{qb}")
            ob = ob_tiles[ci]
            # tail outprojs borrow the scores PSUM slots (attention is done
            # by then), keeping mm->copy->mm free of slot serialization
            tag = "scores" if tail else "proj"
            po = ps.tile([128, QC], FP32, tag=tag, bufs=2,
                         name=f"po_{qb}_{fc}")
            for mc in range(2):
                nc.tensor.matmul(
                    po[:],
                    lhsT=ctxT[mc][:, qb * 128:(qb + 1) * 128],
                    rhs=wot[mc][:, fc * QC:(fc + 1) * QC],
                    start=(mc == 0), stop=(mc == 1))
            if last and fc == 1:
                # final piece: stage on the (idle-by-now) ACT engine so the
                # two last copies run in parallel instead of serializing on
                # the DVE queue
                nc.scalar.activation(
                    ob[:, qq * E + fc * QC: qq * E + (fc + 1) * QC], po[:],
                    mybir.ActivationFunctionType.Copy)
            else:
                nc.vector.tensor_copy(
                    ob[:, qq * E + fc * QC: qq * E + (fc + 1) * QC], po[:])
            if last:
                # final chunk: one merged store after both halves are staged
                # (HWDGE generation is a fixed 625ns per dma_start, serial)
                if fc == 1:
                    nc.sync.dma_start(
                        out[qb * 128:(qb + 1) * 128, :],
                        ob[:, qq * E: (qq + 1) * E])
                if (qq, fc) == (nqb - 1, 1):
                    del ob_tiles[ci]
            elif (qq, fc) == (nqb - 1, 1):
                nc.sync.dma_start(
                    out_3d[:, q0 // 128: q0 // 128 + nqb, :],
                    ob.rearrange("p (q f) -> p q f", f=E))
                del ob_tiles[ci]

        def outproj_pieces(ci, tail=False):
            _, Q, _ = CHUNKS[ci]
            return [lambda qq=qq, fc=fc: emit_outproj(ci, qq, fc, tail=tail)
                    for qq in range(Q // 128) for fc in range(2)]

        # ---- attention waves (one head PAIR, grp k-blocks) ----
        # kd = kb*128 - q0: offset of the k-block's diagonal within the
        # chunk's q columns. kd >= 128: cols [0, kd) are fully masked -> skip
        # in scores (exp still covers them for kd == 128; the garbage is
        # never consumed). kd >= 0: stair-mask cols [kd, kd+128).
        # For grp > 1, each wave covers grp consecutive k-blocks laid out as
        # column groups of width Q inside the head's PSUM half, sharing one
        # exp call.
        def wave_scores(ci, pair, g):
            s0, Q, grp = CHUNKS[ci]
            mt = pair
            fp8 = ci >= 1
            sc_ps = ps.tile([128, 2 * QC], FP32, tag="scores", bufs=2,
                            name=f"s_{ci}_{pair}_{g}")
            kds = [(j, (g * grp + j) * 128 - s0) for j in range(grp)]
            lo_e = 0
            for hh in range(2):
                r0 = hh * 64
                off = hh * QC
                for j, kd in kds:
                    kb = g * grp + j
                    lo = kd if (kd >= 128 and grp == 1) else 0
                    if hh == 0 and kd >= 128 and grp == 1:
                        lo_e = kd
                    if fp8:
                        # DoubleRow: dh 2x32 k-tiles, head at base 32*hh;
                        # moving free = 2*w caps piece width at 256
                        a = lo
                        while a < Q:
                            b = min(a + 256, Q)
                            nc.tensor.matmul(
                                sc_ps[:, off + j * Q + a: off + j * Q + b],
                                lhsT=qk8p5[32 * hh:32 * hh + 32, :, 1, mt,
                                           kb * 128:(kb + 1) * 128],
                                rhs=qk8p5[32 * hh:32 * hh + 32, :, 0, mt,
                                          s0 + a: s0 + b],
                                start=True, stop=True,
                                perf_mode=DRMODE)
                            a = b
                    else:
                        nc.tensor.matmul(
                            sc_ps[:, off + j * Q + lo: off + (j + 1) * Q],
                            lhsT=kTt[mt][r0:r0 + 64, kb * 128:(kb + 1) * 128],
                            rhs=qTt[mt][r0:r0 + 64, s0 + lo: s0 + Q],
                            start=True, stop=True)
            ex = expp.tile([128, 2 * QC], BF16, tag="ex",
                           name=f"e_{ci}_{pair}_{g}")
            W = grp * Q
            if lo_e or W < QC:
                # both heads in one strided-AP call: the ACT engine charges
                # by total free size, so this halves the per-call init cost
                # vs one call per head
                ex3 = ex.rearrange("p (h q) -> p h q", h=2)
                sc3 = sc_ps.rearrange("p (h q) -> p h q", h=2)
                nc.scalar.activation(ex3[:, :, lo_e:W], sc3[:, :, lo_e:W],
                                     Exp, scale=SCALE)
            else:
                nc.scalar.activation(ex[:], sc_ps[:], Exp, scale=SCALE)
            for hh in range(2):
                off = hh * QC
                for j, kd in kds:
                    if kd >= 0:
                        nc.vector.tensor_mul(
                            ex[:, off + j * Q + kd: off + j * Q + kd + 128],
                            ex[:, off + j * Q + kd: off + j * Q + kd + 128],
                            mask[:])
            return ex

        def wave_ctx(ci, pair, g, ex, ctx_pair, nkb):
            s0, Q, grp = CHUNKS[ci]
            for hh in range(2):
                h = 2 * pair + hh
                off = hh * QC
                for j in range(grp):
                    kb = g * grp + j
                    kd = kb * 128 - s0
                    lo = max(kd, 0)
                    nc.tensor.matmul(
                        ctx_pair[hh][:, lo:Q],
                        lhsT=v1_3d[:, kb * HPC + h, :],
                        rhs=ex[:, off + j * Q + lo: off + (j + 1) * Q],
                        start=(kb == 0), stop=(kb == nkb - 1),
                        skip_group_check=True)

        def norm_pieces(ci, items):
            # deferred norm for a finished pair, split into 4 wave-pieces
            # (recips / broadcasts / mul A / mul B) so the multiplies never
            # head-of-line-block the in-order DVE queue waiting on the Pool
            # broadcasts. The norm multiply reads ctx PSUM directly (no
            # staging copy); the slot is released when it completes.
            s0, Q, _ = CHUNKS[ci]
            state = {}

            def p_recips():
                state["recs"] = []
                for h, ctx_ps in items:
                    rec = scr.tile([1, QC], FP32, tag="rec",
                                   name=f"r_{ci}_{h}")
                    nc.vector.reciprocal(rec[:, 0:Q], ctx_ps[64:65, 0:Q])
                    state["recs"].append(rec)

            def p_bcasts():
                state["recbs"] = []
                for (h, _), rec in zip(items, state["recs"]):
                    recb = scr.tile([64, QC], FP32, tag="recb",
                                    name=f"rb_{ci}_{h}")
                    nc.gpsimd.partition_broadcast(recb[:, 0:Q], rec[:, 0:Q])
                    state["recbs"].append(recb)

            def p_mul(i):
                h, ctx_ps = items[i]
                mt, r0 = h // 2, (h % 2) * 64
                nc.vector.tensor_mul(
                    ctxT[mt][r0:r0 + 64, s0:s0 + Q],
                    ctx_ps[0:64, 0:Q], state["recbs"][i][:, 0:Q])

            return [p_recips, p_bcasts,
                    lambda: p_mul(0), lambda: p_mul(1)]

        def emit_norm_pair(ci, items):
            # final-pair norms: interleave the two heads' recip/broadcast/
            # multiply so the DVE and Pool stages pipeline instead of
            # serializing head-by-head at the kernel tail
            s0, Q, _ = CHUNKS[ci]
            recs = []
            for h, ctx_ps in items:
                rec = scr.tile([1, QC], FP32, tag="rec", name=f"r_{ci}_{h}")
                nc.vector.reciprocal(rec[:, 0:Q], ctx_ps[64:65, 0:Q])
                recs.append(rec)
            recbs = []
            for (h, _), rec in zip(items, recs):
                recb = scr.tile([64, QC], FP32, tag="recb",
                                name=f"rb_{ci}_{h}")
                nc.gpsimd.partition_broadcast(recb[:, 0:Q], rec[:, 0:Q])
                recbs.append(recb)
            for (h, ctx_ps), recb in zip(items, recbs):
                mt, r0 = h // 2, (h % 2) * 64
                nc.vector.tensor_mul(
                    ctxT[mt][r0:r0 + 64, s0:s0 + Q],
                    ctx_ps[0:64, 0:Q], recb[:, 0:Q])

        # ---- main schedule ----
        # exp-table warm: overlap the ~2.7us table load with the initial DMA
        warm = scr.tile([1, 1], FP32, tag="warm", bufs=1, name="warm")
        nc.gpsimd.memset(warm[:], 0.0)
        nc.scalar.activation(warm[:], warm[:], Exp)

        emit_proj_qk_interleaved(0, 0)
        pending_norms = []
        for ci in range(NCH):
            q0, Q, grp = CHUNKS[ci]
            nkb = (q0 + Q) // 128
            ngrp = nkb // grp
            waves = [(pair, g) for pair in range(2)
                     for g in range(ngrp)]
            head = []   # pieces pinned to the earliest waves, one per wave
            extra = []  # pieces distributed evenly over all waves
            pins = {}   # wave -> pieces with exact placement constraints
            # Each chunk's waves carry: its own V projections (head), the
            # NEXT chunk's full q/k projection + fused rearrange (extra, so
            # the DMA-staging chain completes well before that chunk's first
            # scores), and out-projection backlog.
            if ci == 0:
                qk0 = proj_qk_pieces(0)
                v0 = proj_v_pieces(range(0, 4))
                pins = {0: [qk0[2], v0[0]], 1: [qk0[3], v0[1]],
                        2: [v0[2], lambda: emit_rearrange(0, k_only=True)],
                        3: [v0[3]]}
                extra += proj_qk_pieces(1) + [lambda: emit_rearrange(1)]
            elif ci == 1:
                head += proj_v_pieces(range(4, 8))
                extra += proj_qk_pieces(2) + [lambda: emit_rearrange(2)]
            elif ci == 2:
                head += proj_v_pieces(range(8, 12))
                extra += (proj_qk_pieces(3) + [lambda: emit_rearrange(3)]
                          + outproj_pieces(0))
            elif ci == 3:
                head += proj_v_pieces(range(12, 16))
                extra += (proj_qk_pieces(4) + [lambda: emit_rearrange(4)]
                          + outproj_pieces(1))
            else:
                extra += outproj_pieces(2) + outproj_pieces(3)
            sched = {w: [] for w in range(len(waves))}
            for w, pcs in pins.items():
                sched[w].extend(pcs)
            for j, pc in enumerate(head):
                sched[j].append(pc)
            if extra:
                w0 = max(0, min(2 if ci == 0 else 4,
                                len(waves) - len(extra)))
                span_w = len(waves) - w0
                for j, pc in enumerate(extra):
                    sched[w0 + j * span_w // len(extra)].append(pc)

            ctx_tiles = {}
            ctx_queue = []
            for w, (pair, g) in enumerate(waves):
                if g == 0:
                    ctx_tiles[pair] = [
                        ps.tile([65, QC], FP32, tag="ctx", bufs=2,
                                name=f"c_{ci}_{pair}_{hh}")
                        for hh in range(2)]
                ex = wave_scores(ci, pair, g)
                if pending_norms:
                    pending_norms.pop(0)()
                last_of_pair = g == ngrp - 1
                final_pair = last_of_pair and pair == 1 and ci + 1 == NCH
                if not final_pair:
                    for pc in sched[w]:
                        pc()
                ctx_queue.append((pair, g, ex))
                # defer ctx so (a) the pair's first ctx matmuls don't stall
                # on PSUM slots still being normed, (b) PE has scores to run
                # while exp catches up. Grouped chunks defer until wave 3 so
                # the previous pair's lazily-spread norm muls (waves 2,3)
                # have released the slots.
                if grp > 1:
                    lag = max(0, 3 - g)
                else:
                    lag = 14 if g < 14 else 0
                while len(ctx_queue) > lag or \
                        (ctx_queue and last_of_pair):
                    qpair, qg, qex = ctx_queue.pop(0)
                    wave_ctx(ci, qpair, qg, qex, ctx_tiles[qpair], nkb)
                if last_of_pair:
                    h0 = 2 * pair
                    items = [(h0 + hh, ctx_tiles[pair][hh])
                             for hh in range(2)]
                    if final_pair:
                        # kernel tail: emit the norm chain ahead of this
                        # wave's filler copies so the recips don't queue
                        # behind them on the in-order DVE
                        emit_norm_pair(ci, items)
                        for pc in sched[w]:
                            pc()
                    else:
                        # lazily spread over the next 4 waves (pair 0's run
                        # inside this chunk's pair-1 waves; pair 1's inside
                        # the next chunk)
                        pending_norms = norm_pieces(ci, items)
        for pc in outproj_pieces(NCH - 1, tail=True):
            pc()


def build_module():
    nc = bacc.Bacc("TRN2", target_bir_lowering=False, debug=False)
    xT = nc.dram_tensor("xT", [E, S], BF16, kind="ExternalInput").ap()
    wq = nc.dram_tensor("wq", [E, M], BF16, kind="ExternalInput").ap()
    wk = nc.dram_tensor("wk", [E, M], BF16, kind="ExternalInput").ap()
    wv = nc.dram_tensor("wv", [E, M], BF16, kind="ExternalInput").ap()
    wo = nc.dram_tensor("wo", [M, E], BF16, kind="ExternalInput").ap()
    x8 = nc.dram_tensor("x8", [128, 8 * S], F8, kind="ExternalInput").ap()
    wk8 = nc.dram_tensor("wk8", [128, 8 * M], F8, kind="ExternalInput").ap()
    out = nc.dram_tensor("out", [S, E], BF16, kind="ExternalOutput").ap()
    with tile.TileContext(nc) as tc:
        _emit_kernel(tc, xT, wq, wk, wv, wo, x8, wk8, out)
    nc.compile()
    return nc


def _pack_epairs(aT):
    """[E, N] -> [128, 4*2*N] fp8: e-tile pairs side by side per partition
    (DoubleRow packing: out[p, j, t, n] = aT[(2j+t)*128 + p, n])."""
    e4m3 = ml_dtypes.float8_e4m3
    E_, N = aT.shape
    a = np.asarray(aT, dtype=np.float32).reshape(4, 2, 128, N)
    a = np.ascontiguousarray(a.transpose(2, 0, 1, 3)).astype(e4m3)
    return a.reshape(128, 8 * N)


def make_in_maps(x, w_qkv):
    """Per-core input dicts (bf16/fp8, pre-transposed host-side)."""
    bf = ml_dtypes.bfloat16
    xTb = [np.ascontiguousarray(x[b].T).astype(bf) for b in range(B)]
    x8b = [_pack_epairs(x[b].T) for b in range(B)]
    in_maps = []
    for c in range(NCORES):
        b, g = c // 4, c % 4
        cols = slice(g * M, (g + 1) * M)
        wkT = np.ascontiguousarray(w_qkv[E:][cols, :].T)
        in_maps.append({
            "xT": xTb[b],
            "wq": np.ascontiguousarray(w_qkv[cols, :].T).astype(bf),
            "wk": wkT.astype(bf),
            "wv": np.ascontiguousarray(w_qkv[2 * E:][cols, :].T).astype(bf),
            "x8": x8b[b],
            "wk8": _pack_epairs(wkT * 64.0),
            "wo": None,  # filled in kernel(), needs w_out
        })
    return in_maps


_RUNNER = None
_SHARDED = None


def _get_runner():
    """Build the Bass module once and return a cached callable
    (in_maps) -> [NCORES, S, E] bf16 partial outputs."""
    global _RUNNER
    if _RUNNER is not None:
        return _RUNNER

    nc = build_module()

    from concourse import bass2jax
    import jax
    from jax.sharding import Mesh, PartitionSpec
    from jax.experimental.shard_map import shard_map

    bass2jax.install_neuronx_cc_hook()

    in_names = ["xT", "wq", "wk", "wv", "x8", "wk8", "wo"]
    out_names = ["out"]
    out_avals = [jax.core.ShapedArray((S, E), ml_dtypes.bfloat16)]
    n_params = len(in_names)
    all_names = in_names + out_names
    partition_name = (nc.partition_id_tensor.name
                      if nc.partition_id_tensor is not None else None)
    if partition_name is not None:
        all_names = all_names + [partition_name]

    def _body(*args):
        operands = list(args)
        if partition_name is not None:
            operands.append(bass2jax.partition_id_tensor())
        outs = bass2jax._bass_exec_p.bind(
            *operands,
            out_avals=tuple(out_avals),
            in_names=tuple(all_names),
            out_names=tuple(out_names),
            lowering_input_output_aliases=(),
            sim_require_finite=True,
            sim_require_nnan=True,
            nc=nc,
        )
        return tuple(outs)

    devices = jax.devices()[:NCORES]
    mesh = Mesh(np.asarray(devices), ("core",))
    n_outs = len(out_names)
    in_specs = (PartitionSpec("core"),) * (n_params + n_outs)
    out_specs = (PartitionSpec("core"),) * n_outs
    sharded = jax.jit(
        shard_map(_body, mesh=mesh, in_specs=in_specs, out_specs=out_specs,
                  check_rep=False),
        donate_argnums=tuple(range(n_params, n_params + n_outs)),
        keep_unused=True,
    )
    global _SHARDED
    _SHARDED = sharded

    def run(in_maps):
        concat_in = [
            np.concatenate([np.asarray(in_maps[c][n]) for c in range(NCORES)],
                           axis=0)
            for n in in_names
        ]
        concat_zeros = [np.zeros((NCORES * S, E), ml_dtypes.bfloat16)]
        out_arrs = sharded(*concat_in, *concat_zeros)
        return np.asarray(out_arrs[0]).reshape(NCORES, S, E)

    _RUNNER = run
    return run


def kernel(x, w_qkv, w_out, b_out):
    x = np.asarray(x, dtype=np.float32)
    w_qkv = np.asarray(w_qkv, dtype=np.float32)
    w_out = np.asarray(w_out, dtype=np.float32)
    b_out = np.asarray(b_out, dtype=np.float32)

    bf = ml_dtypes.bfloat16
    in_maps = make_in_maps(x, w_qkv)
    for c in range(NCORES):
        g = c % 4
        cols = slice(g * M, (g + 1) * M)
        in_maps[c]["wo"] = np.ascontiguousarray(w_out[:, cols].T).astype(bf)

    run = _get_runner()
    partials = run(in_maps)  # [8, S, E] bf16

    out = np.empty((B, S, E), np.float32)
    for b in range(B):
        acc = partials[4 * b].astype(np.float64)
        for i in range(1, 4):
            acc += partials[4 * b + i].astype(np.float64)
        out[b] = (acc + b_out.astype(np.float64)).astype(np.float32)
    return out



# revision 44
# speedup vs baseline: 1.0109x; 1.0109x over previous
"""Multi-head causal self-attention (B=2, S=2048, E=1024, H=16, D=64) on 8
Trainium2 NeuronCores.

Sharding: batch x head-group. Core c handles batch (c // 4) and heads
[4*(c%4), 4*(c%4)+4). Each core computes QKV projection for its 4 heads,
causal flash-attention, and a partial output projection over its head
columns. Host sums the 4 partial outputs per batch and adds b_out.

v4 changes vs v3:
  - flipped ctx matmuls for chunks 0-3: out = [q-part 128, 65] with
    lhsT = ex q-window (stationary), rhs = v1 slab. PE cost per
    (head, kb, qb) drops from `cols` to 65 rows (full 128-partition
    output). The softmax denominator lands per-q-PARTITION, so the norm
    is a cheap [128,nqb] reciprocal + per-partition-scalar multiplies
    (no Pool partition_broadcast).
  - ctx_qm [q, m] bf16 is transposed back to ctxT [m, q] with
    dma_start_transpose (XBAR), one [128,128] tile per (pair, q-block).
  - chunk 4 (last 128 cols) keeps the v3 unflipped path so the kernel
    tail avoids the DMA-transpose latency.
  - q projection for chunks 1-4 via fp8 DoubleRow (host-packed wq8,
    x8), mirroring the k8 path: 1/4 the PE cost of the bf16 proj.
  - stair masking of ex moved from DVE tensor_mul to Pool affine_select
    (SBUF->SBUF, one call covers both heads of a wave).
"""

import sys

if "/opt/trn_rl_repo" not in sys.path:
    sys.path.insert(0, "/opt/trn_rl_repo")

import numpy as np
import ml_dtypes

import concourse.bacc as bacc
import concourse.mybir as mybir
import concourse.tile as tile

BF16 = mybir.dt.bfloat16
FP32 = mybir.dt.float32
F8 = mybir.dt.float8e4
DRMODE = mybir.MatmulPerfMode.DoubleRow

B, S, E = 2, 2048, 1024
H, DH = 16, 64
NCORES = 8
HPC = 4            # heads per core
M = HPC * DH       # 256 ctx columns per core
QC = 512           # q chunk (max wave width; also PSUM head stride)
KB = 128           # k block
SCALE = 1.0 / np.sqrt(DH)
NWARM = 64         # warmup dummy matmuls (128 cols each)
# q-chunks (q0, Q, grp). The last 512 splits 384+128 so the final
# norm/outproj tail is 4x smaller. grp = k-blocks per wave for the narrow
# final chunk (shares one exp call across 4 k-blocks).
CHUNKS = [(0, 512, 1), (512, 512, 1), (1024, 512, 1),
          (1536, 384, 1), (1920, 128, 4)]
NCH = len(CHUNKS)


def _emit_kernel(tc, xT, wq, wk, wv, wo_d, x8, wk8, wq8, out):
    nc = tc.nc
    Exp = mybir.ActivationFunctionType.Exp

    with tc.tile_pool(name="res", bufs=1) as res, \
         tc.tile_pool(name="ps", bufs=1, space="PSUM") as ps, \
         tc.tile_pool(name="expp", bufs=16) as expp, \
         tc.tile_pool(name="scr", bufs=4) as scr, \
         tc.tile_pool(name="cqm", bufs=2) as cqm_pool, \
         tc.tile_pool(name="outb", bufs=2) as outb:

        # ---- resident SBUF tiles ----
        xt_all = res.tile([128, 8 * S], BF16, name="xt_all")
        xt = [xt_all[:, e * S:(e + 1) * S] for e in range(8)]
        xt_3d = xt_all.rearrange("p (e s) -> p e s", s=S)
        wqt = res.tile([128, 8 * M], BF16, name="wqt")
        wkt = res.tile([128, 8 * M], BF16, name="wkt")
        wvt = res.tile([128, 8 * M], BF16, name="wvt")
        wot = [res.tile([128, E], BF16, name=f"wot{i}") for i in range(2)]
        qTt = [res.tile([128, S], BF16, name=f"qTt{i}") for i in range(2)]
        kTt = [res.tile([128, S], BF16, name=f"kTt{i}") for i in range(2)]
        ctxT = [res.tile([128, S], BF16, name=f"ctxT{i}") for i in range(2)]
        # fp8 scores path (queries >= 512): per chunk one classic-layout fp8
        # staging tile (free dims qk x mt x s) and one DoubleRow "pair" tile
        # [64, j x qk x mt x s] with head parity on partition halves {0,32}
        # and the two dh-32 k-tiles (j) in the free dim. PER-CHUNK tiles:
        # the dependency tracker flattens strided APs to byte ranges, so a
        # single shared tile makes chunk ci's scores falsely wait on chunk
        # ci+1's rearrange DMAs.
        qk8cs = [res.tile([128, 4 * CH[1]], F8, name=f"qk8c{i}")
                 for i, CH in enumerate(CHUNKS)]
        qk8c4s = [t.rearrange("p (t m s) -> p t m s", t=2, m=2)
                  for t in qk8cs]
        qk8ps = [res.tile([64, 8 * CH[1]], F8, name=f"qk8p{i}")
                 for i, CH in enumerate(CHUNKS)]
        qk8p5s = [t.rearrange("p (j t m s) -> p j t m s", j=2, t=2, m=2)
                  for t in qk8ps]

        def kb_loc(kb):
            # global k-block -> (chunk index, local column offset)
            for i in range(NCH - 1, -1, -1):
                if kb * 128 >= CHUNKS[i][0]:
                    return i, kb * 128 - CHUNKS[i][0]
            raise AssertionError
        # fp8 projection operands (host-packed e-pair layout): q/k columns
        # that are only ever consumed by the fp8 score path are projected
        # with fp8 DoubleRow matmuls at 1/4 the PE cost
        x8t = res.tile([128, 8 * S], F8, name="x8t")
        x8_4d = x8t.rearrange("p (j t s) -> p j t s", j=4, t=2)
        wk8t = res.tile([128, 8 * M], F8, name="wk8t")
        wk8_4d = wk8t.rearrange("p (j t m) -> p j t m", j=4, t=2)
        wq8t = res.tile([128, 8 * M], F8, name="wq8t")
        wq8_4d = wq8t.rearrange("p (j t m) -> p j t m", j=4, t=2)
        # V with ones column: per (k-block kb, head h) a [128, 65] slab
        v1 = res.tile([128, (S // KB) * HPC * 65], BF16, name="v1")
        v1_3d = v1.rearrange("p (n c) -> p n c", c=65)
        warm_src = res.tile([128, 128], BF16, name="warm_src")
        ident = res.tile([128, 128], BF16, name="ident")

        # ---- warmup: dummy matmuls keep the PE busy (and its p-state
        # ramping) through the DMA-gated startup.
        nc.gpsimd.memset(warm_src[:], 0.0)
        warm_ps = ps.tile([128, QC], FP32, tag="proj", bufs=2, name="warm_ps")
        for i in range(NWARM):
            nc.tensor.matmul(
                warm_ps[:, 0:128], lhsT=warm_src[:], rhs=warm_src[:],
                start=True, stop=True)

        # ---- input DMA: one batched transfer per tensor/chunk ----
        wqt_3d = wqt.rearrange("p (e m) -> p e m", m=M)
        wkt_3d = wkt.rearrange("p (e m) -> p e m", m=M)
        wvt_3d = wvt.rearrange("p (e m) -> p e m", m=M)
        xT_3d = xT.rearrange("(e p) s -> p e s", p=128)
        # order: chunk-0 bf16 operands first (pqi/pki), then the fp8
        # operands for ALL chunks (small; chunks 1+ exp work can only start
        # once q8/k8 are projected + rearranged, and that work is what keeps
        # the ACT engine fed during the remaining xt transfers), then the
        # bf16 x chunks (V projections, consumed later) and wo.
        nc.sync.dma_start(wqt_3d[:], wq.rearrange("(e p) m -> p e m", p=128))
        nc.sync.dma_start(xt_3d[:, :, 0:256], xT_3d[:, :, 0:256])
        nc.sync.dma_start(wkt_3d[:], wk.rearrange("(e p) m -> p e m", p=128))
        nc.sync.dma_start(xt_3d[:, :, 256:QC], xT_3d[:, :, 256:QC])
        x8_dram = x8.rearrange("p (j t s) -> p j t s", j=4, t=2)
        nc.sync.dma_start(wq8t[:], wq8)
        nc.sync.dma_start(wk8t[:], wk8)

        def emit_x8_load(chunk):
            nc.sync.dma_start(
                x8_4d[:, :, :, chunk * QC:(chunk + 1) * QC],
                x8_dram[:, :, :, chunk * QC:(chunk + 1) * QC])

        emit_x8_load(1)
        nc.sync.dma_start(wvt_3d[:], wv.rearrange("(e p) m -> p e m", p=128))
        emit_x8_load(2)
        emit_x8_load(3)

        # xt[1..3] and wot are consumed late (V projections of later chunks,
        # outproj). Their dma_starts are deferred into the wave schedule so
        # the per-chunk qk8p rearranges (which gate each chunk's scores and
        # hence the ACT-bound steady state) aren't queued behind them on the
        # serial DMA device.
        def emit_xt_load(chunk):
            nc.sync.dma_start(
                xt_3d[:, :, chunk * QC:(chunk + 1) * QC],
                xT_3d[:, :, chunk * QC:(chunk + 1) * QC])

        def emit_wot_load():
            for i in range(2):
                nc.sync.dma_start(wot[i][:], wo_d[i * 128:(i + 1) * 128, :])

        # ---- constants ----
        # exp-table warm first: the ~2.7us table load + warm call must not
        # queue behind the big v1 memset on the Pool engine
        warm = scr.tile([1, 1], FP32, tag="warm", bufs=1, name="warm")
        nc.gpsimd.memset(warm[:], 0.0)
        nc.scalar.activation(warm[:], warm[:],
                             mybir.ActivationFunctionType.Exp)
        nc.gpsimd.memset(v1[:], 1.0)  # data columns overwritten by V proj
        # identity matrix for PE transposes (keep where q_local == partition)
        nc.gpsimd.memset(ident[:], 1.0)
        nc.gpsimd.affine_select(
            out=ident[:], in_=ident[:],
            compare_op=mybir.AluOpType.is_equal,
            fill=0.0, base=0,
            pattern=[[1, 128]],
            channel_multiplier=-1,
        )

        # ---- emission helpers ----
        def stage_f8(ci, mt, kind, pqk, scale=None):
            # fp8 classic staging into the (qk, mt) slab of chunk ci's qk8c
            s0, Q, _ = CHUNKS[ci]
            t = 0 if kind == "q" else 1
            if scale is None:
                nc.vector.tensor_copy(qk8c4s[ci][:, t, mt, 0:Q], pqk[:, 0:Q])
            else:
                nc.vector.tensor_scalar_mul(qk8c4s[ci][:, t, mt, 0:Q],
                                            pqk[:, 0:Q], scale)

        def emit_rearrange(ci, mt, k_only=False):
            # partition rearrange into the DoubleRow pair tile for chunk
            # ci's mt slab (per-mt so pair 0's scores aren't gated on the
            # mt=1 projections)
            s0, Q, _ = CHUNKS[ci]
            t0 = 1 if k_only else 0
            for hh in range(2):
                for j in range(2):
                    nc.sync.dma_start(
                        qk8p5s[ci][32 * hh:32 * hh + 32, j, t0:2,
                                   mt:mt + 1, 0:Q],
                        qk8c4s[ci][64 * hh + 32 * j: 64 * hh + 32 * j + 32,
                                   t0:2, mt:mt + 1, 0:Q])

        def stage_qk(ci, mt, kind, pqk):
            # chunk 0 queries score in bf16 (classic layout); all other
            # queries score in fp8 DoubleRow. k is needed in fp8 by every
            # fp8 chunk, and in bf16 only for chunk 0's k-blocks.
            s0, Q, _ = CHUNKS[ci]
            dstt = qTt if kind == "q" else kTt
            if ci == 0:
                nc.vector.tensor_copy(dstt[mt][:, s0:s0 + Q], pqk[:, 0:Q])
            if kind == "k" or ci >= 1:
                stage_f8(ci, mt, kind, pqk)

        def emit_proj_qk8(ci, mt, kind):
            # q/k projection for fp8-only consumers via fp8 DoubleRow over
            # host-packed e-pairs: 1/4 the PE cost of the bf16 projection
            s0, Q, _ = CHUNKS[ci]
            w8 = wq8_4d if kind == "q" else wk8_4d
            pk = ps.tile([128, QC], FP32, tag="proj", bufs=2,
                         name=f"p8{kind}_{ci}_{mt}")
            # a-piece OUTER: interleaving two DoubleRow accumulation groups
            # (j inner per region) miscomputes on hardware -- each region's
            # 4-instruction group must run contiguously
            for a in range(0, Q, 256):
                b = min(a + 256, Q)
                for j in range(4):
                    nc.tensor.matmul(
                        pk[:, a:b],
                        lhsT=w8[:, j, :, mt * 128:(mt + 1) * 128],
                        rhs=x8_4d[:, j, :, s0 + a: s0 + b],
                        start=(j == 0), stop=(j == 3),
                        perf_mode=DRMODE)
            # w8 is host-scaled by 64 (w values ~0.02 sit in e4m3's
            # subnormal range, which the PE flushes to zero); undo here
            stage_f8(ci, mt, kind, pk, scale=1.0 / 64.0)

        def emit_proj_qk_interleaved(ci, mt):
            # startup projection: q first (wave 0 needs all 512 q columns),
            # then k in two pieces so wave 0 only gates on its first k-block
            s0, Q, _ = CHUNKS[ci]
            pq = ps.tile([128, QC], FP32, tag="proj", bufs=2,
                         name=f"pqi_{ci}_{mt}")
            pk = ps.tile([128, QC], FP32, tag="proj", bufs=2,
                         name=f"pki_{ci}_{mt}")
            pieces = [(pq, wqt, 0, 256), (pk, wkt, 0, 128),
                      (pq, wqt, 256, Q), (pk, wkt, 128, Q)]
            for dst, wt, a, b in pieces:
                for e in range(8):
                    nc.tensor.matmul(
                        dst[:, a:b],
                        lhsT=wt[:, e * M + mt * 128: e * M + (mt + 1) * 128],
                        rhs=xt[e][:, s0 + a: s0 + b],
                        start=(e == 0), stop=(e == 7))
                tgt = qTt if wt is wqt else kTt
                nc.vector.tensor_copy(tgt[mt][:, s0 + a: s0 + b],
                                      dst[:, a:b])
            stage_f8(ci, mt, "k", pk)

        def emit_proj_v(sblk):
            pv = ps.tile([128, M], FP32, tag="proj", bufs=2, name=f"pv_{sblk}")
            for e in range(8):
                nc.tensor.matmul(
                    pv[:],
                    lhsT=xt[e][:, sblk * 128:(sblk + 1) * 128],
                    rhs=wvt[:, e * M:(e + 1) * M],
                    start=(e == 0), stop=(e == 7))
            nc.vector.tensor_copy(
                v1_3d[:, sblk * HPC:(sblk + 1) * HPC, 0:64],
                pv[:].rearrange("p (h c) -> p h c", c=64))

        def proj_qk_pieces(ci):
            pcs = []
            for mt in range(2):
                pcs.append(lambda mt=mt: emit_proj_qk8(ci, mt, "q"))
                pcs.append(lambda mt=mt: emit_proj_qk8(ci, mt, "k"))
                pcs.append(lambda mt=mt: emit_rearrange(ci, mt))
            return pcs

        def proj_v_pieces(blks):
            return [lambda sb=sb: emit_proj_v(sb) for sb in blks]

        ob_tiles = {}
        out_3d = out.rearrange("(q p) f -> p q f", p=128)

        def emit_outproj(ci, qq, fc, tail=False):
            q0, Q, _ = CHUNKS[ci]
            nqb = Q // 128
            qb = q0 // 128 + qq
            last = ci == NCH - 1
            # last two chunks store per-q-block so the kernel-tail store
            # isn't queued behind one big merged transfer
            perqb = ci >= NCH - 2
            if qq == 0 and fc == 0:
                ob_tiles[ci] = outb.tile([128, nqb * E], BF16, tag="ob",
                                         name=f"ob_{qb}")
            ob = ob_tiles[ci]
            # tail outprojs borrow the scores PSUM slots (attention is done
            # by then), keeping mm->copy->mm free of slot serialization
            tag = "scores" if tail else "proj"
            po = ps.tile([128, QC], FP32, tag=tag, bufs=2,
                         name=f"po_{qb}_{fc}")
            for mc in range(2):
                nc.tensor.matmul(
                    po[:],
                    lhsT=ctxT[mc][:, qb * 128:(qb + 1) * 128],
                    rhs=wot[mc][:, fc * QC:(fc + 1) * QC],
                    start=(mc == 0), stop=(mc == 1))
            if last and fc == 1:
                # final piece: stage on the (idle by now) ACT engine so the
                # two last copies run in parallel instead of serializing on
                # the DVE queue
                nc.scalar.activation(
                    ob[:, qq * E + fc * QC: qq * E + (fc + 1) * QC], po[:],
                    mybir.ActivationFunctionType.Copy)
            else:
                nc.vector.tensor_copy(
                    ob[:, qq * E + fc * QC: qq * E + (fc + 1) * QC], po[:])
            if perqb:
                if fc == 1:
                    nc.sync.dma_start(
                        out[qb * 128:(qb + 1) * 128, :],
                        ob[:, qq * E: (qq + 1) * E])
                if (qq, fc) == (nqb - 1, 1):
                    del ob_tiles[ci]
            elif (qq, fc) == (nqb - 1, 1):
                nc.sync.dma_start(
                    out_3d[:, q0 // 128: q0 // 128 + nqb, :],
                    ob.rearrange("p (q f) -> p q f", f=E))
                del ob_tiles[ci]

        def outproj_pieces(ci, tail=False):
            _, Q, _ = CHUNKS[ci]
            return [lambda qq=qq, fc=fc: emit_outproj(ci, qq, fc, tail=tail)
                    for qq in range(Q // 128) for fc in range(2)]

        # ---- attention waves (one head PAIR, grp k-blocks) ----
        def wave_scores(ci, pair, g):
            s0, Q, grp = CHUNKS[ci]
            mt = pair
            fp8 = ci >= 1
            sc_ps = ps.tile([128, 2 * QC], FP32, tag="scores", bufs=2,
                            name=f"s_{ci}_{pair}_{g}")
            kds = [(j, (g * grp + j) * 128 - s0) for j in range(grp)]
            lo_e = 0
            for hh in range(2):
                r0 = hh * 64
                off = hh * QC
                for j, kd in kds:
                    kb = g * grp + j
                    lo = kd if (kd >= 128 and grp == 1) else 0
                    if hh == 0 and kd >= 128 and grp == 1:
                        lo_e = kd
                    if fp8:
                        # DoubleRow: dh 2x32 k-tiles, head at base 32*hh;
                        # moving free = 2*w caps piece width at 256
                        ck, koff = kb_loc(kb)
                        a = lo
                        while a < Q:
                            b = min(a + 256, Q)
                            nc.tensor.matmul(
                                sc_ps[:, off + j * Q + a: off + j * Q + b],
                                lhsT=qk8p5s[ck][32 * hh:32 * hh + 32, :, 1,
                                                mt, koff:koff + 128],
                                rhs=qk8p5s[ci][32 * hh:32 * hh + 32, :, 0,
                                               mt, a:b],
                                start=True, stop=True,
                                perf_mode=DRMODE)
                            a = b
                    elif (pair, g) != (0, 0):
                        nc.tensor.matmul(
                            sc_ps[:, off + j * Q + lo: off + (j + 1) * Q],
                            lhsT=kTt[mt][r0:r0 + 64, kb * 128:(kb + 1) * 128],
                            rhs=qTt[mt][r0:r0 + 64, s0 + lo: s0 + Q],
                            start=True, stop=True)
            if not fp8 and (pair, g) == (0, 0):
                # very first wave: scores in column pieces matching the
                # split startup projection, COLUMN-outer / head-inner (the
                # PE wait queue is FIFO, so a blocked later-column piece
                # must not sit in front of a ready first-column piece)
                for a, b in [(0, 256), (256, Q)]:
                    for hh in range(2):
                        r0, off = hh * 64, hh * QC
                        nc.tensor.matmul(
                            sc_ps[:, off + a: off + b],
                            lhsT=kTt[mt][r0:r0 + 64, 0:128],
                            rhs=qTt[mt][r0:r0 + 64, a:b],
                            start=True, stop=True)
            ex = expp.tile([128, 2 * QC], BF16, tag="ex",
                           name=f"e_{ci}_{pair}_{g}")
            W = grp * Q
            ex3 = ex.rearrange("p (h q) -> p h q", h=2)
            sc3 = sc_ps.rearrange("p (h q) -> p h q", h=2)
            if (ci, pair, g) == (0, 0, 0):
                # very first wave: exp per (head, column-half) in contiguous
                # slices (a strided 2-head AP flattens to a byte range that
                # would falsely depend on the later column pieces)
                for a, b in [(0, 256), (256, W)]:
                    for hh in range(2):
                        nc.scalar.activation(ex3[:, hh, a:b],
                                             sc3[:, hh, a:b],
                                             Exp, scale=SCALE)
            elif lo_e or W < QC:
                # both heads in one strided-AP call: the ACT engine charges
                # by total free size, so this halves the per-call init cost
                # vs one call per head
                nc.scalar.activation(ex3[:, :, lo_e:W], sc3[:, :, lo_e:W],
                                     Exp, scale=SCALE)
            else:
                nc.scalar.activation(ex[:], sc_ps[:], Exp, scale=SCALE)
            # stair mask on the diagonal 128-block: zero ex where
            # k_local > q_local. Pool affine_select (SBUF->SBUF), one call
            # covers both heads: keep where (q_local - k_partition) >= 0.
            for j, kd in kds:
                if kd >= 0:
                    nc.gpsimd.affine_select(
                        out=ex3[:, :, j * Q + kd: j * Q + kd + 128],
                        in_=ex3[:, :, j * Q + kd: j * Q + kd + 128],
                        compare_op=mybir.AluOpType.is_ge,
                        fill=0.0, base=0,
                        pattern=[[0, 2], [1, 128]],
                        channel_multiplier=-1,
                    )
            return ex

        def wave_ctx_flip(ci, pair, g, ex, ctx_pair, nqb):
            # flipped ctx: lhsT = ex q-window (stationary), rhs = v1 slab,
            # out = [q-part 128, 65] accumulated over kb. qb's last
            # contribution is its diagonal block.
            # start=True ONLY on the tile's first matmul: start marks the
            # whole 2KB PSUM zero-region pending-zero, so a second start
            # would corrupt sibling q-blocks' accumulations. Later q-blocks'
            # first writes zero-on-first-write via that same pending flag.
            s0, Q, grp = CHUNKS[ci]
            qb_base = s0 // 128
            for hh in range(2):
                h = 2 * pair + hh
                off = hh * QC
                for j in range(grp):
                    kb = g * grp + j
                    kd = kb * 128 - s0
                    qb0 = max(0, kd // 128)
                    for qb in range(qb0, nqb):
                        nc.tensor.matmul(
                            ctx_pair[hh][:, qb * 128: qb * 128 + 65],
                            lhsT=ex[:, off + j * Q + qb * 128:
                                    off + j * Q + qb * 128 + 128],
                            rhs=v1_3d[:, kb * HPC + h, :],
                            start=(kb == 0 and qb == 0),
                            stop=(kb == qb_base + qb),
                            skip_group_check=True)

        def flip_norm_pieces(ci, pair, items):
            # flipped-ctx norm: per head a [128, nqb] reciprocal of the
            # per-q-partition denominators (col 64 of each qb slice), then
            # per (head, qb) a tensor_scalar multiply into the ctx_qm
            # staging tile, then one XBAR dma-transpose per q-block into
            # ctxT. Spread over the next waves.
            s0, Q, _ = CHUNKS[ci]
            nqb = Q // 128
            qb_lo = s0 // 128
            state = {}
            cqm = cqm_pool.tile([128, nqb * 128], BF16, tag="cqm",
                                name=f"cqm_{ci}_{pair}")

            def p_recips():
                state["recs"] = []
                for h, ctx_ps in items:
                    rec = scr.tile([128, 4], FP32, tag="rec",
                                   name=f"r_{ci}_{h}")
                    c3 = ctx_ps.rearrange("p (qb c) -> p qb c", c=128)
                    r3 = rec.rearrange("p (a b) -> p a b", b=1)
                    nc.vector.reciprocal(r3[:, 0:nqb, :],
                                         c3[:, 0:nqb, 64:65])
                    state["recs"].append(rec)

            def p_muls(i):
                h, ctx_ps = items[i]
                hh = h % 2
                rec = state["recs"][i]
                for qb in range(nqb):
                    nc.vector.tensor_scalar_mul(
                        cqm[:, qb * 128 + hh * 64: qb * 128 + hh * 64 + 64],
                        ctx_ps[:, qb * 128: qb * 128 + 64],
                        rec[:, qb:qb + 1])

            def p_transposes(qbs):
                # PE transpose (cheap: 128 rows each) into a bf16 PSUM tile
                # riding the proj slot rotation, then a DVE copy into ctxT.
                # Avoids the SP/HWDGE queue entirely (in-order SP.SEQ would
                # head-of-line-block later rearrange DMA issues).
                for qb in qbs:
                    tp = ps.tile([128, 128], BF16, tag="proj", bufs=2,
                                 name=f"tp_{ci}_{pair}_{qb}")
                    nc.tensor.transpose(
                        tp[:], cqm[:, qb * 128:(qb + 1) * 128], ident[:])
                    nc.vector.tensor_copy(
                        ctxT[pair][:, (qb_lo + qb) * 128:
                                   (qb_lo + qb + 1) * 128], tp[:])

            cut = min(2, nqb)
            return [p_recips, lambda: p_muls(0), lambda: p_muls(1),
                    lambda: p_transposes(range(0, cut)),
                    lambda: p_transposes(range(cut, nqb))]

        # ---- main schedule ----
        emit_proj_qk_interleaved(0, 0)
        pending_norms = []
        for ci in range(NCH):
            q0, Q, grp = CHUNKS[ci]
            nkb = (q0 + Q) // 128
            nqb = Q // 128
            ngrp = nkb // grp
            waves = [(pair, g) for pair in range(2)
                     for g in range(ngrp)]
            head = []   # pieces pinned to the earliest waves, one per wave
            extra = []  # pieces distributed evenly over all waves
            pins = {}   # wave -> pieces with exact placement constraints
            if ci == 0:
                # chunk 1's fp8 prep is pinned to the earliest waves so its
                # scores (the ACT feed during the xt input transfers) start
                # the moment x8[1] lands; chunk 2's prep spreads behind it
                # chunk 1's mt0 prep FIRST (ahead of chunk 0's mt1 startup
                # proj in the 2-slot proj PSUM rotation): it gates chunk 1's
                # scores, the main ACT feed once chunk 0's thin exps end
                qk1 = proj_qk_pieces(1)
                v0 = proj_v_pieces(range(0, 4))
                pins = {0: [qk1[0], qk1[1], qk1[2], v0[0],
                            lambda: emit_xt_load(1)],
                        1: [lambda: emit_proj_qk_interleaved(0, 1),
                            lambda: emit_rearrange(0, 0, k_only=True),
                            v0[1]],
                        2: [qk1[3], qk1[4], qk1[5], v0[2],
                            lambda: emit_rearrange(0, 1, k_only=True)],
                        3: [v0[3]]}
                extra += proj_qk_pieces(2) + [lambda: emit_xt_load(2)]
            elif ci == 1:
                head += proj_v_pieces(range(4, 8))
                extra += (proj_qk_pieces(3) + [lambda: emit_xt_load(3),
                                               emit_wot_load])
            elif ci == 2:
                head += proj_v_pieces(range(8, 12))
                extra += proj_qk_pieces(4) + outproj_pieces(0)
            elif ci == 3:
                head += proj_v_pieces(range(12, 16))
                extra += outproj_pieces(1) + outproj_pieces(2)
            else:
                extra += outproj_pieces(3)
            sched = {w: [] for w in range(len(waves))}
            for w, pcs in pins.items():
                sched[w].extend(pcs)
            for j, pc in enumerate(head):
                sched[j].append(pc)
            if extra:
                w0 = max(0, min(2 if ci == 0 else (3 if ci == NCH - 1 else 4),
                                len(waves) - len(extra)))
                span_w = len(waves) - w0
                for j, pc in enumerate(extra):
                    sched[w0 + j * span_w // len(extra)].append(pc)

            ctx_tiles = {}
            ctx_queue = []
            for w, (pair, g) in enumerate(waves):
                if g == 0:
                    # one PSUM bank per head: [128, nqb*128-float slices],
                    # 65 floats used per qb slice
                    ctx_tiles[pair] = [
                        ps.tile([128, QC], FP32, tag="ctx", bufs=2,
                                name=f"c_{ci}_{pair}_{hh}")
                        for hh in range(2)]
                ex = wave_scores(ci, pair, g)
                # two pieces per wave: the previous pair's transposes must
                # all be emitted before any outproj piece that reads them
                for _ in range(2):
                    if pending_norms:
                        pending_norms.pop(0)()
                last_of_pair = g == ngrp - 1
                final_pair = last_of_pair and pair == 1 and ci + 1 == NCH
                if not final_pair:
                    for pc in sched[w]:
                        pc()
                ctx_queue.append((pair, g, ex))
                # defer ctx so the PE has scores to run while exp catches
                # up; drain continuously (small lag) so the pair-end flush
                # is small and the norm reciprocal doesn't head-of-line-
                # block the in-order DVE queue.
                lag = max(0, 3 - g) if grp > 1 else 4
                while len(ctx_queue) > lag or \
                        (ctx_queue and last_of_pair):
                    qpair, qg, qex = ctx_queue.pop(0)
                    wave_ctx_flip(ci, qpair, qg, qex, ctx_tiles[qpair], nqb)
                if last_of_pair:
                    h0 = 2 * pair
                    items = [(h0 + hh, ctx_tiles[pair][hh])
                             for hh in range(2)]
                    while pending_norms:  # drain leftovers before reassign
                        pending_norms.pop(0)()
                    if final_pair:
                        # kernel tail: emit the whole norm + transpose chain
                        # now, ahead of this wave's filler pieces
                        for pc in flip_norm_pieces(ci, pair, items):
                            pc()
                        for pc in sched[w]:
                            pc()
                    else:
                        pending_norms = flip_norm_pieces(ci, pair, items)
        for pc in outproj_pieces(NCH - 1, tail=True):
            pc()


def build_module():
    nc = bacc.Bacc("TRN2", target_bir_lowering=False, debug=False)
    xT = nc.dram_tensor("xT", [E, S], BF16, kind="ExternalInput").ap()
    wq = nc.dram_tensor("wq", [E, M], BF16, kind="ExternalInput").ap()
    wk = nc.dram_tensor("wk", [E, M], BF16, kind="ExternalInput").ap()
    wv = nc.dram_tensor("wv", [E, M], BF16, kind="ExternalInput").ap()
    wo = nc.dram_tensor("wo", [M, E], BF16, kind="ExternalInput").ap()
    x8 = nc.dram_tensor("x8", [128, 8 * S], F8, kind="ExternalInput").ap()
    wk8 = nc.dram_tensor("wk8", [128, 8 * M], F8, kind="ExternalInput").ap()
    wq8 = nc.dram_tensor("wq8", [128, 8 * M], F8, kind="ExternalInput").ap()
    out = nc.dram_tensor("out", [S, E], BF16, kind="ExternalOutput").ap()
    with tile.TileContext(nc) as tc:
        _emit_kernel(tc, xT, wq, wk, wv, wo, x8, wk8, wq8, out)
    nc.compile()
    return nc


def _pack_epairs(aT):
    """[E, N] -> [128, 4*2*N] fp8: e-tile pairs side by side per partition
    (DoubleRow packing: out[p, j, t, n] = aT[(2j+t)*128 + p, n])."""
    e4m3 = ml_dtypes.float8_e4m3
    E_, N = aT.shape
    a = np.asarray(aT, dtype=np.float32).reshape(4, 2, 128, N)
    a = np.ascontiguousarray(a.transpose(2, 0, 1, 3)).astype(e4m3)
    return a.reshape(128, 8 * N)


def make_in_maps(x, w_qkv):
    """Per-core input dicts (bf16/fp8, pre-transposed host-side)."""
    bf = ml_dtypes.bfloat16
    xTb = [np.ascontiguousarray(x[b].T).astype(bf) for b in range(B)]
    x8b = [_pack_epairs(x[b].T) for b in range(B)]
    in_maps = []
    for c in range(NCORES):
        b, g = c // 4, c % 4
        cols = slice(g * M, (g + 1) * M)
        wqT = np.ascontiguousarray(w_qkv[cols, :].T)
        wkT = np.ascontiguousarray(w_qkv[E:][cols, :].T)
        in_maps.append({
            "xT": xTb[b],
            "wq": wqT.astype(bf),
            "wk": wkT.astype(bf),
            "wv": np.ascontiguousarray(w_qkv[2 * E:][cols, :].T).astype(bf),
            "x8": x8b[b],
            "wk8": _pack_epairs(wkT * 64.0),
            "wq8": _pack_epairs(wqT * 64.0),
            "wo": None,  # filled in kernel(), needs w_out
        })
    return in_maps


_RUNNER = None
_SHARDED = None


def _get_runner():
    """Build the Bass module once and return a cached callable
    (in_maps) -> [NCORES, S, E] bf16 partial outputs."""
    global _RUNNER
    if _RUNNER is not None:
        return _RUNNER

    nc = build_module()

    from concourse import bass2jax
    import jax
    from jax.sharding import Mesh, PartitionSpec
    from jax.experimental.shard_map import shard_map

    bass2jax.install_neuronx_cc_hook()

    in_names = ["xT", "wq", "wk", "wv", "x8", "wk8", "wq8", "wo"]
    out_names = ["out"]
    out_avals = [jax.core.ShapedArray((S, E), ml_dtypes.bfloat16)]
    n_params = len(in_names)
    all_names = in_names + out_names
    partition_name = (nc.partition_id_tensor.name
                      if nc.partition_id_tensor is not None else None)
    if partition_name is not None:
        all_names = all_names + [partition_name]

    def _body(*args):
        operands = list(args)
        if partition_name is not None:
            operands.append(bass2jax.partition_id_tensor())
        outs = bass2jax._bass_exec_p.bind(
            *operands,
            out_avals=tuple(out_avals),
            in_names=tuple(all_names),
            out_names=tuple(out_names),
            lowering_input_output_aliases=(),
            sim_require_finite=True,
            sim_require_nnan=True,
            nc=nc,
        )
        return tuple(outs)

    devices = jax.devices()[:NCORES]
    mesh = Mesh(np.asarray(devices), ("core",))
    n_outs = len(out_names)
    in_specs = (PartitionSpec("core"),) * (n_params + n_outs)
    out_specs = (PartitionSpec("core"),) * n_outs
    sharded = jax.jit(
        shard_map(_body, mesh=mesh, in_specs=in_specs, out_specs=out_specs,
                  check_rep=False),
        donate_argnums=tuple(range(n_params, n_params + n_outs)),
        keep_unused=True,
    )
    global _SHARDED
    _SHARDED = sharded

    def run(in_maps):
        concat_in = [
            np.concatenate([np.asarray(in_maps[c][n]) for c in range(NCORES)],
                           axis=0)
            for n in in_names
        ]
        concat_zeros = [np.zeros((NCORES * S, E), ml_dtypes.bfloat16)]
        out_arrs = sharded(*concat_in, *concat_zeros)
        return np.asarray(out_arrs[0]).reshape(NCORES, S, E)

    _RUNNER = run
    return run


def kernel(x, w_qkv, w_out, b_out):
    x = np.asarray(x, dtype=np.float32)
    w_qkv = np.asarray(w_qkv, dtype=np.float32)
    w_out = np.asarray(w_out, dtype=np.float32)
    b_out = np.asarray(b_out, dtype=np.float32)

    bf = ml_dtypes.bfloat16
    in_maps = make_in_maps(x, w_qkv)
    for c in range(NCORES):
        g = c % 4
        cols = slice(g * M, (g + 1) * M)
        in_maps[c]["wo"] = np.ascontiguousarray(w_out[:, cols].T).astype(bf)

    run = _get_runner()
    partials = run(in_maps)  # [8, S, E] bf16

    out = np.empty((B, S, E), np.float32)
    for b in range(B):
        acc = partials[4 * b].astype(np.float64)
        for i in range(1, 4):
            acc += partials[4 * b + i].astype(np.float64)
        out[b] = (acc + b_out.astype(np.float64)).astype(np.float32)
    return out


# revision 50
# speedup vs baseline: 1.0549x; 1.0435x over previous
"""Multi-head causal self-attention (B=2, S=2048, E=1024, H=16, D=64) on 8
Trainium2 NeuronCores.

Sharding: batch x head-group. Core c handles batch (c // 4) and heads
[4*(c%4), 4*(c%4)+4). Each core computes QKV projection for its 4 heads,
causal flash-attention, and a partial output projection over its head
columns. Host sums the 4 partial outputs per batch and adds b_out.

v4 changes vs v3:
  - flipped ctx matmuls for chunks 0-3: out = [q-part 128, 65] with
    lhsT = ex q-window (stationary), rhs = v1 slab. PE cost per
    (head, kb, qb) drops from `cols` to 65 rows (full 128-partition
    output). The softmax denominator lands per-q-PARTITION, so the norm
    is a cheap [128,nqb] reciprocal + per-partition-scalar multiplies
    (no Pool partition_broadcast).
  - ctx_qm [q, m] bf16 is transposed back to ctxT [m, q] with
    dma_start_transpose (XBAR), one [128,128] tile per (pair, q-block).
  - chunk 4 (last 128 cols) keeps the v3 unflipped path so the kernel
    tail avoids the DMA-transpose latency.
  - q projection for chunks 1-4 via fp8 DoubleRow (host-packed wq8,
    x8), mirroring the k8 path: 1/4 the PE cost of the bf16 proj.
  - stair masking of ex moved from DVE tensor_mul to Pool affine_select
    (SBUF->SBUF, one call covers both heads of a wave).
"""

import sys

if "/opt/trn_rl_repo" not in sys.path:
    sys.path.insert(0, "/opt/trn_rl_repo")

import numpy as np
import ml_dtypes

import concourse.bacc as bacc
import concourse.mybir as mybir
import concourse.tile as tile

BF16 = mybir.dt.bfloat16
FP32 = mybir.dt.float32
F8 = mybir.dt.float8e4
DRMODE = mybir.MatmulPerfMode.DoubleRow

B, S, E = 2, 2048, 1024
H, DH = 16, 64
NCORES = 8
HPC = 4            # heads per core
M = HPC * DH       # 256 ctx columns per core
QC = 512           # q chunk (max wave width; also PSUM head stride)
KB = 128           # k block
SCALE = 1.0 / np.sqrt(DH)
NWARM = 64         # warmup dummy matmuls (128 cols each)
# q-chunks (q0, Q, grp). The last 512 splits 384+128 so the final
# norm/outproj tail is 4x smaller. grp = k-blocks per wave for the narrow
# final chunk (shares one exp call across 4 k-blocks).
CHUNKS = [(0, 512, 1), (512, 512, 1), (1024, 512, 1),
          (1536, 384, 1), (1920, 128, 4)]
NCH = len(CHUNKS)


def _emit_kernel(tc, xT, wq, wk, wv, wo_d, x8, wk8, wq8, out):
    nc = tc.nc
    Exp = mybir.ActivationFunctionType.Exp

    with tc.tile_pool(name="res", bufs=1) as res, \
         tc.tile_pool(name="ps", bufs=1, space="PSUM") as ps, \
         tc.tile_pool(name="expp", bufs=16) as expp, \
         tc.tile_pool(name="scr", bufs=4) as scr, \
         tc.tile_pool(name="cqm", bufs=2) as cqm_pool, \
         tc.tile_pool(name="outb", bufs=2) as outb:

        # ---- resident SBUF tiles ----
        xt_all = res.tile([128, 8 * S], BF16, name="xt_all")
        xt = [xt_all[:, e * S:(e + 1) * S] for e in range(8)]
        xt_3d = xt_all.rearrange("p (e s) -> p e s", s=S)
        wqt = res.tile([128, 8 * M], BF16, name="wqt")
        wkt = res.tile([128, 8 * M], BF16, name="wkt")
        wvt = res.tile([128, 8 * M], BF16, name="wvt")
        wot = [res.tile([128, E], BF16, name=f"wot{i}") for i in range(2)]
        qTt = [res.tile([128, S], BF16, name=f"qTt{i}") for i in range(2)]
        kTt = [res.tile([128, S], BF16, name=f"kTt{i}") for i in range(2)]
        ctxT = [res.tile([128, S], BF16, name=f"ctxT{i}") for i in range(2)]
        # fp8 scores path (queries >= 512): per chunk one classic-layout fp8
        # staging tile (free dims qk x mt x s) and one DoubleRow "pair" tile
        # [64, j x qk x mt x s] with head parity on partition halves {0,32}
        # and the two dh-32 k-tiles (j) in the free dim. PER-CHUNK tiles:
        # the dependency tracker flattens strided APs to byte ranges, so a
        # single shared tile makes chunk ci's scores falsely wait on chunk
        # ci+1's rearrange DMAs.
        qk8cs = [res.tile([128, 4 * CH[1]], F8, name=f"qk8c{i}")
                 for i, CH in enumerate(CHUNKS)]
        qk8c4s = [t.rearrange("p (t m s) -> p t m s", t=2, m=2)
                  for t in qk8cs]
        qk8ps = [res.tile([64, 8 * CH[1]], F8, name=f"qk8p{i}")
                 for i, CH in enumerate(CHUNKS)]
        qk8p5s = [t.rearrange("p (j t m s) -> p j t m s", j=2, t=2, m=2)
                  for t in qk8ps]

        def kb_loc(kb):
            # global k-block -> (chunk index, local column offset)
            for i in range(NCH - 1, -1, -1):
                if kb * 128 >= CHUNKS[i][0]:
                    return i, kb * 128 - CHUNKS[i][0]
            raise AssertionError
        # fp8 projection operands (host-packed e-pair layout): q/k columns
        # that are only ever consumed by the fp8 score path are projected
        # with fp8 DoubleRow matmuls at 1/4 the PE cost
        x8t = res.tile([128, 8 * S], F8, name="x8t")
        x8_4d = x8t.rearrange("p (j t s) -> p j t s", j=4, t=2)
        wk8t = res.tile([128, 8 * M], F8, name="wk8t")
        wk8_4d = wk8t.rearrange("p (j t m) -> p j t m", j=4, t=2)
        wq8t = res.tile([128, 8 * M], F8, name="wq8t")
        wq8_4d = wq8t.rearrange("p (j t m) -> p j t m", j=4, t=2)
        # V with ones column: per (k-block kb, head h) a [128, 65] slab
        v1 = res.tile([128, (S // KB) * HPC * 65], BF16, name="v1")
        v1_3d = v1.rearrange("p (n c) -> p n c", c=65)
        warm_src = res.tile([128, 128], BF16, name="warm_src")
        ident = res.tile([128, 128], BF16, name="ident")

        # ---- warmup: dummy matmuls keep the PE busy (and its p-state
        # ramping) through the DMA-gated startup.
        nc.gpsimd.memset(warm_src[:], 0.0)
        warm_ps = ps.tile([128, QC], FP32, tag="proj", bufs=2, name="warm_ps")
        for i in range(NWARM):
            nc.tensor.matmul(
                warm_ps[:, 0:128], lhsT=warm_src[:], rhs=warm_src[:],
                start=True, stop=True)

        # ---- input DMA: one batched transfer per tensor/chunk ----
        wqt_3d = wqt.rearrange("p (e m) -> p e m", m=M)
        wkt_3d = wkt.rearrange("p (e m) -> p e m", m=M)
        wvt_3d = wvt.rearrange("p (e m) -> p e m", m=M)
        xT_3d = xT.rearrange("(e p) s -> p e s", p=128)
        # order: chunk-0 bf16 operands first (pqi/pki), then the fp8
        # operands for ALL chunks (small; chunks 1+ exp work can only start
        # once q8/k8 are projected + rearranged, and that work is what keeps
        # the ACT engine fed during the remaining xt transfers), then the
        # bf16 x chunks (V projections, consumed later) and wo.
        nc.sync.dma_start(wqt_3d[:], wq.rearrange("(e p) m -> p e m", p=128))
        nc.sync.dma_start(xt_3d[:, :, 0:256], xT_3d[:, :, 0:256])
        nc.sync.dma_start(wkt_3d[:], wk.rearrange("(e p) m -> p e m", p=128))
        nc.sync.dma_start(xt_3d[:, :, 256:QC], xT_3d[:, :, 256:QC])
        x8_dram = x8.rearrange("p (j t s) -> p j t s", j=4, t=2)
        nc.sync.dma_start(wq8t[:], wq8)
        nc.sync.dma_start(wk8t[:], wk8)

        def emit_x8_load(chunk):
            nc.sync.dma_start(
                x8_4d[:, :, :, chunk * QC:(chunk + 1) * QC],
                x8_dram[:, :, :, chunk * QC:(chunk + 1) * QC])

        emit_x8_load(1)
        nc.sync.dma_start(wvt_3d[:], wv.rearrange("(e p) m -> p e m", p=128))
        emit_x8_load(2)
        emit_x8_load(3)

        # xt[1..3] and wot are consumed late (V projections of later chunks,
        # outproj). Their dma_starts are deferred into the wave schedule so
        # the per-chunk qk8p rearranges (which gate each chunk's scores and
        # hence the ACT-bound steady state) aren't queued behind them on the
        # serial DMA device.
        def emit_xt_load(chunk):
            nc.sync.dma_start(
                xt_3d[:, :, chunk * QC:(chunk + 1) * QC],
                xT_3d[:, :, chunk * QC:(chunk + 1) * QC])

        def emit_wot_load():
            for i in range(2):
                nc.sync.dma_start(wot[i][:], wo_d[i * 128:(i + 1) * 128, :])

        # ---- constants ----
        # exp-table warm first: the ~2.7us table load + warm call must not
        # queue behind the big v1 memset on the Pool engine
        warm = scr.tile([1, 1], FP32, tag="warm", bufs=1, name="warm")
        nc.gpsimd.memset(warm[:], 0.0)
        nc.scalar.activation(warm[:], warm[:],
                             mybir.ActivationFunctionType.Exp)
        nc.gpsimd.memset(v1[:], 1.0)  # data columns overwritten by V proj
        # identity matrix for PE transposes (keep where q_local == partition)
        nc.gpsimd.memset(ident[:], 1.0)
        nc.gpsimd.affine_select(
            out=ident[:], in_=ident[:],
            compare_op=mybir.AluOpType.is_equal,
            fill=0.0, base=0,
            pattern=[[1, 128]],
            channel_multiplier=-1,
        )

        # ---- emission helpers ----
        def stage_f8(ci, mt, kind, pqk, scale=None):
            # fp8 classic staging into the (qk, mt) slab of chunk ci's qk8c
            s0, Q, _ = CHUNKS[ci]
            t = 0 if kind == "q" else 1
            if scale is None:
                nc.vector.tensor_copy(qk8c4s[ci][:, t, mt, 0:Q], pqk[:, 0:Q])
            else:
                nc.vector.tensor_scalar_mul(qk8c4s[ci][:, t, mt, 0:Q],
                                            pqk[:, 0:Q], scale)

        def emit_rearrange(ci, mt, k_only=False):
            # partition rearrange into the DoubleRow pair tile for chunk
            # ci's mt slab (per-mt so pair 0's scores aren't gated on the
            # mt=1 projections)
            s0, Q, _ = CHUNKS[ci]
            t0 = 1 if k_only else 0
            for hh in range(2):
                for j in range(2):
                    nc.sync.dma_start(
                        qk8p5s[ci][32 * hh:32 * hh + 32, j, t0:2,
                                   mt:mt + 1, 0:Q],
                        qk8c4s[ci][64 * hh + 32 * j: 64 * hh + 32 * j + 32,
                                   t0:2, mt:mt + 1, 0:Q])

        def stage_qk(ci, mt, kind, pqk):
            # chunk 0 queries score in bf16 (classic layout); all other
            # queries score in fp8 DoubleRow. k is needed in fp8 by every
            # fp8 chunk, and in bf16 only for chunk 0's k-blocks.
            s0, Q, _ = CHUNKS[ci]
            dstt = qTt if kind == "q" else kTt
            if ci == 0:
                nc.vector.tensor_copy(dstt[mt][:, s0:s0 + Q], pqk[:, 0:Q])
            if kind == "k" or ci >= 1:
                stage_f8(ci, mt, kind, pqk)

        def emit_proj_qk8(ci, mt, kind):
            # q/k projection for fp8-only consumers via fp8 DoubleRow over
            # host-packed e-pairs: 1/4 the PE cost of the bf16 projection
            s0, Q, _ = CHUNKS[ci]
            w8 = wq8_4d if kind == "q" else wk8_4d
            pk = ps.tile([128, QC], FP32, tag="proj", bufs=2,
                         name=f"p8{kind}_{ci}_{mt}")
            # a-piece OUTER: interleaving two DoubleRow accumulation groups
            # (j inner per region) miscomputes on hardware -- each region's
            # 4-instruction group must run contiguously
            for a in range(0, Q, 256):
                b = min(a + 256, Q)
                for j in range(4):
                    nc.tensor.matmul(
                        pk[:, a:b],
                        lhsT=w8[:, j, :, mt * 128:(mt + 1) * 128],
                        rhs=x8_4d[:, j, :, s0 + a: s0 + b],
                        start=(j == 0), stop=(j == 3),
                        perf_mode=DRMODE)
            # w8 is host-scaled by 64 (w values ~0.02 sit in e4m3's
            # subnormal range, which the PE flushes to zero); undo here
            stage_f8(ci, mt, kind, pk, scale=1.0 / 64.0)

        def emit_proj_qk_interleaved(ci, mt):
            # startup projection: q first (wave 0 needs all 512 q columns),
            # then k in two pieces so wave 0 only gates on its first k-block
            s0, Q, _ = CHUNKS[ci]
            pq = ps.tile([128, QC], FP32, tag="proj", bufs=2,
                         name=f"pqi_{ci}_{mt}")
            pk = ps.tile([128, QC], FP32, tag="proj", bufs=2,
                         name=f"pki_{ci}_{mt}")
            pieces = [(pq, wqt, 0, 256), (pk, wkt, 0, 128),
                      (pq, wqt, 256, Q), (pk, wkt, 128, Q)]
            for dst, wt, a, b in pieces:
                for e in range(8):
                    nc.tensor.matmul(
                        dst[:, a:b],
                        lhsT=wt[:, e * M + mt * 128: e * M + (mt + 1) * 128],
                        rhs=xt[e][:, s0 + a: s0 + b],
                        start=(e == 0), stop=(e == 7))
                tgt = qTt if wt is wqt else kTt
                nc.vector.tensor_copy(tgt[mt][:, s0 + a: s0 + b],
                                      dst[:, a:b])
            stage_f8(ci, mt, "k", pk)

        def emit_proj_v(sblk):
            pv = ps.tile([128, M], FP32, tag="proj", bufs=2, name=f"pv_{sblk}")
            for e in range(8):
                nc.tensor.matmul(
                    pv[:],
                    lhsT=xt[e][:, sblk * 128:(sblk + 1) * 128],
                    rhs=wvt[:, e * M:(e + 1) * M],
                    start=(e == 0), stop=(e == 7))
            nc.vector.tensor_copy(
                v1_3d[:, sblk * HPC:(sblk + 1) * HPC, 0:64],
                pv[:].rearrange("p (h c) -> p h c", c=64))

        def proj_qk_pieces(ci):
            pcs = []
            for mt in range(2):
                pcs.append(lambda mt=mt: emit_proj_qk8(ci, mt, "q"))
                pcs.append(lambda mt=mt: emit_proj_qk8(ci, mt, "k"))
                pcs.append(lambda mt=mt: emit_rearrange(ci, mt))
            return pcs

        def proj_v_pieces(blks):
            return [lambda sb=sb: emit_proj_v(sb) for sb in blks]

        ob_tiles = {}
        out_3d = out.rearrange("(q p) f -> p q f", p=128)

        def emit_outproj(ci, qq, fc, tail=False):
            q0, Q, _ = CHUNKS[ci]
            nqb = Q // 128
            qb = q0 // 128 + qq
            last = ci == NCH - 1
            # last two chunks store per-q-block so the kernel-tail store
            # isn't queued behind one big merged transfer
            perqb = ci >= NCH - 2
            if qq == 0 and fc == 0:
                ob_tiles[ci] = outb.tile([128, nqb * E], BF16, tag="ob",
                                         name=f"ob_{qb}")
            ob = ob_tiles[ci]
            # tail outprojs borrow the scores PSUM slots (attention is done
            # by then), keeping mm->copy->mm free of slot serialization
            tag = "scores" if tail else "proj"
            po = ps.tile([128, QC], FP32, tag=tag, bufs=2,
                         name=f"po_{qb}_{fc}")
            for mc in range(2):
                nc.tensor.matmul(
                    po[:],
                    lhsT=ctxT[mc][:, qb * 128:(qb + 1) * 128],
                    rhs=wot[mc][:, fc * QC:(fc + 1) * QC],
                    start=(mc == 0), stop=(mc == 1))
            if last and fc == 1:
                # final piece: stage on the (idle by now) ACT engine so the
                # two last copies run in parallel instead of serializing on
                # the DVE queue
                nc.scalar.activation(
                    ob[:, qq * E + fc * QC: qq * E + (fc + 1) * QC], po[:],
                    mybir.ActivationFunctionType.Copy)
            else:
                nc.vector.tensor_copy(
                    ob[:, qq * E + fc * QC: qq * E + (fc + 1) * QC], po[:])
            if perqb:
                if fc == 1:
                    nc.sync.dma_start(
                        out[qb * 128:(qb + 1) * 128, :],
                        ob[:, qq * E: (qq + 1) * E])
                if (qq, fc) == (nqb - 1, 1):
                    del ob_tiles[ci]
            elif (qq, fc) == (nqb - 1, 1):
                nc.sync.dma_start(
                    out_3d[:, q0 // 128: q0 // 128 + nqb, :],
                    ob.rearrange("p (q f) -> p q f", f=E))
                del ob_tiles[ci]

        def outproj_pieces(ci, tail=False):
            _, Q, _ = CHUNKS[ci]
            return [lambda qq=qq, fc=fc: emit_outproj(ci, qq, fc, tail=tail)
                    for qq in range(Q // 128) for fc in range(2)]

        # ---- attention waves (one head PAIR, grp k-blocks) ----
        def wave_scores(ci, pair, g):
            s0, Q, grp = CHUNKS[ci]
            mt = pair
            fp8 = ci >= 1
            sc_ps = ps.tile([128, 2 * QC], FP32, tag="scores", bufs=2,
                            name=f"s_{ci}_{pair}_{g}")
            kds = [(j, (g * grp + j) * 128 - s0) for j in range(grp)]
            lo_e = 0
            for hh in range(2):
                r0 = hh * 64
                off = hh * QC
                for j, kd in kds:
                    kb = g * grp + j
                    lo = kd if (kd >= 128 and grp == 1) else 0
                    if hh == 0 and kd >= 128 and grp == 1:
                        lo_e = kd
                    if fp8:
                        # DoubleRow: dh 2x32 k-tiles, head at base 32*hh;
                        # moving free = 2*w caps piece width at 256
                        ck, koff = kb_loc(kb)
                        a = lo
                        while a < Q:
                            b = min(a + 256, Q)
                            nc.tensor.matmul(
                                sc_ps[:, off + j * Q + a: off + j * Q + b],
                                lhsT=qk8p5s[ck][32 * hh:32 * hh + 32, :, 1,
                                                mt, koff:koff + 128],
                                rhs=qk8p5s[ci][32 * hh:32 * hh + 32, :, 0,
                                               mt, a:b],
                                start=True, stop=True,
                                perf_mode=DRMODE)
                            a = b
                    elif (pair, g) != (0, 0):
                        nc.tensor.matmul(
                            sc_ps[:, off + j * Q + lo: off + (j + 1) * Q],
                            lhsT=kTt[mt][r0:r0 + 64, kb * 128:(kb + 1) * 128],
                            rhs=qTt[mt][r0:r0 + 64, s0 + lo: s0 + Q],
                            start=True, stop=True)
            if not fp8 and (pair, g) == (0, 0):
                # very first wave: scores in column pieces matching the
                # split startup projection, COLUMN-outer / head-inner (the
                # PE wait queue is FIFO, so a blocked later-column piece
                # must not sit in front of a ready first-column piece)
                for a, b in [(0, 256), (256, Q)]:
                    for hh in range(2):
                        r0, off = hh * 64, hh * QC
                        nc.tensor.matmul(
                            sc_ps[:, off + a: off + b],
                            lhsT=kTt[mt][r0:r0 + 64, 0:128],
                            rhs=qTt[mt][r0:r0 + 64, a:b],
                            start=True, stop=True)
            ex = expp.tile([128, 2 * QC], BF16, tag="ex",
                           name=f"e_{ci}_{pair}_{g}")
            W = grp * Q
            ex3 = ex.rearrange("p (h q) -> p h q", h=2)
            sc3 = sc_ps.rearrange("p (h q) -> p h q", h=2)
            if (ci, pair, g) == (0, 0, 0):
                # very first wave: exp per (head, column-half) in contiguous
                # slices (a strided 2-head AP flattens to a byte range that
                # would falsely depend on the later column pieces)
                for a, b in [(0, 256), (256, W)]:
                    for hh in range(2):
                        nc.scalar.activation(ex3[:, hh, a:b],
                                             sc3[:, hh, a:b],
                                             Exp, scale=SCALE)
            elif lo_e or W < QC:
                # both heads in one strided-AP call: the ACT engine charges
                # by total free size, so this halves the per-call init cost
                # vs one call per head
                nc.scalar.activation(ex3[:, :, lo_e:W], sc3[:, :, lo_e:W],
                                     Exp, scale=SCALE)
            else:
                nc.scalar.activation(ex[:], sc_ps[:], Exp, scale=SCALE)
            # stair mask on the diagonal 128-block: zero ex where
            # k_local > q_local. Pool affine_select (SBUF->SBUF), one call
            # covers both heads: keep where (q_local - k_partition) >= 0.
            for j, kd in kds:
                if kd >= 0:
                    nc.gpsimd.affine_select(
                        out=ex3[:, :, j * Q + kd: j * Q + kd + 128],
                        in_=ex3[:, :, j * Q + kd: j * Q + kd + 128],
                        compare_op=mybir.AluOpType.is_ge,
                        fill=0.0, base=0,
                        pattern=[[0, 2], [1, 128]],
                        channel_multiplier=-1,
                    )
            return ex

        def wave_ctx_flip(ci, pair, g, ex, ctx_pair, nqb):
            # flipped ctx: lhsT = ex q-window (stationary), rhs = v1 slab,
            # out = [q-part 128, 65] accumulated over kb. qb's last
            # contribution is its diagonal block.
            # start=True ONLY on the tile's first matmul: start marks the
            # whole 2KB PSUM zero-region pending-zero, so a second start
            # would corrupt sibling q-blocks' accumulations. Later q-blocks'
            # first writes zero-on-first-write via that same pending flag.
            s0, Q, grp = CHUNKS[ci]
            qb_base = s0 // 128
            for hh in range(2):
                h = 2 * pair + hh
                off = hh * QC
                for j in range(grp):
                    kb = g * grp + j
                    kd = kb * 128 - s0
                    qb0 = max(0, kd // 128)
                    for qb in range(qb0, nqb):
                        nc.tensor.matmul(
                            ctx_pair[hh][:, qb * 128: qb * 128 + 65],
                            lhsT=ex[:, off + j * Q + qb * 128:
                                    off + j * Q + qb * 128 + 128],
                            rhs=v1_3d[:, kb * HPC + h, :],
                            start=(kb == 0 and qb == 0),
                            stop=(kb == qb_base + qb),
                            skip_group_check=True)

        def flip_norm_pieces(ci, pair, items):
            # flipped-ctx norm: per head a [128, nqb] reciprocal of the
            # per-q-partition denominators (col 64 of each qb slice), then
            # per (head, qb) a tensor_scalar multiply into the ctx_qm
            # staging tile, then one XBAR dma-transpose per q-block into
            # ctxT. Spread over the next waves.
            s0, Q, _ = CHUNKS[ci]
            nqb = Q // 128
            qb_lo = s0 // 128
            state = {}
            cqm = cqm_pool.tile([128, nqb * 128], BF16, tag="cqm",
                                name=f"cqm_{ci}_{pair}")

            def p_recips():
                state["recs"] = []
                for h, ctx_ps in items:
                    rec = scr.tile([128, 4], FP32, tag="rec",
                                   name=f"r_{ci}_{h}")
                    c3 = ctx_ps.rearrange("p (qb c) -> p qb c", c=128)
                    r3 = rec.rearrange("p (a b) -> p a b", b=1)
                    nc.vector.reciprocal(r3[:, 0:nqb, :],
                                         c3[:, 0:nqb, 64:65])
                    state["recs"].append(rec)

            def p_muls(i):
                h, ctx_ps = items[i]
                hh = h % 2
                rec = state["recs"][i]
                for qb in range(nqb):
                    nc.vector.tensor_scalar_mul(
                        cqm[:, qb * 128 + hh * 64: qb * 128 + hh * 64 + 64],
                        ctx_ps[:, qb * 128: qb * 128 + 64],
                        rec[:, qb:qb + 1])

            def p_transposes(qbs):
                # PE transpose (cheap: 128 rows each) into a bf16 PSUM tile
                # riding the proj slot rotation, then a DVE copy into ctxT.
                # Avoids the SP/HWDGE queue entirely (in-order SP.SEQ would
                # head-of-line-block later rearrange DMA issues).
                for qb in qbs:
                    tp = ps.tile([128, 128], BF16, tag="proj", bufs=2,
                                 name=f"tp_{ci}_{pair}_{qb}")
                    nc.tensor.transpose(
                        tp[:], cqm[:, qb * 128:(qb + 1) * 128], ident[:])
                    nc.vector.tensor_copy(
                        ctxT[pair][:, (qb_lo + qb) * 128:
                                   (qb_lo + qb + 1) * 128], tp[:])

            cut = min(2, nqb)
            return [p_recips, lambda: p_muls(0), lambda: p_muls(1),
                    lambda: p_transposes(range(0, cut)),
                    lambda: p_transposes(range(cut, nqb))]

        # ---- main schedule ----
        emit_proj_qk_interleaved(0, 0)
        pending_norms = []
        for ci in range(NCH):
            q0, Q, grp = CHUNKS[ci]
            nkb = (q0 + Q) // 128
            nqb = Q // 128
            ngrp = nkb // grp
            waves = [(pair, g) for pair in range(2)
                     for g in range(ngrp)]
            head = []   # pieces pinned to the earliest waves, one per wave
            extra = []  # pieces distributed evenly over all waves
            pins = {}   # wave -> pieces with exact placement constraints
            if ci == 0:
                # chunk 1's fp8 prep is pinned to the earliest waves so its
                # scores (the ACT feed during the xt input transfers) start
                # the moment x8[1] lands; chunk 2's prep spreads behind it
                # chunk 1's mt0 prep FIRST (ahead of chunk 0's mt1 startup
                # proj in the 2-slot proj PSUM rotation): it gates chunk 1's
                # scores, the main ACT feed once chunk 0's thin exps end
                qk1 = proj_qk_pieces(1)
                v0 = proj_v_pieces(range(0, 4))
                pins = {0: [qk1[0], qk1[1], qk1[2], v0[0],
                            lambda: emit_xt_load(1)],
                        1: [lambda: emit_proj_qk_interleaved(0, 1),
                            lambda: emit_rearrange(0, 0, k_only=True),
                            v0[1]],
                        2: [qk1[3], qk1[4], qk1[5], v0[2],
                            lambda: emit_rearrange(0, 1, k_only=True)],
                        3: [v0[3]]}
                extra += proj_qk_pieces(2) + [lambda: emit_xt_load(2)]
            elif ci == 1:
                head += proj_v_pieces(range(4, 8))
                extra += (proj_qk_pieces(3) + [lambda: emit_xt_load(3),
                                               emit_wot_load])
            elif ci == 2:
                head += proj_v_pieces(range(8, 12))
                extra += proj_qk_pieces(4) + outproj_pieces(0)
            elif ci == 3:
                head += proj_v_pieces(range(12, 16))
                extra += outproj_pieces(1) + outproj_pieces(2)
            else:
                extra += outproj_pieces(3)
            sched = {w: [] for w in range(len(waves))}
            for w, pcs in pins.items():
                sched[w].extend(pcs)
            for j, pc in enumerate(head):
                sched[j].append(pc)
            if extra:
                if ci == NCH - 1:
                    # outproj(NCH-2) pieces must emit no earlier than the
                    # wave where the previous pair's transpose pops write
                    # their ctxT q-block (pops land at waves 3-4)
                    w0 = 3
                else:
                    w0 = max(0, min(2 if ci == 0 else 4,
                                    len(waves) - len(extra)))
                span_w = len(waves) - w0
                for j, pc in enumerate(extra):
                    sched[w0 + j * span_w // len(extra)].append(pc)

            ctx_tiles = {}
            ctx_queue = []
            for w, (pair, g) in enumerate(waves):
                if g == 0:
                    # one PSUM bank per head: [128, nqb*128-float slices],
                    # 65 floats used per qb slice
                    ctx_tiles[pair] = [
                        ps.tile([128, QC], FP32, tag="ctx", bufs=2,
                                name=f"c_{ci}_{pair}_{hh}")
                        for hh in range(2)]
                ex = wave_scores(ci, pair, g)
                if pending_norms:
                    pending_norms.pop(0)()
                last_of_pair = g == ngrp - 1
                final_pair = last_of_pair and pair == 1 and ci + 1 == NCH
                if not final_pair:
                    for pc in sched[w]:
                        pc()
                ctx_queue.append((pair, g, ex))
                # defer ctx so the PE has scores to run while exp catches
                # up; drain continuously (small lag) so the pair-end flush
                # is small and the norm reciprocal doesn't head-of-line-
                # block the in-order DVE queue.
                lag = max(0, 3 - g) if grp > 1 else 6
                while len(ctx_queue) > lag or \
                        (ctx_queue and last_of_pair):
                    qpair, qg, qex = ctx_queue.pop(0)
                    wave_ctx_flip(ci, qpair, qg, qex, ctx_tiles[qpair], nqb)
                if last_of_pair:
                    h0 = 2 * pair
                    items = [(h0 + hh, ctx_tiles[pair][hh])
                             for hh in range(2)]
                    while pending_norms:  # drain leftovers before reassign
                        pending_norms.pop(0)()
                    if final_pair:
                        # kernel tail: emit the whole norm + transpose chain
                        # now, ahead of this wave's filler pieces
                        for pc in flip_norm_pieces(ci, pair, items):
                            pc()
                        for pc in sched[w]:
                            pc()
                    else:
                        pending_norms = flip_norm_pieces(ci, pair, items)
        for pc in outproj_pieces(NCH - 1, tail=True):
            pc()


def build_module():
    nc = bacc.Bacc("TRN2", target_bir_lowering=False, debug=False)
    xT = nc.dram_tensor("xT", [E, S], BF16, kind="ExternalInput").ap()
    wq = nc.dram_tensor("wq", [E, M], BF16, kind="ExternalInput").ap()
    wk = nc.dram_tensor("wk", [E, M], BF16, kind="ExternalInput").ap()
    wv = nc.dram_tensor("wv", [E, M], BF16, kind="ExternalInput").ap()
    wo = nc.dram_tensor("wo", [M, E], BF16, kind="ExternalInput").ap()
    x8 = nc.dram_tensor("x8", [128, 8 * S], F8, kind="ExternalInput").ap()
    wk8 = nc.dram_tensor("wk8", [128, 8 * M], F8, kind="ExternalInput").ap()
    wq8 = nc.dram_tensor("wq8", [128, 8 * M], F8, kind="ExternalInput").ap()
    out = nc.dram_tensor("out", [S, E], BF16, kind="ExternalOutput").ap()
    with tile.TileContext(nc) as tc:
        _emit_kernel(tc, xT, wq, wk, wv, wo, x8, wk8, wq8, out)
    nc.compile()
    return nc


def _pack_epairs(aT):
    """[E, N] -> [128, 4*2*N] fp8: e-tile pairs side by side per partition
    (DoubleRow packing: out[p, j, t, n] = aT[(2j+t)*128 + p, n])."""
    e4m3 = ml_dtypes.float8_e4m3
    E_, N = aT.shape
    a = np.asarray(aT, dtype=np.float32).reshape(4, 2, 128, N)
    a = np.ascontiguousarray(a.transpose(2, 0, 1, 3)).astype(e4m3)
    return a.reshape(128, 8 * N)


def make_in_maps(x, w_qkv):
    """Per-core input dicts (bf16/fp8, pre-transposed host-side)."""
    bf = ml_dtypes.bfloat16
    xTb = [np.ascontiguousarray(x[b].T).astype(bf) for b in range(B)]
    x8b = [_pack_epairs(x[b].T) for b in range(B)]
    in_maps = []
    for c in range(NCORES):
        b, g = c // 4, c % 4
        cols = slice(g * M, (g + 1) * M)
        wqT = np.ascontiguousarray(w_qkv[cols, :].T)
        wkT = np.ascontiguousarray(w_qkv[E:][cols, :].T)
        in_maps.append({
            "xT": xTb[b],
            "wq": wqT.astype(bf),
            "wk": wkT.astype(bf),
            "wv": np.ascontiguousarray(w_qkv[2 * E:][cols, :].T).astype(bf),
            "x8": x8b[b],
            "wk8": _pack_epairs(wkT * 64.0),
            "wq8": _pack_epairs(wqT * 64.0),
            "wo": None,  # filled in kernel(), needs w_out
        })
    return in_maps


_RUNNER = None
_SHARDED = None


def _get_runner():
    """Build the Bass module once and return a cached callable
    (in_maps) -> [NCORES, S, E] bf16 partial outputs."""
    global _RUNNER
    if _RUNNER is not None:
        return _RUNNER

    nc = build_module()

    from concourse import bass2jax
    import jax
    from jax.sharding import Mesh, PartitionSpec
    from jax.experimental.shard_map import shard_map

    bass2jax.install_neuronx_cc_hook()

    in_names = ["xT", "wq", "wk", "wv", "x8", "wk8", "wq8", "wo"]
    out_names = ["out"]
    out_avals = [jax.core.ShapedArray((S, E), ml_dtypes.bfloat16)]
    n_params = len(in_names)
    all_names = in_names + out_names
    partition_name = (nc.partition_id_tensor.name
                      if nc.partition_id_tensor is not None else None)
    if partition_name is not None:
        all_names = all_names + [partition_name]

    def _body(*args):
        operands = list(args)
        if partition_name is not None:
            operands.append(bass2jax.partition_id_tensor())
        outs = bass2jax._bass_exec_p.bind(
            *operands,
            out_avals=tuple(out_avals),
            in_names=tuple(all_names),
            out_names=tuple(out_names),
            lowering_input_output_aliases=(),
            sim_require_finite=True,
            sim_require_nnan=True,
            nc=nc,
        )
        return tuple(outs)

    devices = jax.devices()[:NCORES]
    mesh = Mesh(np.asarray(devices), ("core",))
    n_outs = len(out_names)
    in_specs = (PartitionSpec("core"),) * (n_params + n_outs)
    out_specs = (PartitionSpec("core"),) * n_outs
    sharded = jax.jit(
        shard_map(_body, mesh=mesh, in_specs=in_specs, out_specs=out_specs,
                  check_rep=False),
        donate_argnums=tuple(range(n_params, n_params + n_outs)),
        keep_unused=True,
    )
    global _SHARDED
    _SHARDED = sharded

    def run(in_maps):
        concat_in = [
            np.concatenate([np.asarray(in_maps[c][n]) for c in range(NCORES)],
                           axis=0)
            for n in in_names
        ]
        concat_zeros = [np.zeros((NCORES * S, E), ml_dtypes.bfloat16)]
        out_arrs = sharded(*concat_in, *concat_zeros)
        return np.asarray(out_arrs[0]).reshape(NCORES, S, E)

    _RUNNER = run
    return run


def kernel(x, w_qkv, w_out, b_out):
    x = np.asarray(x, dtype=np.float32)
    w_qkv = np.asarray(w_qkv, dtype=np.float32)
    w_out = np.asarray(w_out, dtype=np.float32)
    b_out = np.asarray(b_out, dtype=np.float32)

    bf = ml_dtypes.bfloat16
    in_maps = make_in_maps(x, w_qkv)
    for c in range(NCORES):
        g = c % 4
        cols = slice(g * M, (g + 1) * M)
        in_maps[c]["wo"] = np.ascontiguousarray(w_out[:, cols].T).astype(bf)

    run = _get_runner()
    partials = run(in_maps)  # [8, S, E] bf16

    out = np.empty((B, S, E), np.float32)
    for b in range(B):
        acc = partials[4 * b].astype(np.float64)
        for i in range(1, 4):
            acc += partials[4 * b + i].astype(np.float64)
        out[b] = (acc + b_out.astype(np.float64)).astype(np.float32)
    return out


# revision 61
# speedup vs baseline: 1.0562x; 1.0013x over previous
"""Multi-head causal self-attention (B=2, S=2048, E=1024, H=16, D=64) on 8
Trainium2 NeuronCores.

Sharding: batch x head-group. Core c handles batch (c // 4) and heads
[4*(c%4), 4*(c%4)+4). Each core computes QKV projection for its 4 heads,
causal flash-attention, and a partial output projection over its head
columns. Host sums the 4 partial outputs per batch and adds b_out.

v4 changes vs v3:
  - flipped ctx matmuls for chunks 0-3: out = [q-part 128, 65] with
    lhsT = ex q-window (stationary), rhs = v1 slab. PE cost per
    (head, kb, qb) drops from `cols` to 65 rows (full 128-partition
    output). The softmax denominator lands per-q-PARTITION, so the norm
    is a cheap [128,nqb] reciprocal + per-partition-scalar multiplies
    (no Pool partition_broadcast).
  - ctx_qm [q, m] bf16 is transposed back to ctxT [m, q] with
    dma_start_transpose (XBAR), one [128,128] tile per (pair, q-block).
  - chunk 4 (last 128 cols) keeps the v3 unflipped path so the kernel
    tail avoids the DMA-transpose latency.
  - q projection for chunks 1-4 via fp8 DoubleRow (host-packed wq8,
    x8), mirroring the k8 path: 1/4 the PE cost of the bf16 proj.
  - stair masking of ex moved from DVE tensor_mul to Pool affine_select
    (SBUF->SBUF, one call covers both heads of a wave).
"""

import sys

if "/opt/trn_rl_repo" not in sys.path:
    sys.path.insert(0, "/opt/trn_rl_repo")

import numpy as np
import ml_dtypes

import concourse.bacc as bacc
import concourse.mybir as mybir
import concourse.tile as tile

BF16 = mybir.dt.bfloat16
FP32 = mybir.dt.float32
F8 = mybir.dt.float8e4
DRMODE = mybir.MatmulPerfMode.DoubleRow

B, S, E = 2, 2048, 1024
H, DH = 16, 64
NCORES = 8
HPC = 4            # heads per core
M = HPC * DH       # 256 ctx columns per core
QC = 512           # q chunk (max wave width; also PSUM head stride)
KB = 128           # k block
SCALE = 1.0 / np.sqrt(DH)
NWARM = 64         # warmup dummy matmuls (128 cols each)
# q-chunks (q0, Q, grp). The last 512 splits 384+128 so the final
# norm/outproj tail is 4x smaller. grp = k-blocks per wave for the narrow
# final chunk (shares one exp call across 4 k-blocks).
CHUNKS = [(0, 512, 1), (512, 512, 1), (1024, 512, 1),
          (1536, 384, 1), (1920, 128, 4)]
NCH = len(CHUNKS)


def _emit_kernel(tc, xT, wq, wk, wv, wo_d, x8, wk8, wq8, out):
    nc = tc.nc
    Exp = mybir.ActivationFunctionType.Exp

    with tc.tile_pool(name="res", bufs=1) as res, \
         tc.tile_pool(name="ps", bufs=1, space="PSUM") as ps, \
         tc.tile_pool(name="expp", bufs=16) as expp, \
         tc.tile_pool(name="scr", bufs=4) as scr, \
         tc.tile_pool(name="cqm", bufs=2) as cqm_pool, \
         tc.tile_pool(name="outb", bufs=2) as outb:

        # ---- resident SBUF tiles ----
        xt_all = res.tile([128, 8 * S], BF16, name="xt_all")
        xt = [xt_all[:, e * S:(e + 1) * S] for e in range(8)]
        xt_3d = xt_all.rearrange("p (e s) -> p e s", s=S)
        wqt = res.tile([128, 8 * M], BF16, name="wqt")
        wkt = res.tile([128, 8 * M], BF16, name="wkt")
        wvt = res.tile([128, 8 * M], BF16, name="wvt")
        wot = [res.tile([128, E], BF16, name=f"wot{i}") for i in range(2)]
        qTt = [res.tile([128, S], BF16, name=f"qTt{i}") for i in range(2)]
        kTt = [res.tile([128, S], BF16, name=f"kTt{i}") for i in range(2)]
        ctxT = [res.tile([128, S], BF16, name=f"ctxT{i}") for i in range(2)]
        # fp8 scores path (queries >= 512): per chunk one classic-layout fp8
        # staging tile (free dims qk x mt x s) and one DoubleRow "pair" tile
        # [64, j x qk x mt x s] with head parity on partition halves {0,32}
        # and the two dh-32 k-tiles (j) in the free dim. PER-CHUNK tiles:
        # the dependency tracker flattens strided APs to byte ranges, so a
        # single shared tile makes chunk ci's scores falsely wait on chunk
        # ci+1's rearrange DMAs.
        qk8cs = [res.tile([128, 4 * CH[1]], F8, name=f"qk8c{i}")
                 for i, CH in enumerate(CHUNKS)]
        qk8c4s = [t.rearrange("p (t m s) -> p t m s", t=2, m=2)
                  for t in qk8cs]
        qk8ps = [res.tile([64, 8 * CH[1]], F8, name=f"qk8p{i}")
                 for i, CH in enumerate(CHUNKS)]
        qk8p5s = [t.rearrange("p (j t m s) -> p j t m s", j=2, t=2, m=2)
                  for t in qk8ps]

        def kb_loc(kb):
            # global k-block -> (chunk index, local column offset)
            for i in range(NCH - 1, -1, -1):
                if kb * 128 >= CHUNKS[i][0]:
                    return i, kb * 128 - CHUNKS[i][0]
            raise AssertionError
        # fp8 projection operands (host-packed e-pair layout): q/k columns
        # that are only ever consumed by the fp8 score path are projected
        # with fp8 DoubleRow matmuls at 1/4 the PE cost
        x8t = res.tile([128, 8 * S], F8, name="x8t")
        x8_4d = x8t.rearrange("p (j t s) -> p j t s", j=4, t=2)
        wk8t = res.tile([128, 8 * M], F8, name="wk8t")
        wk8_4d = wk8t.rearrange("p (j t m) -> p j t m", j=4, t=2)
        wq8t = res.tile([128, 8 * M], F8, name="wq8t")
        wq8_4d = wq8t.rearrange("p (j t m) -> p j t m", j=4, t=2)
        # V with ones column: per (k-block kb, head h) a [128, 65] slab
        v1 = res.tile([128, (S // KB) * HPC * 65], BF16, name="v1")
        v1_3d = v1.rearrange("p (n c) -> p n c", c=65)
        warm_src = res.tile([128, 128], BF16, name="warm_src")
        ident = res.tile([128, 128], BF16, name="ident")

        # ---- warmup: dummy matmuls keep the PE busy (and its p-state
        # ramping) through the DMA-gated startup.
        nc.gpsimd.memset(warm_src[:], 0.0)
        warm_ps = ps.tile([128, QC], FP32, tag="proj", bufs=2, name="warm_ps")
        for i in range(NWARM):
            nc.tensor.matmul(
                warm_ps[:, 0:128], lhsT=warm_src[:], rhs=warm_src[:],
                start=True, stop=True)

        # ---- input DMA: one batched transfer per tensor/chunk ----
        wqt_3d = wqt.rearrange("p (e m) -> p e m", m=M)
        wkt_3d = wkt.rearrange("p (e m) -> p e m", m=M)
        wvt_3d = wvt.rearrange("p (e m) -> p e m", m=M)
        xT_3d = xT.rearrange("(e p) s -> p e s", p=128)
        # order: chunk-0 bf16 operands first (pqi/pki), then the fp8
        # operands for ALL chunks (small; chunks 1+ exp work can only start
        # once q8/k8 are projected + rearranged, and that work is what keeps
        # the ACT engine fed during the remaining xt transfers), then the
        # bf16 x chunks (V projections, consumed later) and wo.
        nc.sync.dma_start(wqt_3d[:], wq.rearrange("(e p) m -> p e m", p=128))
        nc.sync.dma_start(xt_3d[:, :, 0:256], xT_3d[:, :, 0:256])
        nc.sync.dma_start(wkt_3d[:], wk.rearrange("(e p) m -> p e m", p=128))
        nc.sync.dma_start(xt_3d[:, :, 256:QC], xT_3d[:, :, 256:QC])
        x8_dram = x8.rearrange("p (j t s) -> p j t s", j=4, t=2)
        nc.sync.dma_start(wq8t[:], wq8)
        nc.sync.dma_start(wk8t[:], wk8)

        def emit_x8_load(chunk):
            nc.sync.dma_start(
                x8_4d[:, :, :, chunk * QC:(chunk + 1) * QC],
                x8_dram[:, :, :, chunk * QC:(chunk + 1) * QC])

        emit_x8_load(1)
        nc.sync.dma_start(wvt_3d[:], wv.rearrange("(e p) m -> p e m", p=128))
        emit_x8_load(2)
        emit_x8_load(3)

        # xt[1..3] and wot are consumed late (V projections of later chunks,
        # outproj). Their dma_starts are deferred into the wave schedule so
        # the per-chunk qk8p rearranges (which gate each chunk's scores and
        # hence the ACT-bound steady state) aren't queued behind them on the
        # serial DMA device.
        def emit_xt_load(chunk):
            nc.sync.dma_start(
                xt_3d[:, :, chunk * QC:(chunk + 1) * QC],
                xT_3d[:, :, chunk * QC:(chunk + 1) * QC])

        def emit_wot_load():
            for i in range(2):
                nc.sync.dma_start(wot[i][:], wo_d[i * 128:(i + 1) * 128, :])

        # ---- constants ----
        # exp-table warm first: the ~2.7us table load + warm call must not
        # queue behind the big v1 memset on the Pool engine
        warm = scr.tile([1, 1], FP32, tag="warm", bufs=1, name="warm")
        nc.gpsimd.memset(warm[:], 0.0)
        nc.scalar.activation(warm[:], warm[:],
                             mybir.ActivationFunctionType.Exp)
        nc.gpsimd.memset(v1[:], 1.0)  # data columns overwritten by V proj
        # identity matrix for PE transposes (keep where q_local == partition)
        nc.gpsimd.memset(ident[:], 1.0)
        nc.gpsimd.affine_select(
            out=ident[:], in_=ident[:],
            compare_op=mybir.AluOpType.is_equal,
            fill=0.0, base=0,
            pattern=[[1, 128]],
            channel_multiplier=-1,
        )

        # ---- emission helpers ----
        def stage_f8(ci, mt, kind, pqk, scale=None):
            # fp8 classic staging into the (qk, mt) slab of chunk ci's qk8c
            s0, Q, _ = CHUNKS[ci]
            t = 0 if kind == "q" else 1
            if scale is None:
                nc.vector.tensor_copy(qk8c4s[ci][:, t, mt, 0:Q], pqk[:, 0:Q])
            else:
                nc.vector.tensor_scalar_mul(qk8c4s[ci][:, t, mt, 0:Q],
                                            pqk[:, 0:Q], scale)

        def emit_rearrange(ci, mt, k_only=False):
            # partition rearrange into the DoubleRow pair tile for chunk
            # ci's mt slab (per-mt so pair 0's scores aren't gated on the
            # mt=1 projections)
            s0, Q, _ = CHUNKS[ci]
            t0 = 1 if k_only else 0
            for hh in range(2):
                for j in range(2):
                    nc.sync.dma_start(
                        qk8p5s[ci][32 * hh:32 * hh + 32, j, t0:2,
                                   mt:mt + 1, 0:Q],
                        qk8c4s[ci][64 * hh + 32 * j: 64 * hh + 32 * j + 32,
                                   t0:2, mt:mt + 1, 0:Q])

        def stage_qk(ci, mt, kind, pqk):
            # chunk 0 queries score in bf16 (classic layout); all other
            # queries score in fp8 DoubleRow. k is needed in fp8 by every
            # fp8 chunk, and in bf16 only for chunk 0's k-blocks.
            s0, Q, _ = CHUNKS[ci]
            dstt = qTt if kind == "q" else kTt
            if ci == 0:
                nc.vector.tensor_copy(dstt[mt][:, s0:s0 + Q], pqk[:, 0:Q])
            if kind == "k" or ci >= 1:
                stage_f8(ci, mt, kind, pqk)

        def emit_proj_qk8(ci, mt, kind):
            # q/k projection for fp8-only consumers via fp8 DoubleRow over
            # host-packed e-pairs: 1/4 the PE cost of the bf16 projection
            s0, Q, _ = CHUNKS[ci]
            w8 = wq8_4d if kind == "q" else wk8_4d
            pk = ps.tile([128, QC], FP32, tag="proj", bufs=2,
                         name=f"p8{kind}_{ci}_{mt}")
            # a-piece OUTER: interleaving two DoubleRow accumulation groups
            # (j inner per region) miscomputes on hardware -- each region's
            # 4-instruction group must run contiguously
            for a in range(0, Q, 256):
                b = min(a + 256, Q)
                for j in range(4):
                    nc.tensor.matmul(
                        pk[:, a:b],
                        lhsT=w8[:, j, :, mt * 128:(mt + 1) * 128],
                        rhs=x8_4d[:, j, :, s0 + a: s0 + b],
                        start=(j == 0), stop=(j == 3),
                        perf_mode=DRMODE)
            # w8 is host-scaled by 64 (w values ~0.02 sit in e4m3's
            # subnormal range, which the PE flushes to zero); undo here
            stage_f8(ci, mt, kind, pk, scale=1.0 / 64.0)

        def emit_proj_qk_interleaved(ci, mt):
            # startup projection: q first (wave 0 needs all 512 q columns),
            # then k in two pieces so wave 0 only gates on its first k-block
            s0, Q, _ = CHUNKS[ci]
            pq = ps.tile([128, QC], FP32, tag="proj", bufs=2,
                         name=f"pqi_{ci}_{mt}")
            pk = ps.tile([128, QC], FP32, tag="proj", bufs=2,
                         name=f"pki_{ci}_{mt}")
            pieces = [(pq, wqt, 0, 256), (pk, wkt, 0, 128),
                      (pq, wqt, 256, Q), (pk, wkt, 128, Q)]
            for dst, wt, a, b in pieces:
                for e in range(8):
                    nc.tensor.matmul(
                        dst[:, a:b],
                        lhsT=wt[:, e * M + mt * 128: e * M + (mt + 1) * 128],
                        rhs=xt[e][:, s0 + a: s0 + b],
                        start=(e == 0), stop=(e == 7))
                tgt = qTt if wt is wqt else kTt
                nc.vector.tensor_copy(tgt[mt][:, s0 + a: s0 + b],
                                      dst[:, a:b])
            stage_f8(ci, mt, "k", pk)

        def emit_proj_v(sblk):
            pv = ps.tile([128, M], FP32, tag="proj", bufs=2, name=f"pv_{sblk}")
            for e in range(8):
                nc.tensor.matmul(
                    pv[:],
                    lhsT=xt[e][:, sblk * 128:(sblk + 1) * 128],
                    rhs=wvt[:, e * M:(e + 1) * M],
                    start=(e == 0), stop=(e == 7))
            nc.vector.tensor_copy(
                v1_3d[:, sblk * HPC:(sblk + 1) * HPC, 0:64],
                pv[:].rearrange("p (h c) -> p h c", c=64))

        def proj_qk_pieces(ci):
            pcs = []
            for mt in range(2):
                pcs.append(lambda mt=mt: emit_proj_qk8(ci, mt, "q"))
                pcs.append(lambda mt=mt: emit_proj_qk8(ci, mt, "k"))
                pcs.append(lambda mt=mt: emit_rearrange(ci, mt))
            return pcs

        def proj_v_pieces(blks):
            return [lambda sb=sb: emit_proj_v(sb) for sb in blks]

        ob_tiles = {}
        out_3d = out.rearrange("(q p) f -> p q f", p=128)

        def emit_outproj(ci, qq, fc, tail=False):
            q0, Q, _ = CHUNKS[ci]
            nqb = Q // 128
            qb = q0 // 128 + qq
            last = ci == NCH - 1
            # last two chunks store per-q-block so the kernel-tail store
            # isn't queued behind one big merged transfer
            perqb = ci >= NCH - 2
            if qq == 0 and fc == 0:
                ob_tiles[ci] = outb.tile([128, nqb * E], BF16, tag="ob",
                                         name=f"ob_{qb}")
            ob = ob_tiles[ci]
            # tail outprojs borrow the scores PSUM slots (attention is done
            # by then), keeping mm->copy->mm free of slot serialization
            tag = "scores" if tail else "proj"
            po = ps.tile([128, QC], FP32, tag=tag, bufs=2,
                         name=f"po_{qb}_{fc}")
            for mc in range(2):
                nc.tensor.matmul(
                    po[:],
                    lhsT=ctxT[mc][:, qb * 128:(qb + 1) * 128],
                    rhs=wot[mc][:, fc * QC:(fc + 1) * QC],
                    start=(mc == 0), stop=(mc == 1))
            if last and fc == 1:
                # final piece: stage on the (idle by now) ACT engine so the
                # two last copies run in parallel instead of serializing on
                # the DVE queue
                nc.scalar.activation(
                    ob[:, qq * E + fc * QC: qq * E + (fc + 1) * QC], po[:],
                    mybir.ActivationFunctionType.Copy)
            else:
                nc.vector.tensor_copy(
                    ob[:, qq * E + fc * QC: qq * E + (fc + 1) * QC], po[:])
            if perqb:
                if fc == 1:
                    nc.sync.dma_start(
                        out[qb * 128:(qb + 1) * 128, :],
                        ob[:, qq * E: (qq + 1) * E])
                if (qq, fc) == (nqb - 1, 1):
                    del ob_tiles[ci]
            elif (qq, fc) == (nqb - 1, 1):
                nc.sync.dma_start(
                    out_3d[:, q0 // 128: q0 // 128 + nqb, :],
                    ob.rearrange("p (q f) -> p q f", f=E))
                del ob_tiles[ci]

        def outproj_pieces(ci, tail=False):
            _, Q, _ = CHUNKS[ci]
            return [lambda qq=qq, fc=fc: emit_outproj(ci, qq, fc, tail=tail)
                    for qq in range(Q // 128) for fc in range(2)]

        # ---- attention waves (one head PAIR, grp k-blocks) ----
        def wave_scores(ci, pair, g):
            s0, Q, grp = CHUNKS[ci]
            mt = pair
            fp8 = ci >= 1
            sc_ps = ps.tile([128, 2 * QC], FP32, tag="scores", bufs=2,
                            name=f"s_{ci}_{pair}_{g}")
            kds = [(j, (g * grp + j) * 128 - s0) for j in range(grp)]
            lo_e = 0
            for hh in range(2):
                r0 = hh * 64
                off = hh * QC
                for j, kd in kds:
                    kb = g * grp + j
                    # cols [0, kd) of this k-block's region are fully
                    # masked -> skip in scores. Only for grp == 1 (where
                    # the exp also skips them); grp > 1 diagonal waves
                    # compute the ~128 masked cols (27ns) so the exp never
                    # reads unwritten PSUM.
                    lo = kd if (kd >= 128 and grp == 1) else 0
                    if hh == 0 and kd >= 128 and grp == 1:
                        lo_e = kd
                    if fp8:
                        # DoubleRow: dh 2x32 k-tiles, head at base 32*hh;
                        # moving free = 2*w caps piece width at 256
                        ck, koff = kb_loc(kb)
                        a = lo
                        while a < Q:
                            b = min(a + 256, Q)
                            nc.tensor.matmul(
                                sc_ps[:, off + j * Q + a: off + j * Q + b],
                                lhsT=qk8p5s[ck][32 * hh:32 * hh + 32, :, 1,
                                                mt, koff:koff + 128],
                                rhs=qk8p5s[ci][32 * hh:32 * hh + 32, :, 0,
                                               mt, a:b],
                                start=True, stop=True,
                                perf_mode=DRMODE)
                            a = b
                    elif (pair, g) != (0, 0):
                        nc.tensor.matmul(
                            sc_ps[:, off + j * Q + lo: off + (j + 1) * Q],
                            lhsT=kTt[mt][r0:r0 + 64, kb * 128:(kb + 1) * 128],
                            rhs=qTt[mt][r0:r0 + 64, s0 + lo: s0 + Q],
                            start=True, stop=True)
            if not fp8 and (pair, g) == (0, 0):
                # very first wave: scores in column pieces matching the
                # split startup projection, COLUMN-outer / head-inner (the
                # PE wait queue is FIFO, so a blocked later-column piece
                # must not sit in front of a ready first-column piece)
                for a, b in [(0, 256), (256, Q)]:
                    for hh in range(2):
                        r0, off = hh * 64, hh * QC
                        nc.tensor.matmul(
                            sc_ps[:, off + a: off + b],
                            lhsT=kTt[mt][r0:r0 + 64, 0:128],
                            rhs=qTt[mt][r0:r0 + 64, a:b],
                            start=True, stop=True)
            ex = expp.tile([128, 2 * QC], BF16, tag="ex",
                           name=f"e_{ci}_{pair}_{g}")
            W = grp * Q
            ex3 = ex.rearrange("p (h q) -> p h q", h=2)
            sc3 = sc_ps.rearrange("p (h q) -> p h q", h=2)
            if (ci, pair, g) == (0, 0, 0):
                # very first wave: exp per (head, column-half) in contiguous
                # slices (a strided 2-head AP flattens to a byte range that
                # would falsely depend on the later column pieces)
                for a, b in [(0, 256), (256, W)]:
                    for hh in range(2):
                        nc.scalar.activation(ex3[:, hh, a:b],
                                             sc3[:, hh, a:b],
                                             Exp, scale=SCALE)
            elif lo_e or W < QC:
                # both heads in one strided-AP call: the ACT engine charges
                # by total free size, so this halves the per-call init cost
                # vs one call per head
                nc.scalar.activation(ex3[:, :, lo_e:W], sc3[:, :, lo_e:W],
                                     Exp, scale=SCALE)
            else:
                nc.scalar.activation(ex[:], sc_ps[:], Exp, scale=SCALE)
            # stair mask on the diagonal 128-block: zero ex where
            # k_local > q_local. Pool affine_select (SBUF->SBUF), one call
            # covers both heads: keep where (q_local - k_partition) >= 0.
            for j, kd in kds:
                if kd >= 0:
                    nc.gpsimd.affine_select(
                        out=ex3[:, :, j * Q + kd: j * Q + kd + 128],
                        in_=ex3[:, :, j * Q + kd: j * Q + kd + 128],
                        compare_op=mybir.AluOpType.is_ge,
                        fill=0.0, base=0,
                        pattern=[[0, 2], [1, 128]],
                        channel_multiplier=-1,
                    )
            return ex

        def wave_ctx_flip(ci, pair, g, ex, ctx_pair, nqb):
            # flipped ctx: lhsT = ex q-window (stationary), rhs = v1 slab,
            # out = [q-part 128, 65] accumulated over kb. qb's last
            # contribution is its diagonal block.
            # start=True ONLY on the tile's first matmul: start marks the
            # whole 2KB PSUM zero-region pending-zero, so a second start
            # would corrupt sibling q-blocks' accumulations. Later q-blocks'
            # first writes zero-on-first-write via that same pending flag.
            s0, Q, grp = CHUNKS[ci]
            qb_base = s0 // 128
            for hh in range(2):
                h = 2 * pair + hh
                off = hh * QC
                for j in range(grp):
                    kb = g * grp + j
                    kd = kb * 128 - s0
                    qb0 = max(0, kd // 128)
                    for qb in range(qb0, nqb):
                        nc.tensor.matmul(
                            ctx_pair[hh][:, qb * 128: qb * 128 + 65],
                            lhsT=ex[:, off + j * Q + qb * 128:
                                    off + j * Q + qb * 128 + 128],
                            rhs=v1_3d[:, kb * HPC + h, :],
                            start=(kb == 0 and qb == 0),
                            stop=(kb == qb_base + qb),
                            skip_group_check=True)

        def flip_norm_pieces(ci, pair, items):
            # flipped-ctx norm: per head a [128, nqb] reciprocal of the
            # per-q-partition denominators (col 64 of each qb slice), then
            # per (head, qb) a tensor_scalar multiply into the ctx_qm
            # staging tile, then one XBAR dma-transpose per q-block into
            # ctxT. Spread over the next waves.
            s0, Q, _ = CHUNKS[ci]
            nqb = Q // 128
            qb_lo = s0 // 128
            state = {}
            cqm = cqm_pool.tile([128, nqb * 128], BF16, tag="cqm",
                                name=f"cqm_{ci}_{pair}")

            def p_recips():
                state["recs"] = []
                for h, ctx_ps in items:
                    rec = scr.tile([128, 4], FP32, tag="rec",
                                   name=f"r_{ci}_{h}")
                    c3 = ctx_ps.rearrange("p (qb c) -> p qb c", c=128)
                    r3 = rec.rearrange("p (a b) -> p a b", b=1)
                    nc.vector.reciprocal(r3[:, 0:nqb, :],
                                         c3[:, 0:nqb, 64:65])
                    state["recs"].append(rec)

            def p_muls(i):
                h, ctx_ps = items[i]
                hh = h % 2
                rec = state["recs"][i]
                for qb in range(nqb):
                    nc.vector.tensor_scalar_mul(
                        cqm[:, qb * 128 + hh * 64: qb * 128 + hh * 64 + 64],
                        ctx_ps[:, qb * 128: qb * 128 + 64],
                        rec[:, qb:qb + 1])

            def p_transposes(qbs):
                # PE transpose (cheap: 128 rows each) into a bf16 PSUM tile
                # riding the proj slot rotation, then a DVE copy into ctxT.
                # Avoids the SP/HWDGE queue entirely (in-order SP.SEQ would
                # head-of-line-block later rearrange DMA issues).
                for qb in qbs:
                    tp = ps.tile([128, 128], BF16, tag="proj", bufs=2,
                                 name=f"tp_{ci}_{pair}_{qb}")
                    nc.tensor.transpose(
                        tp[:], cqm[:, qb * 128:(qb + 1) * 128], ident[:])
                    nc.vector.tensor_copy(
                        ctxT[pair][:, (qb_lo + qb) * 128:
                                   (qb_lo + qb + 1) * 128], tp[:])

            cut = min(2, nqb)
            return [p_recips, lambda: p_muls(0), lambda: p_muls(1),
                    lambda: p_transposes(range(0, cut)),
                    lambda: p_transposes(range(cut, nqb))]

        # ---- main schedule ----
        emit_proj_qk_interleaved(0, 0)
        pending_norms = []
        for ci in range(NCH):
            q0, Q, grp = CHUNKS[ci]
            nkb = (q0 + Q) // 128
            nqb = Q // 128
            ngrp = nkb // grp
            waves = [(pair, g) for pair in range(2)
                     for g in range(ngrp)]
            head = []   # pieces pinned to the earliest waves, one per wave
            extra = []  # pieces distributed evenly over all waves
            pins = {}   # wave -> pieces with exact placement constraints
            if ci == 0:
                # chunk 1's fp8 prep is pinned to the earliest waves so its
                # scores (the ACT feed during the xt input transfers) start
                # the moment x8[1] lands; chunk 2's prep spreads behind it
                # chunk 1's mt0 prep FIRST (ahead of chunk 0's mt1 startup
                # proj in the 2-slot proj PSUM rotation): it gates chunk 1's
                # scores, the main ACT feed once chunk 0's thin exps end
                qk1 = proj_qk_pieces(1)
                v0 = proj_v_pieces(range(0, 4))
                pins = {0: [qk1[0], qk1[1], qk1[2], v0[0],
                            lambda: emit_xt_load(1)],
                        1: [lambda: emit_proj_qk_interleaved(0, 1),
                            lambda: emit_rearrange(0, 0, k_only=True),
                            v0[1]],
                        2: [qk1[3], qk1[4], qk1[5], v0[2],
                            lambda: emit_rearrange(0, 1, k_only=True)],
                        3: [v0[3]]}
                extra += proj_qk_pieces(2) + [lambda: emit_xt_load(2)]
            elif ci == 1:
                head += proj_v_pieces(range(4, 8))
                extra += (proj_qk_pieces(3) + [lambda: emit_xt_load(3),
                                               emit_wot_load])
            elif ci == 2:
                head += proj_v_pieces(range(8, 12))
                extra += proj_qk_pieces(4) + outproj_pieces(0)
            elif ci == 3:
                head += proj_v_pieces(range(12, 16))
                extra += outproj_pieces(1) + outproj_pieces(2)
            else:
                extra += outproj_pieces(3)
            sched = {w: [] for w in range(len(waves))}
            for w, pcs in pins.items():
                sched[w].extend(pcs)
            for j, pc in enumerate(head):
                sched[j].append(pc)
            if extra:
                if ci == NCH - 1:
                    # outproj(NCH-2) pieces: no earlier than wave 4 (the
                    # previous pair's transpose pops land at waves 3-4) and
                    # packed 2/wave so the last store clears the tail
                    w0 = 4
                    span_w = len(waves) - w0 - 1
                else:
                    w0 = max(0, min(2 if ci == 0 else 4,
                                    len(waves) - len(extra)))
                    span_w = len(waves) - w0
                for j, pc in enumerate(extra):
                    sched[w0 + j * span_w // len(extra)].append(pc)

            ctx_tiles = {}
            ctx_queue = []
            for w, (pair, g) in enumerate(waves):
                if g == 0:
                    # one PSUM bank per head: [128, nqb*128-float slices],
                    # 65 floats used per qb slice
                    ctx_tiles[pair] = [
                        ps.tile([128, QC], FP32, tag="ctx", bufs=2,
                                name=f"c_{ci}_{pair}_{hh}")
                        for hh in range(2)]
                ex = wave_scores(ci, pair, g)
                if pending_norms:
                    pending_norms.pop(0)()
                last_of_pair = g == ngrp - 1
                final_pair = last_of_pair and pair == 1 and ci + 1 == NCH
                if not final_pair:
                    for pc in sched[w]:
                        pc()
                ctx_queue.append((pair, g, ex))
                # defer ctx so the PE has scores to run while exp catches
                # up; drain continuously (small lag) so the pair-end flush
                # is small and the norm reciprocal doesn't head-of-line-
                # block the in-order DVE queue.
                lag = max(0, 3 - g) if grp > 1 else 6
                while len(ctx_queue) > lag or \
                        (ctx_queue and last_of_pair):
                    qpair, qg, qex = ctx_queue.pop(0)
                    wave_ctx_flip(ci, qpair, qg, qex, ctx_tiles[qpair], nqb)
                if last_of_pair:
                    h0 = 2 * pair
                    items = [(h0 + hh, ctx_tiles[pair][hh])
                             for hh in range(2)]
                    while pending_norms:  # drain leftovers before reassign
                        pending_norms.pop(0)()
                    if final_pair:
                        # kernel tail: the final q-block's outproj mc0
                        # halves read ctxT[0] (ready since pair 0's norm),
                        # so emit them first — they run under the norm
                        # chain; only the mc1 halves wait on the final
                        # transpose. Then the norm chain ahead of this
                        # wave's filler pieces.
                        qbf = S // 128 - 1
                        po_t = []
                        for fc in range(2):
                            po = ps.tile([128, QC], FP32, tag="scores",
                                         bufs=2, name=f"pot_{fc}")
                            nc.tensor.matmul(
                                po[:],
                                lhsT=ctxT[0][:, qbf * 128:(qbf + 1) * 128],
                                rhs=wot[0][:, fc * QC:(fc + 1) * QC],
                                start=True, stop=False,
                                skip_group_check=True)
                            po_t.append(po)
                        for pc in flip_norm_pieces(ci, pair, items):
                            pc()
                        for pc in sched[w]:
                            pc()
                    else:
                        pending_norms = flip_norm_pieces(ci, pair, items)
        # ---- kernel tail: final q-block mc1 + staging + store ----
        qbf = S // 128 - 1
        ob_f = outb.tile([128, E], BF16, tag="ob", name="ob_f")
        for fc in range(2):
            nc.tensor.matmul(
                po_t[fc],
                lhsT=ctxT[1][:, qbf * 128:(qbf + 1) * 128],
                rhs=wot[1][:, fc * QC:(fc + 1) * QC],
                start=False, stop=True,
                skip_group_check=True)
        # stage the two halves on different engines so they run in parallel
        nc.vector.tensor_copy(ob_f[:, 0:QC], po_t[0][:])
        nc.scalar.activation(ob_f[:, QC:E], po_t[1][:],
                             mybir.ActivationFunctionType.Copy)
        nc.sync.dma_start(out[qbf * 128:(qbf + 1) * 128, :], ob_f[:])


def build_module():
    nc = bacc.Bacc("TRN2", target_bir_lowering=False, debug=False)
    xT = nc.dram_tensor("xT", [E, S], BF16, kind="ExternalInput").ap()
    wq = nc.dram_tensor("wq", [E, M], BF16, kind="ExternalInput").ap()
    wk = nc.dram_tensor("wk", [E, M], BF16, kind="ExternalInput").ap()
    wv = nc.dram_tensor("wv", [E, M], BF16, kind="ExternalInput").ap()
    wo = nc.dram_tensor("wo", [M, E], BF16, kind="ExternalInput").ap()
    x8 = nc.dram_tensor("x8", [128, 8 * S], F8, kind="ExternalInput").ap()
    wk8 = nc.dram_tensor("wk8", [128, 8 * M], F8, kind="ExternalInput").ap()
    wq8 = nc.dram_tensor("wq8", [128, 8 * M], F8, kind="ExternalInput").ap()
    out = nc.dram_tensor("out", [S, E], BF16, kind="ExternalOutput").ap()
    with tile.TileContext(nc) as tc:
        _emit_kernel(tc, xT, wq, wk, wv, wo, x8, wk8, wq8, out)
    nc.compile()
    return nc


def _pack_epairs(aT):
    """[E, N] -> [128, 4*2*N] fp8: e-tile pairs side by side per partition
    (DoubleRow packing: out[p, j, t, n] = aT[(2j+t)*128 + p, n])."""
    e4m3 = ml_dtypes.float8_e4m3
    E_, N = aT.shape
    a = np.asarray(aT, dtype=np.float32).reshape(4, 2, 128, N)
    a = np.ascontiguousarray(a.transpose(2, 0, 1, 3)).astype(e4m3)
    return a.reshape(128, 8 * N)


def make_in_maps(x, w_qkv):
    """Per-core input dicts (bf16/fp8, pre-transposed host-side)."""
    bf = ml_dtypes.bfloat16
    xTb = [np.ascontiguousarray(x[b].T).astype(bf) for b in range(B)]
    x8b = [_pack_epairs(x[b].T) for b in range(B)]
    in_maps = []
    for c in range(NCORES):
        b, g = c // 4, c % 4
        cols = slice(g * M, (g + 1) * M)
        wqT = np.ascontiguousarray(w_qkv[cols, :].T)
        wkT = np.ascontiguousarray(w_qkv[E:][cols, :].T)
        in_maps.append({
            "xT": xTb[b],
            "wq": wqT.astype(bf),
            "wk": wkT.astype(bf),
            "wv": np.ascontiguousarray(w_qkv[2 * E:][cols, :].T).astype(bf),
            "x8": x8b[b],
            "wk8": _pack_epairs(wkT * 64.0),
            "wq8": _pack_epairs(wqT * 64.0),
            "wo": None,  # filled in kernel(), needs w_out
        })
    return in_maps


_RUNNER = None
_SHARDED = None


def _get_runner():
    """Build the Bass module once and return a cached callable
    (in_maps) -> [NCORES, S, E] bf16 partial outputs."""
    global _RUNNER
    if _RUNNER is not None:
        return _RUNNER

    nc = build_module()

    from concourse import bass2jax
    import jax
    from jax.sharding import Mesh, PartitionSpec
    from jax.experimental.shard_map import shard_map

    bass2jax.install_neuronx_cc_hook()

    in_names = ["xT", "wq", "wk", "wv", "x8", "wk8", "wq8", "wo"]
    out_names = ["out"]
    out_avals = [jax.core.ShapedArray((S, E), ml_dtypes.bfloat16)]
    n_params = len(in_names)
    all_names = in_names + out_names
    partition_name = (nc.partition_id_tensor.name
                      if nc.partition_id_tensor is not None else None)
    if partition_name is not None:
        all_names = all_names + [partition_name]

    def _body(*args):
        operands = list(args)
        if partition_name is not None:
            operands.append(bass2jax.partition_id_tensor())
        outs = bass2jax._bass_exec_p.bind(
            *operands,
            out_avals=tuple(out_avals),
            in_names=tuple(all_names),
            out_names=tuple(out_names),
            lowering_input_output_aliases=(),
            sim_require_finite=True,
            sim_require_nnan=True,
            nc=nc,
        )
        return tuple(outs)

    devices = jax.devices()[:NCORES]
    mesh = Mesh(np.asarray(devices), ("core",))
    n_outs = len(out_names)
    in_specs = (PartitionSpec("core"),) * (n_params + n_outs)
    out_specs = (PartitionSpec("core"),) * n_outs
    sharded = jax.jit(
        shard_map(_body, mesh=mesh, in_specs=in_specs, out_specs=out_specs,
                  check_rep=False),
        donate_argnums=tuple(range(n_params, n_params + n_outs)),
        keep_unused=True,
    )
    global _SHARDED
    _SHARDED = sharded

    def run(in_maps):
        concat_in = [
            np.concatenate([np.asarray(in_maps[c][n]) for c in range(NCORES)],
                           axis=0)
            for n in in_names
        ]
        concat_zeros = [np.zeros((NCORES * S, E), ml_dtypes.bfloat16)]
        out_arrs = sharded(*concat_in, *concat_zeros)
        return np.asarray(out_arrs[0]).reshape(NCORES, S, E)

    _RUNNER = run
    return run


def kernel(x, w_qkv, w_out, b_out):
    x = np.asarray(x, dtype=np.float32)
    w_qkv = np.asarray(w_qkv, dtype=np.float32)
    w_out = np.asarray(w_out, dtype=np.float32)
    b_out = np.asarray(b_out, dtype=np.float32)

    bf = ml_dtypes.bfloat16
    in_maps = make_in_maps(x, w_qkv)
    for c in range(NCORES):
        g = c % 4
        cols = slice(g * M, (g + 1) * M)
        in_maps[c]["wo"] = np.ascontiguousarray(w_out[:, cols].T).astype(bf)

    run = _get_runner()
    partials = run(in_maps)  # [8, S, E] bf16

    out = np.empty((B, S, E), np.float32)
    for b in range(B):
        acc = partials[4 * b].astype(np.float64)
        for i in range(1, 4):
            acc += partials[4 * b + i].astype(np.float64)
        out[b] = (acc + b_out.astype(np.float64)).astype(np.float32)
    return out


# revision 63
# speedup vs baseline: 1.0570x; 1.0007x over previous
"""Multi-head causal self-attention (B=2, S=2048, E=1024, H=16, D=64) on 8
Trainium2 NeuronCores.

Sharding: batch x head-group. Core c handles batch (c // 4) and heads
[4*(c%4), 4*(c%4)+4). Each core computes QKV projection for its 4 heads,
causal flash-attention, and a partial output projection over its head
columns. Host sums the 4 partial outputs per batch and adds b_out.

v4 changes vs v3 (114.2us -> 108.1us cost-model span):
  - flipped ctx matmuls for ALL chunks: out = [q-part 128, 65] with
    lhsT = ex q-window (stationary), rhs = v1 slab. PE cost per
    (head, kb, qb) drops from `cols` to 65 rows (full 128-partition
    output): ctx 29us -> 15us. The softmax denominator lands
    per-q-PARTITION, so the norm is a cheap [128,nqb] reciprocal +
    per-partition-scalar multiplies (no Pool partition_broadcast).
    start=True only on each ctx tile's FIRST matmul: start marks the
    whole 2KB PSUM zero-region pending-zero, so per-q-block starts
    would corrupt sibling accumulations (lazy zero-on-first-write
    covers the other q-blocks).
  - ctx_qm [q, m] bf16 is transposed back to ctxT [m, q] with PE
    transposes (identity matmul, 128 rows each) + DVE copies; DMA/SP
    queues stay clear (in-order SP.SEQ head-of-line-blocks rearranges).
  - q projection for chunks 1-4 via fp8 DoubleRow (host-packed wq8,
    x8), mirroring the k8 path: 1/4 the PE cost of the bf16 proj.
  - stair masking of ex moved from DVE tensor_mul to Pool affine_select
    (SBUF->SBUF, one call covers both heads of a wave).
  - per-chunk qk8c/qk8p staging tiles: the dependency tracker flattens
    strided APs to byte ranges, so shared tiles made chunk ci's scores
    falsely wait on chunk ci+1's rearrange DMAs.
  - startup: fp8 operands + x8 load before the xt bulk; xt[1..3]/wot
    dma_starts deferred into the wave schedule (the serial DMA device
    processes in issue order, and the per-chunk rearranges gate the
    ACT-bound steady state); chunk1/2 fp8 prep pinned into chunk0's
    waves, per-mt rearranges so pair 0 isn't gated on mt1 projections.
  - tail: outproj spread so the last chunk's pieces land by wave 6;
    final q-block outproj mc0 halves pre-issued against ctxT[0] before
    the final norm; per-q-block output stores for the last two chunks.

The steady state is ACT-bound: exp processes every score element at
0.833ns/col (~58us) plus ~185ns/call init; PE sits at ~71%. Remaining
idle is the DMA-gated startup (~20us) and the ~6us drain tail.
"""

import sys

if "/opt/trn_rl_repo" not in sys.path:
    sys.path.insert(0, "/opt/trn_rl_repo")

import numpy as np
import ml_dtypes

import concourse.bacc as bacc
import concourse.mybir as mybir
import concourse.tile as tile

BF16 = mybir.dt.bfloat16
FP32 = mybir.dt.float32
F8 = mybir.dt.float8e4
DRMODE = mybir.MatmulPerfMode.DoubleRow

B, S, E = 2, 2048, 1024
H, DH = 16, 64
NCORES = 8
HPC = 4            # heads per core
M = HPC * DH       # 256 ctx columns per core
QC = 512           # q chunk (max wave width; also PSUM head stride)
KB = 128           # k block
SCALE = 1.0 / np.sqrt(DH)
NWARM = 64         # warmup dummy matmuls (128 cols each)
# q-chunks (q0, Q, grp). The last 512 splits 384+128 so the final
# norm/outproj tail is 4x smaller. grp = k-blocks per wave for the narrow
# final chunk (shares one exp call across 4 k-blocks).
CHUNKS = [(0, 512, 1), (512, 512, 1), (1024, 512, 1),
          (1536, 384, 1), (1920, 128, 4)]
NCH = len(CHUNKS)


def _emit_kernel(tc, xT, wq, wk, wv, wo_d, x8, wk8, wq8, out):
    nc = tc.nc
    Exp = mybir.ActivationFunctionType.Exp

    with tc.tile_pool(name="res", bufs=1) as res, \
         tc.tile_pool(name="ps", bufs=1, space="PSUM") as ps, \
         tc.tile_pool(name="expp", bufs=16) as expp, \
         tc.tile_pool(name="scr", bufs=4) as scr, \
         tc.tile_pool(name="cqm", bufs=2) as cqm_pool, \
         tc.tile_pool(name="outb", bufs=2) as outb:

        # ---- resident SBUF tiles ----
        xt_all = res.tile([128, 8 * S], BF16, name="xt_all")
        xt = [xt_all[:, e * S:(e + 1) * S] for e in range(8)]
        xt_3d = xt_all.rearrange("p (e s) -> p e s", s=S)
        wqt = res.tile([128, 8 * M], BF16, name="wqt")
        wkt = res.tile([128, 8 * M], BF16, name="wkt")
        wvt = res.tile([128, 8 * M], BF16, name="wvt")
        wot = [res.tile([128, E], BF16, name=f"wot{i}") for i in range(2)]
        qTt = [res.tile([128, S], BF16, name=f"qTt{i}") for i in range(2)]
        kTt = [res.tile([128, S], BF16, name=f"kTt{i}") for i in range(2)]
        ctxT = [res.tile([128, S], BF16, name=f"ctxT{i}") for i in range(2)]
        # fp8 scores path (queries >= 512): per chunk one classic-layout fp8
        # staging tile (free dims qk x mt x s) and one DoubleRow "pair" tile
        # [64, j x qk x mt x s] with head parity on partition halves {0,32}
        # and the two dh-32 k-tiles (j) in the free dim. PER-CHUNK tiles:
        # the dependency tracker flattens strided APs to byte ranges, so a
        # single shared tile makes chunk ci's scores falsely wait on chunk
        # ci+1's rearrange DMAs.
        qk8cs = [res.tile([128, 4 * CH[1]], F8, name=f"qk8c{i}")
                 for i, CH in enumerate(CHUNKS)]
        qk8c4s = [t.rearrange("p (t m s) -> p t m s", t=2, m=2)
                  for t in qk8cs]
        qk8ps = [res.tile([64, 8 * CH[1]], F8, name=f"qk8p{i}")
                 for i, CH in enumerate(CHUNKS)]
        qk8p5s = [t.rearrange("p (j t m s) -> p j t m s", j=2, t=2, m=2)
                  for t in qk8ps]

        def kb_loc(kb):
            # global k-block -> (chunk index, local column offset)
            for i in range(NCH - 1, -1, -1):
                if kb * 128 >= CHUNKS[i][0]:
                    return i, kb * 128 - CHUNKS[i][0]
            raise AssertionError
        # fp8 projection operands (host-packed e-pair layout): q/k columns
        # that are only ever consumed by the fp8 score path are projected
        # with fp8 DoubleRow matmuls at 1/4 the PE cost
        x8t = res.tile([128, 8 * S], F8, name="x8t")
        x8_4d = x8t.rearrange("p (j t s) -> p j t s", j=4, t=2)
        wk8t = res.tile([128, 8 * M], F8, name="wk8t")
        wk8_4d = wk8t.rearrange("p (j t m) -> p j t m", j=4, t=2)
        wq8t = res.tile([128, 8 * M], F8, name="wq8t")
        wq8_4d = wq8t.rearrange("p (j t m) -> p j t m", j=4, t=2)
        # V with ones column: per (k-block kb, head h) a [128, 65] slab
        v1 = res.tile([128, (S // KB) * HPC * 65], BF16, name="v1")
        v1_3d = v1.rearrange("p (n c) -> p n c", c=65)
        warm_src = res.tile([128, 128], BF16, name="warm_src")
        ident = res.tile([128, 128], BF16, name="ident")

        # ---- warmup: dummy matmuls keep the PE busy (and its p-state
        # ramping) through the DMA-gated startup.
        nc.gpsimd.memset(warm_src[:], 0.0)
        warm_ps = ps.tile([128, QC], FP32, tag="proj", bufs=2, name="warm_ps")
        for i in range(NWARM):
            nc.tensor.matmul(
                warm_ps[:, 0:128], lhsT=warm_src[:], rhs=warm_src[:],
                start=True, stop=True)

        # ---- input DMA: one batched transfer per tensor/chunk ----
        wqt_3d = wqt.rearrange("p (e m) -> p e m", m=M)
        wkt_3d = wkt.rearrange("p (e m) -> p e m", m=M)
        wvt_3d = wvt.rearrange("p (e m) -> p e m", m=M)
        xT_3d = xT.rearrange("(e p) s -> p e s", p=128)
        # order: chunk-0 bf16 operands first (pqi/pki), then the fp8
        # operands for ALL chunks (small; chunks 1+ exp work can only start
        # once q8/k8 are projected + rearranged, and that work is what keeps
        # the ACT engine fed during the remaining xt transfers), then the
        # bf16 x chunks (V projections, consumed later) and wo.
        nc.sync.dma_start(wqt_3d[:], wq.rearrange("(e p) m -> p e m", p=128))
        nc.sync.dma_start(xt_3d[:, :, 0:256], xT_3d[:, :, 0:256])
        nc.sync.dma_start(wkt_3d[:], wk.rearrange("(e p) m -> p e m", p=128))
        nc.sync.dma_start(xt_3d[:, :, 256:QC], xT_3d[:, :, 256:QC])
        x8_dram = x8.rearrange("p (j t s) -> p j t s", j=4, t=2)
        nc.sync.dma_start(wq8t[:], wq8)
        nc.sync.dma_start(wk8t[:], wk8)

        def emit_x8_load(chunk):
            nc.sync.dma_start(
                x8_4d[:, :, :, chunk * QC:(chunk + 1) * QC],
                x8_dram[:, :, :, chunk * QC:(chunk + 1) * QC])

        emit_x8_load(1)
        nc.sync.dma_start(wvt_3d[:], wv.rearrange("(e p) m -> p e m", p=128))
        emit_x8_load(2)
        emit_x8_load(3)

        # xt[1..3] and wot are consumed late (V projections of later chunks,
        # outproj). Their dma_starts are deferred into the wave schedule so
        # the per-chunk qk8p rearranges (which gate each chunk's scores and
        # hence the ACT-bound steady state) aren't queued behind them on the
        # serial DMA device.
        def emit_xt_load(chunk):
            nc.sync.dma_start(
                xt_3d[:, :, chunk * QC:(chunk + 1) * QC],
                xT_3d[:, :, chunk * QC:(chunk + 1) * QC])

        def emit_wot_load():
            for i in range(2):
                nc.sync.dma_start(wot[i][:], wo_d[i * 128:(i + 1) * 128, :])

        # ---- constants ----
        # exp-table warm first: the ~2.7us table load + warm call must not
        # queue behind the big v1 memset on the Pool engine
        warm = scr.tile([1, 1], FP32, tag="warm", bufs=1, name="warm")
        nc.gpsimd.memset(warm[:], 0.0)
        nc.scalar.activation(warm[:], warm[:],
                             mybir.ActivationFunctionType.Exp)
        nc.gpsimd.memset(v1[:], 1.0)  # data columns overwritten by V proj
        # identity matrix for PE transposes (keep where q_local == partition)
        nc.gpsimd.memset(ident[:], 1.0)
        nc.gpsimd.affine_select(
            out=ident[:], in_=ident[:],
            compare_op=mybir.AluOpType.is_equal,
            fill=0.0, base=0,
            pattern=[[1, 128]],
            channel_multiplier=-1,
        )

        # ---- emission helpers ----
        def stage_f8(ci, mt, kind, pqk, scale=None):
            # fp8 classic staging into the (qk, mt) slab of chunk ci's qk8c
            s0, Q, _ = CHUNKS[ci]
            t = 0 if kind == "q" else 1
            if scale is None:
                nc.vector.tensor_copy(qk8c4s[ci][:, t, mt, 0:Q], pqk[:, 0:Q])
            else:
                nc.vector.tensor_scalar_mul(qk8c4s[ci][:, t, mt, 0:Q],
                                            pqk[:, 0:Q], scale)

        def emit_rearrange(ci, mt, k_only=False):
            # partition rearrange into the DoubleRow pair tile for chunk
            # ci's mt slab (per-mt so pair 0's scores aren't gated on the
            # mt=1 projections)
            s0, Q, _ = CHUNKS[ci]
            t0 = 1 if k_only else 0
            for hh in range(2):
                for j in range(2):
                    nc.sync.dma_start(
                        qk8p5s[ci][32 * hh:32 * hh + 32, j, t0:2,
                                   mt:mt + 1, 0:Q],
                        qk8c4s[ci][64 * hh + 32 * j: 64 * hh + 32 * j + 32,
                                   t0:2, mt:mt + 1, 0:Q])

        def stage_qk(ci, mt, kind, pqk):
            # chunk 0 queries score in bf16 (classic layout); all other
            # queries score in fp8 DoubleRow. k is needed in fp8 by every
            # fp8 chunk, and in bf16 only for chunk 0's k-blocks.
            s0, Q, _ = CHUNKS[ci]
            dstt = qTt if kind == "q" else kTt
            if ci == 0:
                nc.vector.tensor_copy(dstt[mt][:, s0:s0 + Q], pqk[:, 0:Q])
            if kind == "k" or ci >= 1:
                stage_f8(ci, mt, kind, pqk)

        def emit_proj_qk8(ci, mt, kind):
            # q/k projection for fp8-only consumers via fp8 DoubleRow over
            # host-packed e-pairs: 1/4 the PE cost of the bf16 projection
            s0, Q, _ = CHUNKS[ci]
            w8 = wq8_4d if kind == "q" else wk8_4d
            pk = ps.tile([128, QC], FP32, tag="proj", bufs=2,
                         name=f"p8{kind}_{ci}_{mt}")
            # a-piece OUTER: interleaving two DoubleRow accumulation groups
            # (j inner per region) miscomputes on hardware -- each region's
            # 4-instruction group must run contiguously
            for a in range(0, Q, 256):
                b = min(a + 256, Q)
                for j in range(4):
                    nc.tensor.matmul(
                        pk[:, a:b],
                        lhsT=w8[:, j, :, mt * 128:(mt + 1) * 128],
                        rhs=x8_4d[:, j, :, s0 + a: s0 + b],
                        start=(j == 0), stop=(j == 3),
                        perf_mode=DRMODE)
            # w8 is host-scaled by 64 (w values ~0.02 sit in e4m3's
            # subnormal range, which the PE flushes to zero); undo here
            stage_f8(ci, mt, kind, pk, scale=1.0 / 64.0)

        def emit_proj_qk_interleaved(ci, mt):
            # startup projection: q first (wave 0 needs all 512 q columns),
            # then k in two pieces so wave 0 only gates on its first k-block
            s0, Q, _ = CHUNKS[ci]
            pq = ps.tile([128, QC], FP32, tag="proj", bufs=2,
                         name=f"pqi_{ci}_{mt}")
            pk = ps.tile([128, QC], FP32, tag="proj", bufs=2,
                         name=f"pki_{ci}_{mt}")
            pieces = [(pq, wqt, 0, 256), (pk, wkt, 0, 128),
                      (pq, wqt, 256, Q), (pk, wkt, 128, Q)]
            for dst, wt, a, b in pieces:
                for e in range(8):
                    nc.tensor.matmul(
                        dst[:, a:b],
                        lhsT=wt[:, e * M + mt * 128: e * M + (mt + 1) * 128],
                        rhs=xt[e][:, s0 + a: s0 + b],
                        start=(e == 0), stop=(e == 7))
                tgt = qTt if wt is wqt else kTt
                nc.vector.tensor_copy(tgt[mt][:, s0 + a: s0 + b],
                                      dst[:, a:b])
            stage_f8(ci, mt, "k", pk)

        def emit_proj_v(sblk):
            pv = ps.tile([128, M], FP32, tag="proj", bufs=2, name=f"pv_{sblk}")
            for e in range(8):
                nc.tensor.matmul(
                    pv[:],
                    lhsT=xt[e][:, sblk * 128:(sblk + 1) * 128],
                    rhs=wvt[:, e * M:(e + 1) * M],
                    start=(e == 0), stop=(e == 7))
            nc.vector.tensor_copy(
                v1_3d[:, sblk * HPC:(sblk + 1) * HPC, 0:64],
                pv[:].rearrange("p (h c) -> p h c", c=64))

        def proj_qk_pieces(ci):
            pcs = []
            for mt in range(2):
                pcs.append(lambda mt=mt: emit_proj_qk8(ci, mt, "q"))
                pcs.append(lambda mt=mt: emit_proj_qk8(ci, mt, "k"))
                pcs.append(lambda mt=mt: emit_rearrange(ci, mt))
            return pcs

        def proj_v_pieces(blks):
            return [lambda sb=sb: emit_proj_v(sb) for sb in blks]

        ob_tiles = {}
        out_3d = out.rearrange("(q p) f -> p q f", p=128)

        def emit_outproj(ci, qq, fc, tail=False):
            q0, Q, _ = CHUNKS[ci]
            nqb = Q // 128
            qb = q0 // 128 + qq
            last = ci == NCH - 1
            # last two chunks store per-q-block so the kernel-tail store
            # isn't queued behind one big merged transfer
            perqb = ci >= NCH - 2
            if qq == 0 and fc == 0:
                ob_tiles[ci] = outb.tile([128, nqb * E], BF16, tag="ob",
                                         name=f"ob_{qb}")
            ob = ob_tiles[ci]
            # tail outprojs borrow the scores PSUM slots (attention is done
            # by then), keeping mm->copy->mm free of slot serialization
            tag = "scores" if tail else "proj"
            po = ps.tile([128, QC], FP32, tag=tag, bufs=2,
                         name=f"po_{qb}_{fc}")
            for mc in range(2):
                nc.tensor.matmul(
                    po[:],
                    lhsT=ctxT[mc][:, qb * 128:(qb + 1) * 128],
                    rhs=wot[mc][:, fc * QC:(fc + 1) * QC],
                    start=(mc == 0), stop=(mc == 1))
            if last and fc == 1:
                # final piece: stage on the (idle by now) ACT engine so the
                # two last copies run in parallel instead of serializing on
                # the DVE queue
                nc.scalar.activation(
                    ob[:, qq * E + fc * QC: qq * E + (fc + 1) * QC], po[:],
                    mybir.ActivationFunctionType.Copy)
            else:
                nc.vector.tensor_copy(
                    ob[:, qq * E + fc * QC: qq * E + (fc + 1) * QC], po[:])
            if perqb:
                if fc == 1:
                    nc.sync.dma_start(
                        out[qb * 128:(qb + 1) * 128, :],
                        ob[:, qq * E: (qq + 1) * E])
                if (qq, fc) == (nqb - 1, 1):
                    del ob_tiles[ci]
            elif (qq, fc) == (nqb - 1, 1):
                nc.sync.dma_start(
                    out_3d[:, q0 // 128: q0 // 128 + nqb, :],
                    ob.rearrange("p (q f) -> p q f", f=E))
                del ob_tiles[ci]

        def outproj_pieces(ci, tail=False):
            _, Q, _ = CHUNKS[ci]
            return [lambda qq=qq, fc=fc: emit_outproj(ci, qq, fc, tail=tail)
                    for qq in range(Q // 128) for fc in range(2)]

        # ---- attention waves (one head PAIR, grp k-blocks) ----
        def wave_scores(ci, pair, g):
            s0, Q, grp = CHUNKS[ci]
            mt = pair
            fp8 = ci >= 1
            sc_ps = ps.tile([128, 2 * QC], FP32, tag="scores", bufs=2,
                            name=f"s_{ci}_{pair}_{g}")
            kds = [(j, (g * grp + j) * 128 - s0) for j in range(grp)]
            lo_e = 0
            for hh in range(2):
                r0 = hh * 64
                off = hh * QC
                for j, kd in kds:
                    kb = g * grp + j
                    # cols [0, kd) of this k-block's region are fully
                    # masked -> skip in scores. Only for grp == 1 (where
                    # the exp also skips them); grp > 1 diagonal waves
                    # compute the ~128 masked cols (27ns) so the exp never
                    # reads unwritten PSUM.
                    lo = kd if (kd >= 128 and grp == 1) else 0
                    if hh == 0 and kd >= 128 and grp == 1:
                        lo_e = kd
                    if fp8:
                        # DoubleRow: dh 2x32 k-tiles, head at base 32*hh;
                        # moving free = 2*w caps piece width at 256
                        ck, koff = kb_loc(kb)
                        a = lo
                        while a < Q:
                            b = min(a + 256, Q)
                            nc.tensor.matmul(
                                sc_ps[:, off + j * Q + a: off + j * Q + b],
                                lhsT=qk8p5s[ck][32 * hh:32 * hh + 32, :, 1,
                                                mt, koff:koff + 128],
                                rhs=qk8p5s[ci][32 * hh:32 * hh + 32, :, 0,
                                               mt, a:b],
                                start=True, stop=True,
                                perf_mode=DRMODE)
                            a = b
                    elif (pair, g) != (0, 0):
                        nc.tensor.matmul(
                            sc_ps[:, off + j * Q + lo: off + (j + 1) * Q],
                            lhsT=kTt[mt][r0:r0 + 64, kb * 128:(kb + 1) * 128],
                            rhs=qTt[mt][r0:r0 + 64, s0 + lo: s0 + Q],
                            start=True, stop=True)
            if not fp8 and (pair, g) == (0, 0):
                # very first wave: scores in column pieces matching the
                # split startup projection, COLUMN-outer / head-inner (the
                # PE wait queue is FIFO, so a blocked later-column piece
                # must not sit in front of a ready first-column piece)
                for a, b in [(0, 256), (256, Q)]:
                    for hh in range(2):
                        r0, off = hh * 64, hh * QC
                        nc.tensor.matmul(
                            sc_ps[:, off + a: off + b],
                            lhsT=kTt[mt][r0:r0 + 64, 0:128],
                            rhs=qTt[mt][r0:r0 + 64, a:b],
                            start=True, stop=True)
            ex = expp.tile([128, 2 * QC], BF16, tag="ex",
                           name=f"e_{ci}_{pair}_{g}")
            W = grp * Q
            ex3 = ex.rearrange("p (h q) -> p h q", h=2)
            sc3 = sc_ps.rearrange("p (h q) -> p h q", h=2)
            if (ci, pair, g) == (0, 0, 0):
                # very first wave: exp per (head, column-half) in contiguous
                # slices (a strided 2-head AP flattens to a byte range that
                # would falsely depend on the later column pieces)
                for a, b in [(0, 256), (256, W)]:
                    for hh in range(2):
                        nc.scalar.activation(ex3[:, hh, a:b],
                                             sc3[:, hh, a:b],
                                             Exp, scale=SCALE)
            elif lo_e or W < QC:
                # both heads in one strided-AP call: the ACT engine charges
                # by total free size, so this halves the per-call init cost
                # vs one call per head
                nc.scalar.activation(ex3[:, :, lo_e:W], sc3[:, :, lo_e:W],
                                     Exp, scale=SCALE)
            else:
                nc.scalar.activation(ex[:], sc_ps[:], Exp, scale=SCALE)
            # stair mask on the diagonal 128-block: zero ex where
            # k_local > q_local. Pool affine_select (SBUF->SBUF), one call
            # covers both heads: keep where (q_local - k_partition) >= 0.
            for j, kd in kds:
                if kd >= 0:
                    nc.gpsimd.affine_select(
                        out=ex3[:, :, j * Q + kd: j * Q + kd + 128],
                        in_=ex3[:, :, j * Q + kd: j * Q + kd + 128],
                        compare_op=mybir.AluOpType.is_ge,
                        fill=0.0, base=0,
                        pattern=[[0, 2], [1, 128]],
                        channel_multiplier=-1,
                    )
            return ex

        def wave_ctx_flip(ci, pair, g, ex, ctx_pair, nqb):
            # flipped ctx: lhsT = ex q-window (stationary), rhs = v1 slab,
            # out = [q-part 128, 65] accumulated over kb. qb's last
            # contribution is its diagonal block.
            # start=True ONLY on the tile's first matmul: start marks the
            # whole 2KB PSUM zero-region pending-zero, so a second start
            # would corrupt sibling q-blocks' accumulations. Later q-blocks'
            # first writes zero-on-first-write via that same pending flag.
            s0, Q, grp = CHUNKS[ci]
            qb_base = s0 // 128
            for hh in range(2):
                h = 2 * pair + hh
                off = hh * QC
                for j in range(grp):
                    kb = g * grp + j
                    kd = kb * 128 - s0
                    qb0 = max(0, kd // 128)
                    for qb in range(qb0, nqb):
                        nc.tensor.matmul(
                            ctx_pair[hh][:, qb * 128: qb * 128 + 65],
                            lhsT=ex[:, off + j * Q + qb * 128:
                                    off + j * Q + qb * 128 + 128],
                            rhs=v1_3d[:, kb * HPC + h, :],
                            start=(kb == 0 and qb == 0),
                            stop=(kb == qb_base + qb),
                            skip_group_check=True)

        def flip_norm_pieces(ci, pair, items):
            # flipped-ctx norm: per head a [128, nqb] reciprocal of the
            # per-q-partition denominators (col 64 of each qb slice), then
            # per (head, qb) a tensor_scalar multiply into the ctx_qm
            # staging tile, then one XBAR dma-transpose per q-block into
            # ctxT. Spread over the next waves.
            s0, Q, _ = CHUNKS[ci]
            nqb = Q // 128
            qb_lo = s0 // 128
            state = {}
            cqm = cqm_pool.tile([128, nqb * 128], BF16, tag="cqm",
                                name=f"cqm_{ci}_{pair}")

            def p_recips():
                state["recs"] = []
                for h, ctx_ps in items:
                    rec = scr.tile([128, 4], FP32, tag="rec",
                                   name=f"r_{ci}_{h}")
                    c3 = ctx_ps.rearrange("p (qb c) -> p qb c", c=128)
                    r3 = rec.rearrange("p (a b) -> p a b", b=1)
                    nc.vector.reciprocal(r3[:, 0:nqb, :],
                                         c3[:, 0:nqb, 64:65])
                    state["recs"].append(rec)

            def p_muls(i):
                h, ctx_ps = items[i]
                hh = h % 2
                rec = state["recs"][i]
                for qb in range(nqb):
                    nc.vector.tensor_scalar_mul(
                        cqm[:, qb * 128 + hh * 64: qb * 128 + hh * 64 + 64],
                        ctx_ps[:, qb * 128: qb * 128 + 64],
                        rec[:, qb:qb + 1])

            def p_transposes(qbs):
                # PE transpose (cheap: 128 rows each) into a bf16 PSUM tile
                # riding the proj slot rotation, then a DVE copy into ctxT.
                # Avoids the SP/HWDGE queue entirely (in-order SP.SEQ would
                # head-of-line-block later rearrange DMA issues).
                for qb in qbs:
                    tp = ps.tile([128, 128], BF16, tag="proj", bufs=2,
                                 name=f"tp_{ci}_{pair}_{qb}")
                    nc.tensor.transpose(
                        tp[:], cqm[:, qb * 128:(qb + 1) * 128], ident[:])
                    nc.vector.tensor_copy(
                        ctxT[pair][:, (qb_lo + qb) * 128:
                                   (qb_lo + qb + 1) * 128], tp[:])

            cut = min(2, nqb)
            return [p_recips, lambda: p_muls(0), lambda: p_muls(1),
                    lambda: p_transposes(range(0, cut)),
                    lambda: p_transposes(range(cut, nqb))]

        # ---- main schedule ----
        emit_proj_qk_interleaved(0, 0)
        pending_norms = []
        for ci in range(NCH):
            q0, Q, grp = CHUNKS[ci]
            nkb = (q0 + Q) // 128
            nqb = Q // 128
            ngrp = nkb // grp
            waves = [(pair, g) for pair in range(2)
                     for g in range(ngrp)]
            head = []   # pieces pinned to the earliest waves, one per wave
            extra = []  # pieces distributed evenly over all waves
            pins = {}   # wave -> pieces with exact placement constraints
            if ci == 0:
                # chunk 1's fp8 prep is pinned to the earliest waves so its
                # scores (the ACT feed during the xt input transfers) start
                # the moment x8[1] lands; chunk 2's prep spreads behind it
                # chunk 1's mt0 prep FIRST (ahead of chunk 0's mt1 startup
                # proj in the 2-slot proj PSUM rotation): it gates chunk 1's
                # scores, the main ACT feed once chunk 0's thin exps end
                qk1 = proj_qk_pieces(1)
                v0 = proj_v_pieces(range(0, 4))
                pins = {0: [qk1[0], qk1[1], qk1[2], v0[0],
                            lambda: emit_xt_load(1)],
                        1: [lambda: emit_proj_qk_interleaved(0, 1),
                            lambda: emit_rearrange(0, 0, k_only=True),
                            v0[1]],
                        2: [qk1[3], qk1[4], qk1[5], v0[2],
                            lambda: emit_rearrange(0, 1, k_only=True)],
                        3: [v0[3]]}
                extra += proj_qk_pieces(2) + [lambda: emit_xt_load(2)]
            elif ci == 1:
                head += proj_v_pieces(range(4, 8))
                extra += (proj_qk_pieces(3) + [lambda: emit_xt_load(3),
                                               emit_wot_load])
            elif ci == 2:
                head += proj_v_pieces(range(8, 12))
                extra += proj_qk_pieces(4) + outproj_pieces(0)
            elif ci == 3:
                head += proj_v_pieces(range(12, 16))
                extra += outproj_pieces(1) + outproj_pieces(2)
            else:
                extra += outproj_pieces(3)
            sched = {w: [] for w in range(len(waves))}
            for w, pcs in pins.items():
                sched[w].extend(pcs)
            for j, pc in enumerate(head):
                sched[j].append(pc)
            if extra:
                if ci == NCH - 1:
                    # outproj(NCH-2) pieces: no earlier than wave 4 (the
                    # previous pair's transpose pops land at waves 3-4) and
                    # packed 2/wave so the last store clears the tail
                    w0 = 4
                    span_w = len(waves) - w0 - 1
                else:
                    w0 = max(0, min(2 if ci == 0 else 4,
                                    len(waves) - len(extra)))
                    span_w = len(waves) - w0
                for j, pc in enumerate(extra):
                    sched[w0 + j * span_w // len(extra)].append(pc)

            ctx_tiles = {}
            ctx_queue = []
            for w, (pair, g) in enumerate(waves):
                if g == 0:
                    # one PSUM bank per head: [128, nqb*128-float slices],
                    # 65 floats used per qb slice
                    ctx_tiles[pair] = [
                        ps.tile([128, QC], FP32, tag="ctx", bufs=2,
                                name=f"c_{ci}_{pair}_{hh}")
                        for hh in range(2)]
                ex = wave_scores(ci, pair, g)
                if pending_norms:
                    pending_norms.pop(0)()
                last_of_pair = g == ngrp - 1
                final_pair = last_of_pair and pair == 1 and ci + 1 == NCH
                if not final_pair:
                    for pc in sched[w]:
                        pc()
                ctx_queue.append((pair, g, ex))
                # defer ctx so the PE has scores to run while exp catches
                # up; drain continuously (small lag) so the pair-end flush
                # is small and the norm reciprocal doesn't head-of-line-
                # block the in-order DVE queue.
                lag = max(0, 3 - g) if grp > 1 else 3
                while len(ctx_queue) > lag or \
                        (ctx_queue and last_of_pair):
                    qpair, qg, qex = ctx_queue.pop(0)
                    wave_ctx_flip(ci, qpair, qg, qex, ctx_tiles[qpair], nqb)
                if last_of_pair:
                    h0 = 2 * pair
                    items = [(h0 + hh, ctx_tiles[pair][hh])
                             for hh in range(2)]
                    while pending_norms:  # drain leftovers before reassign
                        pending_norms.pop(0)()
                    if final_pair:
                        # kernel tail: the final q-block's outproj mc0
                        # halves read ctxT[0] (ready since pair 0's norm),
                        # so emit them first — they run under the norm
                        # chain; only the mc1 halves wait on the final
                        # transpose. Then the norm chain ahead of this
                        # wave's filler pieces.
                        qbf = S // 128 - 1
                        po_t = []
                        for fc in range(2):
                            po = ps.tile([128, QC], FP32, tag="scores",
                                         bufs=2, name=f"pot_{fc}")
                            nc.tensor.matmul(
                                po[:],
                                lhsT=ctxT[0][:, qbf * 128:(qbf + 1) * 128],
                                rhs=wot[0][:, fc * QC:(fc + 1) * QC],
                                start=True, stop=False,
                                skip_group_check=True)
                            po_t.append(po)
                        for pc in flip_norm_pieces(ci, pair, items):
                            pc()
                        for pc in sched[w]:
                            pc()
                    else:
                        pending_norms = flip_norm_pieces(ci, pair, items)
        # ---- kernel tail: final q-block mc1 + staging + store ----
        qbf = S // 128 - 1
        ob_f = outb.tile([128, E], BF16, tag="ob", name="ob_f")
        for fc in range(2):
            nc.tensor.matmul(
                po_t[fc],
                lhsT=ctxT[1][:, qbf * 128:(qbf + 1) * 128],
                rhs=wot[1][:, fc * QC:(fc + 1) * QC],
                start=False, stop=True,
                skip_group_check=True)
        # stage the two halves on different engines so they run in parallel
        nc.vector.tensor_copy(ob_f[:, 0:QC], po_t[0][:])
        nc.scalar.activation(ob_f[:, QC:E], po_t[1][:],
                             mybir.ActivationFunctionType.Copy)
        nc.sync.dma_start(out[qbf * 128:(qbf + 1) * 128, :], ob_f[:])


def build_module():
    nc = bacc.Bacc("TRN2", target_bir_lowering=False, debug=False)
    xT = nc.dram_tensor("xT", [E, S], BF16, kind="ExternalInput").ap()
    wq = nc.dram_tensor("wq", [E, M], BF16, kind="ExternalInput").ap()
    wk = nc.dram_tensor("wk", [E, M], BF16, kind="ExternalInput").ap()
    wv = nc.dram_tensor("wv", [E, M], BF16, kind="ExternalInput").ap()
    wo = nc.dram_tensor("wo", [M, E], BF16, kind="ExternalInput").ap()
    x8 = nc.dram_tensor("x8", [128, 8 * S], F8, kind="ExternalInput").ap()
    wk8 = nc.dram_tensor("wk8", [128, 8 * M], F8, kind="ExternalInput").ap()
    wq8 = nc.dram_tensor("wq8", [128, 8 * M], F8, kind="ExternalInput").ap()
    out = nc.dram_tensor("out", [S, E], BF16, kind="ExternalOutput").ap()
    with tile.TileContext(nc) as tc:
        _emit_kernel(tc, xT, wq, wk, wv, wo, x8, wk8, wq8, out)
    nc.compile()
    return nc


def _pack_epairs(aT):
    """[E, N] -> [128, 4*2*N] fp8: e-tile pairs side by side per partition
    (DoubleRow packing: out[p, j, t, n] = aT[(2j+t)*128 + p, n])."""
    e4m3 = ml_dtypes.float8_e4m3
    E_, N = aT.shape
    a = np.asarray(aT, dtype=np.float32).reshape(4, 2, 128, N)
    a = np.ascontiguousarray(a.transpose(2, 0, 1, 3)).astype(e4m3)
    return a.reshape(128, 8 * N)


def make_in_maps(x, w_qkv):
    """Per-core input dicts (bf16/fp8, pre-transposed host-side)."""
    bf = ml_dtypes.bfloat16
    xTb = [np.ascontiguousarray(x[b].T).astype(bf) for b in range(B)]
    x8b = [_pack_epairs(x[b].T) for b in range(B)]
    in_maps = []
    for c in range(NCORES):
        b, g = c // 4, c % 4
        cols = slice(g * M, (g + 1) * M)
        wqT = np.ascontiguousarray(w_qkv[cols, :].T)
        wkT = np.ascontiguousarray(w_qkv[E:][cols, :].T)
        in_maps.append({
            "xT": xTb[b],
            "wq": wqT.astype(bf),
            "wk": wkT.astype(bf),
            "wv": np.ascontiguousarray(w_qkv[2 * E:][cols, :].T).astype(bf),
            "x8": x8b[b],
            "wk8": _pack_epairs(wkT * 64.0),
            "wq8": _pack_epairs(wqT * 64.0),
            "wo": None,  # filled in kernel(), needs w_out
        })
    return in_maps


_RUNNER = None
_SHARDED = None


def _get_runner():
    """Build the Bass module once and return a cached callable
    (in_maps) -> [NCORES, S, E] bf16 partial outputs."""
    global _RUNNER
    if _RUNNER is not None:
        return _RUNNER

    nc = build_module()

    from concourse import bass2jax
    import jax
    from jax.sharding import Mesh, PartitionSpec
    from jax.experimental.shard_map import shard_map

    bass2jax.install_neuronx_cc_hook()

    in_names = ["xT", "wq", "wk", "wv", "x8", "wk8", "wq8", "wo"]
    out_names = ["out"]
    out_avals = [jax.core.ShapedArray((S, E), ml_dtypes.bfloat16)]
    n_params = len(in_names)
    all_names = in_names + out_names
    partition_name = (nc.partition_id_tensor.name
                      if nc.partition_id_tensor is not None else None)
    if partition_name is not None:
        all_names = all_names + [partition_name]

    def _body(*args):
        operands = list(args)
        if partition_name is not None:
            operands.append(bass2jax.partition_id_tensor())
        outs = bass2jax._bass_exec_p.bind(
            *operands,
            out_avals=tuple(out_avals),
            in_names=tuple(all_names),
            out_names=tuple(out_names),
            lowering_input_output_aliases=(),
            sim_require_finite=True,
            sim_require_nnan=True,
            nc=nc,
        )
        return tuple(outs)

    devices = jax.devices()[:NCORES]
    mesh = Mesh(np.asarray(devices), ("core",))
    n_outs = len(out_names)
    in_specs = (PartitionSpec("core"),) * (n_params + n_outs)
    out_specs = (PartitionSpec("core"),) * n_outs
    sharded = jax.jit(
        shard_map(_body, mesh=mesh, in_specs=in_specs, out_specs=out_specs,
                  check_rep=False),
        donate_argnums=tuple(range(n_params, n_params + n_outs)),
        keep_unused=True,
    )
    global _SHARDED
    _SHARDED = sharded

    def run(in_maps):
        concat_in = [
            np.concatenate([np.asarray(in_maps[c][n]) for c in range(NCORES)],
                           axis=0)
            for n in in_names
        ]
        concat_zeros = [np.zeros((NCORES * S, E), ml_dtypes.bfloat16)]
        out_arrs = sharded(*concat_in, *concat_zeros)
        return np.asarray(out_arrs[0]).reshape(NCORES, S, E)

    _RUNNER = run
    return run


def kernel(x, w_qkv, w_out, b_out):
    x = np.asarray(x, dtype=np.float32)
    w_qkv = np.asarray(w_qkv, dtype=np.float32)
    w_out = np.asarray(w_out, dtype=np.float32)
    b_out = np.asarray(b_out, dtype=np.float32)

    bf = ml_dtypes.bfloat16
    in_maps = make_in_maps(x, w_qkv)
    for c in range(NCORES):
        g = c % 4
        cols = slice(g * M, (g + 1) * M)
        in_maps[c]["wo"] = np.ascontiguousarray(w_out[:, cols].T).astype(bf)

    run = _get_runner()
    partials = run(in_maps)  # [8, S, E] bf16

    out = np.empty((B, S, E), np.float32)
    for b in range(B):
        acc = partials[4 * b].astype(np.float64)
        for i in range(1, 4):
            acc += partials[4 * b + i].astype(np.float64)
        out[b] = (acc + b_out.astype(np.float64)).astype(np.float32)
    return out


# revision 73
# speedup vs baseline: 1.0572x; 1.0002x over previous
"""Multi-head causal self-attention (B=2, S=2048, E=1024, H=16, D=64) on 8
Trainium2 NeuronCores.

Sharding: batch x head-group. Core c handles batch (c // 4) and heads
[4*(c%4), 4*(c%4)+4). Each core computes QKV projection for its 4 heads,
causal flash-attention, and a partial output projection over its head
columns. Host sums the 4 partial outputs per batch and adds b_out.

v4 changes vs v3 (114.2us -> 108.1us cost-model span):
  - flipped ctx matmuls for ALL chunks: out = [q-part 128, 65] with
    lhsT = ex q-window (stationary), rhs = v1 slab. PE cost per
    (head, kb, qb) drops from `cols` to 65 rows (full 128-partition
    output): ctx 29us -> 15us. The softmax denominator lands
    per-q-PARTITION, so the norm is a cheap [128,nqb] reciprocal +
    per-partition-scalar multiplies (no Pool partition_broadcast).
    start=True only on each ctx tile's FIRST matmul: start marks the
    whole 2KB PSUM zero-region pending-zero, so per-q-block starts
    would corrupt sibling accumulations (lazy zero-on-first-write
    covers the other q-blocks).
  - ctx_qm [q, m] bf16 is transposed back to ctxT [m, q] with PE
    transposes (identity matmul, 128 rows each) + DVE copies; DMA/SP
    queues stay clear (in-order SP.SEQ head-of-line-blocks rearranges).
  - q projection for chunks 1-4 via fp8 DoubleRow (host-packed wq8,
    x8), mirroring the k8 path: 1/4 the PE cost of the bf16 proj.
  - stair masking of ex moved from DVE tensor_mul to Pool affine_select
    (SBUF->SBUF, one call covers both heads of a wave).
  - per-chunk qk8c/qk8p staging tiles: the dependency tracker flattens
    strided APs to byte ranges, so shared tiles made chunk ci's scores
    falsely wait on chunk ci+1's rearrange DMAs.
  - startup: fp8 operands + x8 load before the xt bulk; xt[1..3]/wot
    dma_starts deferred into the wave schedule (the serial DMA device
    processes in issue order, and the per-chunk rearranges gate the
    ACT-bound steady state); chunk1/2 fp8 prep pinned into chunk0's
    waves, per-mt rearranges so pair 0 isn't gated on mt1 projections.
  - tail: outproj spread so the last chunk's pieces land by wave 6;
    final q-block outproj mc0 halves pre-issued against ctxT[0] before
    the final norm; per-q-block output stores for the last two chunks.

The steady state is ACT-bound: exp processes every score element at
0.833ns/col (~58us) plus ~185ns/call init; PE sits at ~71%. Remaining
idle is the DMA-gated startup (~20us) and the ~6us drain tail.
"""

import sys

if "/opt/trn_rl_repo" not in sys.path:
    sys.path.insert(0, "/opt/trn_rl_repo")

import numpy as np
import ml_dtypes

import concourse.bacc as bacc
import concourse.mybir as mybir
import concourse.tile as tile

BF16 = mybir.dt.bfloat16
FP32 = mybir.dt.float32
F8 = mybir.dt.float8e4
DRMODE = mybir.MatmulPerfMode.DoubleRow

B, S, E = 2, 2048, 1024
H, DH = 16, 64
NCORES = 8
HPC = 4            # heads per core
M = HPC * DH       # 256 ctx columns per core
QC = 512           # q chunk (max wave width; also PSUM head stride)
KB = 128           # k block
SCALE = 1.0 / np.sqrt(DH)
NWARM = 64         # warmup dummy matmuls (128 cols each)
# q-chunks (q0, Q, grp). The last 512 splits 384+128 so the final
# norm/outproj tail is 4x smaller. grp = k-blocks per wave for the narrow
# final chunk (shares one exp call across 4 k-blocks).
CHUNKS = [(0, 512, 1), (512, 512, 1), (1024, 512, 1),
          (1536, 384, 1), (1920, 128, 4)]
NCH = len(CHUNKS)


def _emit_kernel(tc, xT, wq, wk, wv, wo_d, x8, wk8, wq8, out):
    nc = tc.nc
    Exp = mybir.ActivationFunctionType.Exp

    with tc.tile_pool(name="res", bufs=1) as res, \
         tc.tile_pool(name="ps", bufs=1, space="PSUM") as ps, \
         tc.tile_pool(name="expp", bufs=16) as expp, \
         tc.tile_pool(name="scr", bufs=4) as scr, \
         tc.tile_pool(name="cqm", bufs=2) as cqm_pool, \
         tc.tile_pool(name="outb", bufs=2) as outb:

        # ---- resident SBUF tiles ----
        xt_all = res.tile([128, 8 * S], BF16, name="xt_all")
        xt = [xt_all[:, e * S:(e + 1) * S] for e in range(8)]
        xt_3d = xt_all.rearrange("p (e s) -> p e s", s=S)
        wqt = res.tile([128, 8 * M], BF16, name="wqt")
        wkt = res.tile([128, 8 * M], BF16, name="wkt")
        wvt = res.tile([128, 8 * M], BF16, name="wvt")
        wot = [res.tile([128, E], BF16, name=f"wot{i}") for i in range(2)]
        qTt = [res.tile([128, S], BF16, name=f"qTt{i}") for i in range(2)]
        kTt = [res.tile([128, S], BF16, name=f"kTt{i}") for i in range(2)]
        ctxT = [res.tile([128, S], BF16, name=f"ctxT{i}") for i in range(2)]
        # fp8 scores path (queries >= 512): per chunk one classic-layout fp8
        # staging tile (free dims qk x mt x s) and one DoubleRow "pair" tile
        # [64, j x qk x mt x s] with head parity on partition halves {0,32}
        # and the two dh-32 k-tiles (j) in the free dim. PER-CHUNK tiles:
        # the dependency tracker flattens strided APs to byte ranges, so a
        # single shared tile makes chunk ci's scores falsely wait on chunk
        # ci+1's rearrange DMAs.
        qk8cs = [res.tile([128, 4 * CH[1]], F8, name=f"qk8c{i}")
                 for i, CH in enumerate(CHUNKS)]
        qk8c4s = [t.rearrange("p (t m s) -> p t m s", t=2, m=2)
                  for t in qk8cs]
        qk8ps = [res.tile([64, 8 * CH[1]], F8, name=f"qk8p{i}")
                 for i, CH in enumerate(CHUNKS)]
        qk8p5s = [t.rearrange("p (j t m s) -> p j t m s", j=2, t=2, m=2)
                  for t in qk8ps]

        def kb_loc(kb):
            # global k-block -> (chunk index, local column offset)
            for i in range(NCH - 1, -1, -1):
                if kb * 128 >= CHUNKS[i][0]:
                    return i, kb * 128 - CHUNKS[i][0]
            raise AssertionError
        # fp8 projection operands (host-packed e-pair layout): q/k columns
        # that are only ever consumed by the fp8 score path are projected
        # with fp8 DoubleRow matmuls at 1/4 the PE cost
        x8t = res.tile([128, 8 * S], F8, name="x8t")
        x8_4d = x8t.rearrange("p (j t s) -> p j t s", j=4, t=2)
        wk8t = res.tile([128, 8 * M], F8, name="wk8t")
        wk8_4d = wk8t.rearrange("p (j t m) -> p j t m", j=4, t=2)
        wq8t = res.tile([128, 8 * M], F8, name="wq8t")
        wq8_4d = wq8t.rearrange("p (j t m) -> p j t m", j=4, t=2)
        # V with ones column: per (k-block kb, head h) a [128, 65] slab
        v1 = res.tile([128, (S // KB) * HPC * 65], BF16, name="v1")
        v1_3d = v1.rearrange("p (n c) -> p n c", c=65)
        warm_src = res.tile([128, 128], BF16, name="warm_src")
        ident = res.tile([128, 128], BF16, name="ident")

        # ---- warmup: dummy matmuls keep the PE busy (and its p-state
        # ramping) through the DMA-gated startup.
        nc.gpsimd.memset(warm_src[:], 0.0)
        warm_ps = ps.tile([128, QC], FP32, tag="proj", bufs=2, name="warm_ps")
        for i in range(NWARM):
            nc.tensor.matmul(
                warm_ps[:, 0:128], lhsT=warm_src[:], rhs=warm_src[:],
                start=True, stop=True)

        # ---- input DMA: one batched transfer per tensor/chunk ----
        wqt_3d = wqt.rearrange("p (e m) -> p e m", m=M)
        wkt_3d = wkt.rearrange("p (e m) -> p e m", m=M)
        wvt_3d = wvt.rearrange("p (e m) -> p e m", m=M)
        xT_3d = xT.rearrange("(e p) s -> p e s", p=128)
        # order: chunk-0 bf16 operands first (pqi/pki), then the fp8
        # operands for ALL chunks (small; chunks 1+ exp work can only start
        # once q8/k8 are projected + rearranged, and that work is what keeps
        # the ACT engine fed during the remaining xt transfers), then the
        # bf16 x chunks (V projections, consumed later) and wo.
        nc.sync.dma_start(wqt_3d[:], wq.rearrange("(e p) m -> p e m", p=128))
        nc.sync.dma_start(xt_3d[:, :, 0:256], xT_3d[:, :, 0:256])
        nc.sync.dma_start(wkt_3d[:], wk.rearrange("(e p) m -> p e m", p=128))
        nc.sync.dma_start(xt_3d[:, :, 256:QC], xT_3d[:, :, 256:QC])
        x8_dram = x8.rearrange("p (j t s) -> p j t s", j=4, t=2)
        nc.sync.dma_start(wq8t[:], wq8)
        nc.sync.dma_start(wk8t[:], wk8)

        def emit_x8_load(chunk):
            nc.sync.dma_start(
                x8_4d[:, :, :, chunk * QC:(chunk + 1) * QC],
                x8_dram[:, :, :, chunk * QC:(chunk + 1) * QC])

        emit_x8_load(1)
        nc.sync.dma_start(wvt_3d[:], wv.rearrange("(e p) m -> p e m", p=128))
        emit_x8_load(2)
        emit_x8_load(3)

        # xt[1..3] and wot are consumed late (V projections of later chunks,
        # outproj). Their dma_starts are deferred into the wave schedule so
        # the per-chunk qk8p rearranges (which gate each chunk's scores and
        # hence the ACT-bound steady state) aren't queued behind them on the
        # serial DMA device.
        def emit_xt_load(chunk):
            nc.sync.dma_start(
                xt_3d[:, :, chunk * QC:(chunk + 1) * QC],
                xT_3d[:, :, chunk * QC:(chunk + 1) * QC])

        def emit_wot_load():
            for i in range(2):
                nc.sync.dma_start(wot[i][:], wo_d[i * 128:(i + 1) * 128, :])

        # ---- constants ----
        # exp-table warm first: the ~2.7us table load + warm call must not
        # queue behind the big v1 memset on the Pool engine
        warm = scr.tile([1, 1], FP32, tag="warm", bufs=1, name="warm")
        nc.gpsimd.memset(warm[:], 0.0)
        nc.scalar.activation(warm[:], warm[:],
                             mybir.ActivationFunctionType.Exp)
        nc.gpsimd.memset(v1[:], 1.0)  # data columns overwritten by V proj
        # identity matrix for PE transposes (keep where q_local == partition)
        nc.gpsimd.memset(ident[:], 1.0)
        nc.gpsimd.affine_select(
            out=ident[:], in_=ident[:],
            compare_op=mybir.AluOpType.is_equal,
            fill=0.0, base=0,
            pattern=[[1, 128]],
            channel_multiplier=-1,
        )

        # ---- emission helpers ----
        def stage_f8(ci, mt, kind, pqk, scale=None):
            # fp8 classic staging into the (qk, mt) slab of chunk ci's qk8c
            s0, Q, _ = CHUNKS[ci]
            t = 0 if kind == "q" else 1
            if scale is None:
                nc.vector.tensor_copy(qk8c4s[ci][:, t, mt, 0:Q], pqk[:, 0:Q])
            else:
                nc.vector.tensor_scalar_mul(qk8c4s[ci][:, t, mt, 0:Q],
                                            pqk[:, 0:Q], scale)

        def emit_rearrange(ci, mt, k_only=False):
            # partition rearrange into the DoubleRow pair tile for chunk
            # ci's mt slab (per-mt so pair 0's scores aren't gated on the
            # mt=1 projections)
            s0, Q, _ = CHUNKS[ci]
            t0 = 1 if k_only else 0
            for hh in range(2):
                for j in range(2):
                    nc.sync.dma_start(
                        qk8p5s[ci][32 * hh:32 * hh + 32, j, t0:2,
                                   mt:mt + 1, 0:Q],
                        qk8c4s[ci][64 * hh + 32 * j: 64 * hh + 32 * j + 32,
                                   t0:2, mt:mt + 1, 0:Q])

        def stage_qk(ci, mt, kind, pqk):
            # chunk 0 queries score in bf16 (classic layout); all other
            # queries score in fp8 DoubleRow. k is needed in fp8 by every
            # fp8 chunk, and in bf16 only for chunk 0's k-blocks.
            s0, Q, _ = CHUNKS[ci]
            dstt = qTt if kind == "q" else kTt
            if ci == 0:
                nc.vector.tensor_copy(dstt[mt][:, s0:s0 + Q], pqk[:, 0:Q])
            if kind == "k" or ci >= 1:
                stage_f8(ci, mt, kind, pqk)

        def emit_proj_qk8(ci, mt, kind):
            # q/k projection for fp8-only consumers via fp8 DoubleRow over
            # host-packed e-pairs: 1/4 the PE cost of the bf16 projection
            s0, Q, _ = CHUNKS[ci]
            w8 = wq8_4d if kind == "q" else wk8_4d
            pk = ps.tile([128, QC], FP32, tag="proj", bufs=2,
                         name=f"p8{kind}_{ci}_{mt}")
            # a-piece OUTER: interleaving two DoubleRow accumulation groups
            # (j inner per region) miscomputes on hardware -- each region's
            # 4-instruction group must run contiguously
            for a in range(0, Q, 256):
                b = min(a + 256, Q)
                for j in range(4):
                    nc.tensor.matmul(
                        pk[:, a:b],
                        lhsT=w8[:, j, :, mt * 128:(mt + 1) * 128],
                        rhs=x8_4d[:, j, :, s0 + a: s0 + b],
                        start=(j == 0), stop=(j == 3),
                        perf_mode=DRMODE)
            # w8 is host-scaled by 64 (w values ~0.02 sit in e4m3's
            # subnormal range, which the PE flushes to zero); undo here
            stage_f8(ci, mt, kind, pk, scale=1.0 / 64.0)

        def emit_proj_qk_interleaved(ci, mt):
            # startup projection: q first (wave 0 needs all 512 q columns),
            # then k in two pieces so wave 0 only gates on its first k-block
            s0, Q, _ = CHUNKS[ci]
            pq = ps.tile([128, QC], FP32, tag="proj", bufs=2,
                         name=f"pqi_{ci}_{mt}")
            pk = ps.tile([128, QC], FP32, tag="proj", bufs=2,
                         name=f"pki_{ci}_{mt}")
            pieces = [(pq, wqt, 0, 256), (pk, wkt, 0, 128),
                      (pq, wqt, 256, Q), (pk, wkt, 128, Q)]
            for dst, wt, a, b in pieces:
                for e in range(8):
                    nc.tensor.matmul(
                        dst[:, a:b],
                        lhsT=wt[:, e * M + mt * 128: e * M + (mt + 1) * 128],
                        rhs=xt[e][:, s0 + a: s0 + b],
                        start=(e == 0), stop=(e == 7))
                tgt = qTt if wt is wqt else kTt
                nc.vector.tensor_copy(tgt[mt][:, s0 + a: s0 + b],
                                      dst[:, a:b])
            stage_f8(ci, mt, "k", pk)

        def emit_proj_v(sblk):
            pv = ps.tile([128, M], FP32, tag="proj", bufs=2, name=f"pv_{sblk}")
            for e in range(8):
                nc.tensor.matmul(
                    pv[:],
                    lhsT=xt[e][:, sblk * 128:(sblk + 1) * 128],
                    rhs=wvt[:, e * M:(e + 1) * M],
                    start=(e == 0), stop=(e == 7))
            nc.vector.tensor_copy(
                v1_3d[:, sblk * HPC:(sblk + 1) * HPC, 0:64],
                pv[:].rearrange("p (h c) -> p h c", c=64))

        def proj_qk_pieces(ci):
            pcs = []
            for mt in range(2):
                pcs.append(lambda mt=mt: emit_proj_qk8(ci, mt, "q"))
                pcs.append(lambda mt=mt: emit_proj_qk8(ci, mt, "k"))
                pcs.append(lambda mt=mt: emit_rearrange(ci, mt))
            return pcs

        def proj_v_pieces(blks):
            return [lambda sb=sb: emit_proj_v(sb) for sb in blks]

        ob_tiles = {}
        out_3d = out.rearrange("(q p) f -> p q f", p=128)

        def emit_outproj(ci, qq, fc, tail=False):
            q0, Q, _ = CHUNKS[ci]
            nqb = Q // 128
            qb = q0 // 128 + qq
            last = ci == NCH - 1
            # last two chunks store per-q-block so the kernel-tail store
            # isn't queued behind one big merged transfer
            perqb = ci >= NCH - 2
            if qq == 0 and fc == 0:
                ob_tiles[ci] = outb.tile([128, nqb * E], BF16, tag="ob",
                                         name=f"ob_{qb}")
            ob = ob_tiles[ci]
            # tail outprojs borrow the scores PSUM slots (attention is done
            # by then), keeping mm->copy->mm free of slot serialization
            tag = "scores" if tail else "proj"
            po = ps.tile([128, QC], FP32, tag=tag, bufs=2,
                         name=f"po_{qb}_{fc}")
            for mc in range(2):
                nc.tensor.matmul(
                    po[:],
                    lhsT=ctxT[mc][:, qb * 128:(qb + 1) * 128],
                    rhs=wot[mc][:, fc * QC:(fc + 1) * QC],
                    start=(mc == 0), stop=(mc == 1))
            if last and fc == 1:
                # final piece: stage on the (idle by now) ACT engine so the
                # two last copies run in parallel instead of serializing on
                # the DVE queue
                nc.scalar.activation(
                    ob[:, qq * E + fc * QC: qq * E + (fc + 1) * QC], po[:],
                    mybir.ActivationFunctionType.Copy)
            else:
                nc.vector.tensor_copy(
                    ob[:, qq * E + fc * QC: qq * E + (fc + 1) * QC], po[:])
            if perqb:
                if fc == 1:
                    nc.sync.dma_start(
                        out[qb * 128:(qb + 1) * 128, :],
                        ob[:, qq * E: (qq + 1) * E])
                if (qq, fc) == (nqb - 1, 1):
                    del ob_tiles[ci]
            elif (qq, fc) == (nqb - 1, 1):
                nc.sync.dma_start(
                    out_3d[:, q0 // 128: q0 // 128 + nqb, :],
                    ob.rearrange("p (q f) -> p q f", f=E))
                del ob_tiles[ci]

        def outproj_pieces(ci, tail=False):
            _, Q, _ = CHUNKS[ci]
            return [lambda qq=qq, fc=fc: emit_outproj(ci, qq, fc, tail=tail)
                    for qq in range(Q // 128) for fc in range(2)]

        # ---- attention waves (one head PAIR, grp k-blocks) ----
        def wave_scores(ci, pair, g):
            s0, Q, grp = CHUNKS[ci]
            mt = pair
            fp8 = ci >= 1
            sc_ps = ps.tile([128, 2 * QC], FP32, tag="scores", bufs=2,
                            name=f"s_{ci}_{pair}_{g}")
            kds = [(j, (g * grp + j) * 128 - s0) for j in range(grp)]
            lo_e = 0
            for hh in range(2):
                r0 = hh * 64
                off = hh * QC
                for j, kd in kds:
                    kb = g * grp + j
                    # cols [0, kd) of this k-block's region are fully
                    # masked -> skip in scores. Only for grp == 1 (where
                    # the exp also skips them); grp > 1 diagonal waves
                    # compute the ~128 masked cols (27ns) so the exp never
                    # reads unwritten PSUM.
                    lo = kd if (kd >= 128 and grp == 1) else 0
                    if hh == 0 and kd >= 128 and grp == 1:
                        lo_e = kd
                    if fp8:
                        # DoubleRow: dh 2x32 k-tiles, head at base 32*hh;
                        # moving free = 2*w caps piece width at 256
                        ck, koff = kb_loc(kb)
                        a = lo
                        while a < Q:
                            b = min(a + 256, Q)
                            nc.tensor.matmul(
                                sc_ps[:, off + j * Q + a: off + j * Q + b],
                                lhsT=qk8p5s[ck][32 * hh:32 * hh + 32, :, 1,
                                                mt, koff:koff + 128],
                                rhs=qk8p5s[ci][32 * hh:32 * hh + 32, :, 0,
                                               mt, a:b],
                                start=True, stop=True,
                                perf_mode=DRMODE)
                            a = b
                    elif (pair, g) != (0, 0):
                        nc.tensor.matmul(
                            sc_ps[:, off + j * Q + lo: off + (j + 1) * Q],
                            lhsT=kTt[mt][r0:r0 + 64, kb * 128:(kb + 1) * 128],
                            rhs=qTt[mt][r0:r0 + 64, s0 + lo: s0 + Q],
                            start=True, stop=True)
            if not fp8 and (pair, g) == (0, 0):
                # very first wave: scores in column pieces matching the
                # split startup projection, COLUMN-outer / head-inner (the
                # PE wait queue is FIFO, so a blocked later-column piece
                # must not sit in front of a ready first-column piece)
                for a, b in [(0, 256), (256, Q)]:
                    for hh in range(2):
                        r0, off = hh * 64, hh * QC
                        nc.tensor.matmul(
                            sc_ps[:, off + a: off + b],
                            lhsT=kTt[mt][r0:r0 + 64, 0:128],
                            rhs=qTt[mt][r0:r0 + 64, a:b],
                            start=True, stop=True)
            ex = expp.tile([128, 2 * QC], BF16, tag="ex",
                           name=f"e_{ci}_{pair}_{g}")
            W = grp * Q
            ex3 = ex.rearrange("p (h q) -> p h q", h=2)
            sc3 = sc_ps.rearrange("p (h q) -> p h q", h=2)
            if (ci, pair, g) == (0, 0, 0):
                # very first wave: exp per (head, column-half) in contiguous
                # slices (a strided 2-head AP flattens to a byte range that
                # would falsely depend on the later column pieces)
                for a, b in [(0, 256), (256, W)]:
                    for hh in range(2):
                        nc.scalar.activation(ex3[:, hh, a:b],
                                             sc3[:, hh, a:b],
                                             Exp, scale=SCALE)
            elif lo_e or W < QC:
                # both heads in one strided-AP call: the ACT engine charges
                # by total free size, so this halves the per-call init cost
                # vs one call per head
                nc.scalar.activation(ex3[:, :, lo_e:W], sc3[:, :, lo_e:W],
                                     Exp, scale=SCALE)
            else:
                nc.scalar.activation(ex[:], sc_ps[:], Exp, scale=SCALE)
            # stair mask on the diagonal 128-block: zero ex where
            # k_local > q_local. Pool affine_select (SBUF->SBUF), one call
            # covers both heads: keep where (q_local - k_partition) >= 0.
            for j, kd in kds:
                if kd >= 0:
                    nc.gpsimd.affine_select(
                        out=ex3[:, :, j * Q + kd: j * Q + kd + 128],
                        in_=ex3[:, :, j * Q + kd: j * Q + kd + 128],
                        compare_op=mybir.AluOpType.is_ge,
                        fill=0.0, base=0,
                        pattern=[[0, 2], [1, 128]],
                        channel_multiplier=-1,
                    )
            return ex

        def wave_ctx_flip(ci, pair, g, ex, ctx_pair, nqb):
            # flipped ctx: lhsT = ex q-window (stationary), rhs = v1 slab,
            # out = [q-part 128, 65] accumulated over kb. qb's last
            # contribution is its diagonal block.
            # start=True ONLY on the tile's first matmul: start marks the
            # whole 2KB PSUM zero-region pending-zero, so a second start
            # would corrupt sibling q-blocks' accumulations. Later q-blocks'
            # first writes zero-on-first-write via that same pending flag.
            s0, Q, grp = CHUNKS[ci]
            qb_base = s0 // 128
            for hh in range(2):
                h = 2 * pair + hh
                off = hh * QC
                for j in range(grp):
                    kb = g * grp + j
                    kd = kb * 128 - s0
                    qb0 = max(0, kd // 128)
                    for qb in range(qb0, nqb):
                        nc.tensor.matmul(
                            ctx_pair[hh][:, qb * 128: qb * 128 + 65],
                            lhsT=ex[:, off + j * Q + qb * 128:
                                    off + j * Q + qb * 128 + 128],
                            rhs=v1_3d[:, kb * HPC + h, :],
                            start=(kb == 0 and qb == 0),
                            stop=(kb == qb_base + qb),
                            skip_group_check=True)

        def flip_norm_pieces(ci, pair, items):
            # flipped-ctx norm: per head a [128, nqb] reciprocal of the
            # per-q-partition denominators (col 64 of each qb slice), then
            # per (head, qb) a tensor_scalar multiply into the ctx_qm
            # staging tile, then one XBAR dma-transpose per q-block into
            # ctxT. Spread over the next waves.
            s0, Q, _ = CHUNKS[ci]
            nqb = Q // 128
            qb_lo = s0 // 128
            state = {}
            cqm = cqm_pool.tile([128, nqb * 128], BF16, tag="cqm",
                                name=f"cqm_{ci}_{pair}")

            def p_recips():
                state["recs"] = []
                for h, ctx_ps in items:
                    rec = scr.tile([128, 4], FP32, tag="rec",
                                   name=f"r_{ci}_{h}")
                    c3 = ctx_ps.rearrange("p (qb c) -> p qb c", c=128)
                    r3 = rec.rearrange("p (a b) -> p a b", b=1)
                    nc.vector.reciprocal(r3[:, 0:nqb, :],
                                         c3[:, 0:nqb, 64:65])
                    state["recs"].append(rec)

            def p_muls(i):
                h, ctx_ps = items[i]
                hh = h % 2
                rec = state["recs"][i]
                for qb in range(nqb):
                    nc.vector.tensor_scalar_mul(
                        cqm[:, qb * 128 + hh * 64: qb * 128 + hh * 64 + 64],
                        ctx_ps[:, qb * 128: qb * 128 + 64],
                        rec[:, qb:qb + 1])

            def p_transposes(qbs):
                # PE transpose (cheap: 128 rows each) into a bf16 PSUM tile
                # riding the proj slot rotation, then a DVE copy into ctxT.
                # Avoids the SP/HWDGE queue entirely (in-order SP.SEQ would
                # head-of-line-block later rearrange DMA issues).
                for qb in qbs:
                    tp = ps.tile([128, 128], BF16, tag="proj", bufs=2,
                                 name=f"tp_{ci}_{pair}_{qb}")
                    nc.tensor.transpose(
                        tp[:], cqm[:, qb * 128:(qb + 1) * 128], ident[:])
                    nc.vector.tensor_copy(
                        ctxT[pair][:, (qb_lo + qb) * 128:
                                   (qb_lo + qb + 1) * 128], tp[:])

            cut = min(2, nqb)
            return [p_recips, lambda: p_muls(0), lambda: p_muls(1),
                    lambda: p_transposes(range(0, cut)),
                    lambda: p_transposes(range(cut, nqb))]

        # ---- main schedule ----
        emit_proj_qk_interleaved(0, 0)
        pending_norms = []
        for ci in range(NCH):
            q0, Q, grp = CHUNKS[ci]
            nkb = (q0 + Q) // 128
            nqb = Q // 128
            ngrp = nkb // grp
            waves = [(pair, g) for pair in range(2)
                     for g in range(ngrp)]
            head = []   # pieces pinned to the earliest waves, one per wave
            extra = []  # pieces distributed evenly over all waves
            pins = {}   # wave -> pieces with exact placement constraints
            if ci == 0:
                # chunk 1's fp8 prep is pinned to the earliest waves so its
                # scores (the ACT feed during the xt input transfers) start
                # the moment x8[1] lands; chunk 2's prep spreads behind it
                # chunk 1's mt0 prep FIRST (ahead of chunk 0's mt1 startup
                # proj in the 2-slot proj PSUM rotation): it gates chunk 1's
                # scores, the main ACT feed once chunk 0's thin exps end
                qk1 = proj_qk_pieces(1)
                v0 = proj_v_pieces(range(0, 4))
                pins = {0: [qk1[0], qk1[1], qk1[2], v0[0],
                            lambda: emit_xt_load(1)],
                        1: [lambda: emit_proj_qk_interleaved(0, 1),
                            lambda: emit_rearrange(0, 0, k_only=True),
                            v0[1]],
                        2: [qk1[3], qk1[4], qk1[5], v0[2],
                            lambda: emit_rearrange(0, 1, k_only=True)],
                        3: [v0[3]]}
                extra += proj_qk_pieces(2) + [lambda: emit_xt_load(2)]
            elif ci == 1:
                head += proj_v_pieces(range(4, 8))
                extra += (proj_qk_pieces(3) + [lambda: emit_xt_load(3),
                                               emit_wot_load])
            elif ci == 2:
                head += proj_v_pieces(range(8, 12))
                extra += proj_qk_pieces(4) + outproj_pieces(0)
            elif ci == 3:
                head += proj_v_pieces(range(12, 16))
                extra += outproj_pieces(1) + outproj_pieces(2)
            else:
                extra += outproj_pieces(3)
            sched = {w: [] for w in range(len(waves))}
            for w, pcs in pins.items():
                sched[w].extend(pcs)
            for j, pc in enumerate(head):
                sched[j].append(pc)
            if extra:
                if ci == NCH - 1:
                    # outproj(NCH-2) pieces: no earlier than wave 4 (the
                    # previous pair's transpose pops land at waves 3-4) and
                    # packed 2/wave so the last store clears the tail
                    w0 = 4
                    span_w = len(waves) - w0 - 1
                else:
                    w0 = 2 if ci == 0 else max(0, min(4, len(waves)
                                                      - len(extra)))
                    span_w = len(waves) - w0
                for j, pc in enumerate(extra):
                    sched[w0 + j * span_w // len(extra)].append(pc)

            ctx_tiles = {}
            ctx_queue = []
            for w, (pair, g) in enumerate(waves):
                if g == 0:
                    # one PSUM bank per head: [128, nqb*128-float slices],
                    # 65 floats used per qb slice
                    ctx_tiles[pair] = [
                        ps.tile([128, QC], FP32, tag="ctx", bufs=2,
                                name=f"c_{ci}_{pair}_{hh}")
                        for hh in range(2)]
                ex = wave_scores(ci, pair, g)
                if pending_norms:
                    pending_norms.pop(0)()
                last_of_pair = g == ngrp - 1
                final_pair = last_of_pair and pair == 1 and ci + 1 == NCH
                if not final_pair:
                    for pc in sched[w]:
                        pc()
                ctx_queue.append((pair, g, ex))
                # defer ctx so the PE has scores to run while exp catches
                # up; drain continuously (small lag) so the pair-end flush
                # is small and the norm reciprocal doesn't head-of-line-
                # block the in-order DVE queue.
                lag = max(0, 3 - g) if grp > 1 else 3
                while len(ctx_queue) > lag or \
                        (ctx_queue and last_of_pair):
                    qpair, qg, qex = ctx_queue.pop(0)
                    wave_ctx_flip(ci, qpair, qg, qex, ctx_tiles[qpair], nqb)
                if last_of_pair:
                    h0 = 2 * pair
                    items = [(h0 + hh, ctx_tiles[pair][hh])
                             for hh in range(2)]
                    while pending_norms:  # drain leftovers before reassign
                        pending_norms.pop(0)()
                    if final_pair:
                        # kernel tail: the final q-block's outproj mc0
                        # halves read ctxT[0] (ready since pair 0's norm),
                        # so emit them first — they run under the norm
                        # chain; only the mc1 halves wait on the final
                        # transpose. Then the norm chain ahead of this
                        # wave's filler pieces.
                        qbf = S // 128 - 1
                        po_t = []
                        for fc in range(2):
                            po = ps.tile([128, QC], FP32, tag="scores",
                                         bufs=2, name=f"pot_{fc}")
                            nc.tensor.matmul(
                                po[:],
                                lhsT=ctxT[0][:, qbf * 128:(qbf + 1) * 128],
                                rhs=wot[0][:, fc * QC:(fc + 1) * QC],
                                start=True, stop=False,
                                skip_group_check=True)
                            po_t.append(po)
                        for pc in flip_norm_pieces(ci, pair, items):
                            pc()
                        for pc in sched[w]:
                            pc()
                    else:
                        pending_norms = flip_norm_pieces(ci, pair, items)
        # ---- kernel tail: final q-block mc1 + staging + store ----
        qbf = S // 128 - 1
        ob_f = outb.tile([128, E], BF16, tag="ob", name="ob_f")
        for fc in range(2):
            nc.tensor.matmul(
                po_t[fc],
                lhsT=ctxT[1][:, qbf * 128:(qbf + 1) * 128],
                rhs=wot[1][:, fc * QC:(fc + 1) * QC],
                start=False, stop=True,
                skip_group_check=True)
        # stage the two halves on different engines so they run in parallel
        nc.vector.tensor_copy(ob_f[:, 0:QC], po_t[0][:])
        nc.scalar.activation(ob_f[:, QC:E], po_t[1][:],
                             mybir.ActivationFunctionType.Copy)
        nc.sync.dma_start(out[qbf * 128:(qbf + 1) * 128, :], ob_f[:])


def build_module():
    nc = bacc.Bacc("TRN2", target_bir_lowering=False, debug=False)
    xT = nc.dram_tensor("xT", [E, S], BF16, kind="ExternalInput").ap()
    wq = nc.dram_tensor("wq", [E, M], BF16, kind="ExternalInput").ap()
    wk = nc.dram_tensor("wk", [E, M], BF16, kind="ExternalInput").ap()
    wv = nc.dram_tensor("wv", [E, M], BF16, kind="ExternalInput").ap()
    wo = nc.dram_tensor("wo", [M, E], BF16, kind="ExternalInput").ap()
    x8 = nc.dram_tensor("x8", [128, 8 * S], F8, kind="ExternalInput").ap()
    wk8 = nc.dram_tensor("wk8", [128, 8 * M], F8, kind="ExternalInput").ap()
    wq8 = nc.dram_tensor("wq8", [128, 8 * M], F8, kind="ExternalInput").ap()
    out = nc.dram_tensor("out", [S, E], BF16, kind="ExternalOutput").ap()
    with tile.TileContext(nc) as tc:
        _emit_kernel(tc, xT, wq, wk, wv, wo, x8, wk8, wq8, out)
    nc.compile()
    return nc


def _pack_epairs(aT):
    """[E, N] -> [128, 4*2*N] fp8: e-tile pairs side by side per partition
    (DoubleRow packing: out[p, j, t, n] = aT[(2j+t)*128 + p, n])."""
    e4m3 = ml_dtypes.float8_e4m3
    E_, N = aT.shape
    a = np.asarray(aT, dtype=np.float32).reshape(4, 2, 128, N)
    a = np.ascontiguousarray(a.transpose(2, 0, 1, 3)).astype(e4m3)
    return a.reshape(128, 8 * N)


def make_in_maps(x, w_qkv):
    """Per-core input dicts (bf16/fp8, pre-transposed host-side)."""
    bf = ml_dtypes.bfloat16
    xTb = [np.ascontiguousarray(x[b].T).astype(bf) for b in range(B)]
    x8b = [_pack_epairs(x[b].T) for b in range(B)]
    in_maps = []
    for c in range(NCORES):
        b, g = c // 4, c % 4
        cols = slice(g * M, (g + 1) * M)
        wqT = np.ascontiguousarray(w_qkv[cols, :].T)
        wkT = np.ascontiguousarray(w_qkv[E:][cols, :].T)
        in_maps.append({
            "xT": xTb[b],
            "wq": wqT.astype(bf),
            "wk": wkT.astype(bf),
            "wv": np.ascontiguousarray(w_qkv[2 * E:][cols, :].T).astype(bf),
            "x8": x8b[b],
            "wk8": _pack_epairs(wkT * 64.0),
            "wq8": _pack_epairs(wqT * 64.0),
            "wo": None,  # filled in kernel(), needs w_out
        })
    return in_maps


_RUNNER = None
_SHARDED = None


def _get_runner():
    """Build the Bass module once and return a cached callable
    (in_maps) -> [NCORES, S, E] bf16 partial outputs."""
    global _RUNNER
    if _RUNNER is not None:
        return _RUNNER

    nc = build_module()

    from concourse import bass2jax
    import jax
    from jax.sharding import Mesh, PartitionSpec
    from jax.experimental.shard_map import shard_map

    bass2jax.install_neuronx_cc_hook()

    in_names = ["xT", "wq", "wk", "wv", "x8", "wk8", "wq8", "wo"]
    out_names = ["out"]
    out_avals = [jax.core.ShapedArray((S, E), ml_dtypes.bfloat16)]
    n_params = len(in_names)
    all_names = in_names + out_names
    partition_name = (nc.partition_id_tensor.name
                      if nc.partition_id_tensor is not None else None)
    if partition_name is not None:
        all_names = all_names + [partition_name]

    def _body(*args):
        operands = list(args)
        if partition_name is not None:
            operands.append(bass2jax.partition_id_tensor())
        outs = bass2jax._bass_exec_p.bind(
            *operands,
            out_avals=tuple(out_avals),
            in_names=tuple(all_names),
            out_names=tuple(out_names),
            lowering_input_output_aliases=(),
            sim_require_finite=True,
            sim_require_nnan=True,
            nc=nc,
        )
        return tuple(outs)

    devices = jax.devices()[:NCORES]
    mesh = Mesh(np.asarray(devices), ("core",))
    n_outs = len(out_names)
    in_specs = (PartitionSpec("core"),) * (n_params + n_outs)
    out_specs = (PartitionSpec("core"),) * n_outs
    sharded = jax.jit(
        shard_map(_body, mesh=mesh, in_specs=in_specs, out_specs=out_specs,
                  check_rep=False),
        donate_argnums=tuple(range(n_params, n_params + n_outs)),
        keep_unused=True,
    )
    global _SHARDED
    _SHARDED = sharded

    def run(in_maps):
        concat_in = [
            np.concatenate([np.asarray(in_maps[c][n]) for c in range(NCORES)],
                           axis=0)
            for n in in_names
        ]
        concat_zeros = [np.zeros((NCORES * S, E), ml_dtypes.bfloat16)]
        out_arrs = sharded(*concat_in, *concat_zeros)
        return np.asarray(out_arrs[0]).reshape(NCORES, S, E)

    _RUNNER = run
    return run


def kernel(x, w_qkv, w_out, b_out):
    x = np.asarray(x, dtype=np.float32)
    w_qkv = np.asarray(w_qkv, dtype=np.float32)
    w_out = np.asarray(w_out, dtype=np.float32)
    b_out = np.asarray(b_out, dtype=np.float32)

    bf = ml_dtypes.bfloat16
    in_maps = make_in_maps(x, w_qkv)
    for c in range(NCORES):
        g = c % 4
        cols = slice(g * M, (g + 1) * M)
        in_maps[c]["wo"] = np.ascontiguousarray(w_out[:, cols].T).astype(bf)

    run = _get_runner()
    partials = run(in_maps)  # [8, S, E] bf16

    out = np.empty((B, S, E), np.float32)
    for b in range(B):
        acc = partials[4 * b].astype(np.float64)
        for i in range(1, 4):
            acc += partials[4 * b + i].astype(np.float64)
        out[b] = (acc + b_out.astype(np.float64)).astype(np.float32)
    return out


# revision 76
# speedup vs baseline: 1.0593x; 1.0020x over previous
"""Multi-head causal self-attention (B=2, S=2048, E=1024, H=16, D=64) on 8
Trainium2 NeuronCores.

Sharding: batch x head-group. Core c handles batch (c // 4) and heads
[4*(c%4), 4*(c%4)+4). Each core computes QKV projection for its 4 heads,
causal flash-attention, and a partial output projection over its head
columns. Host sums the 4 partial outputs per batch and adds b_out.

v4 changes vs v3 (114.2us -> 108.1us cost-model span):
  - flipped ctx matmuls for ALL chunks: out = [q-part 128, 65] with
    lhsT = ex q-window (stationary), rhs = v1 slab. PE cost per
    (head, kb, qb) drops from `cols` to 65 rows (full 128-partition
    output): ctx 29us -> 15us. The softmax denominator lands
    per-q-PARTITION, so the norm is a cheap [128,nqb] reciprocal +
    per-partition-scalar multiplies (no Pool partition_broadcast).
    start=True only on each ctx tile's FIRST matmul: start marks the
    whole 2KB PSUM zero-region pending-zero, so per-q-block starts
    would corrupt sibling accumulations (lazy zero-on-first-write
    covers the other q-blocks).
  - ctx_qm [q, m] bf16 is transposed back to ctxT [m, q] with PE
    transposes (identity matmul, 128 rows each) + DVE copies; DMA/SP
    queues stay clear (in-order SP.SEQ head-of-line-blocks rearranges).
  - q projection for chunks 1-4 via fp8 DoubleRow (host-packed wq8,
    x8), mirroring the k8 path: 1/4 the PE cost of the bf16 proj.
  - stair masking of ex moved from DVE tensor_mul to Pool affine_select
    (SBUF->SBUF, one call covers both heads of a wave).
  - per-chunk qk8c/qk8p staging tiles: the dependency tracker flattens
    strided APs to byte ranges, so shared tiles made chunk ci's scores
    falsely wait on chunk ci+1's rearrange DMAs.
  - startup: fp8 operands + x8 load before the xt bulk; xt[1..3]/wot
    dma_starts deferred into the wave schedule (the serial DMA device
    processes in issue order, and the per-chunk rearranges gate the
    ACT-bound steady state); chunk1/2 fp8 prep pinned into chunk0's
    waves, per-mt rearranges so pair 0 isn't gated on mt1 projections.
  - tail: outproj spread so the last chunk's pieces land by wave 6;
    final q-block outproj mc0 halves pre-issued against ctxT[0] before
    the final norm; per-q-block output stores for the last two chunks.

The steady state is ACT-bound: exp processes every score element at
0.833ns/col (~58us) plus ~185ns/call init; PE sits at ~71%. Remaining
idle is the DMA-gated startup (~20us) and the ~6us drain tail.
"""

import sys

if "/opt/trn_rl_repo" not in sys.path:
    sys.path.insert(0, "/opt/trn_rl_repo")

import numpy as np
import ml_dtypes

import concourse.bacc as bacc
import concourse.mybir as mybir
import concourse.tile as tile

BF16 = mybir.dt.bfloat16
FP32 = mybir.dt.float32
F8 = mybir.dt.float8e4
DRMODE = mybir.MatmulPerfMode.DoubleRow

B, S, E = 2, 2048, 1024
H, DH = 16, 64
NCORES = 8
HPC = 4            # heads per core
M = HPC * DH       # 256 ctx columns per core
QC = 512           # q chunk (max wave width; also PSUM head stride)
KB = 128           # k block
SCALE = 1.0 / np.sqrt(DH)
NWARM = 64         # warmup dummy matmuls (128 cols each)
# q-chunks (q0, Q, grp). The last 512 splits 384+128 so the final
# norm/outproj tail is 4x smaller. grp = k-blocks per wave for the narrow
# final chunk (shares one exp call across 4 k-blocks).
CHUNKS = [(0, 512, 1), (512, 512, 1), (1024, 512, 1),
          (1536, 384, 1), (1920, 128, 4)]
NCH = len(CHUNKS)


def _emit_kernel(tc, xT, wq, wk, wv, wo_d, x8, wk8, wq8, out):
    nc = tc.nc
    Exp = mybir.ActivationFunctionType.Exp

    with tc.tile_pool(name="res", bufs=1) as res, \
         tc.tile_pool(name="ps", bufs=1, space="PSUM") as ps, \
         tc.tile_pool(name="expp", bufs=16) as expp, \
         tc.tile_pool(name="scr", bufs=4) as scr, \
         tc.tile_pool(name="cqm", bufs=2) as cqm_pool, \
         tc.tile_pool(name="outb", bufs=2) as outb:

        # ---- resident SBUF tiles ----
        xt_all = res.tile([128, 8 * S], BF16, name="xt_all")
        xt = [xt_all[:, e * S:(e + 1) * S] for e in range(8)]
        xt_3d = xt_all.rearrange("p (e s) -> p e s", s=S)
        wqt = res.tile([128, 8 * M], BF16, name="wqt")
        wkt = res.tile([128, 8 * M], BF16, name="wkt")
        wvt = res.tile([128, 8 * M], BF16, name="wvt")
        wot = [res.tile([128, E], BF16, name=f"wot{i}") for i in range(2)]
        qTt = [res.tile([128, S], BF16, name=f"qTt{i}") for i in range(2)]
        kTt = [res.tile([128, S], BF16, name=f"kTt{i}") for i in range(2)]
        ctxT = [res.tile([128, S], BF16, name=f"ctxT{i}") for i in range(2)]
        # fp8 scores path (queries >= 512): per chunk one classic-layout fp8
        # staging tile (free dims qk x mt x s) and one DoubleRow "pair" tile
        # [64, j x qk x mt x s] with head parity on partition halves {0,32}
        # and the two dh-32 k-tiles (j) in the free dim. PER-CHUNK tiles:
        # the dependency tracker flattens strided APs to byte ranges, so a
        # single shared tile makes chunk ci's scores falsely wait on chunk
        # ci+1's rearrange DMAs.
        qk8cs = [res.tile([128, 4 * CH[1]], F8, name=f"qk8c{i}")
                 for i, CH in enumerate(CHUNKS)]
        qk8c4s = [t.rearrange("p (t m s) -> p t m s", t=2, m=2)
                  for t in qk8cs]
        qk8ps = [res.tile([64, 8 * CH[1]], F8, name=f"qk8p{i}")
                 for i, CH in enumerate(CHUNKS)]
        qk8p5s = [t.rearrange("p (j t m s) -> p j t m s", j=2, t=2, m=2)
                  for t in qk8ps]

        def kb_loc(kb):
            # global k-block -> (chunk index, local column offset)
            for i in range(NCH - 1, -1, -1):
                if kb * 128 >= CHUNKS[i][0]:
                    return i, kb * 128 - CHUNKS[i][0]
            raise AssertionError
        # fp8 projection operands (host-packed e-pair layout): q/k columns
        # that are only ever consumed by the fp8 score path are projected
        # with fp8 DoubleRow matmuls at 1/4 the PE cost
        x8t = res.tile([128, 8 * S], F8, name="x8t")
        x8_4d = x8t.rearrange("p (j t s) -> p j t s", j=4, t=2)
        wk8t = res.tile([128, 8 * M], F8, name="wk8t")
        wk8_4d = wk8t.rearrange("p (j t m) -> p j t m", j=4, t=2)
        wq8t = res.tile([128, 8 * M], F8, name="wq8t")
        wq8_4d = wq8t.rearrange("p (j t m) -> p j t m", j=4, t=2)
        # V with ones column: per (k-block kb, head h) a [128, 65] slab
        v1 = res.tile([128, (S // KB) * HPC * 65], BF16, name="v1")
        v1_3d = v1.rearrange("p (n c) -> p n c", c=65)
        warm_src = res.tile([128, 128], BF16, name="warm_src")
        ident = res.tile([128, 128], BF16, name="ident")

        # ---- warmup: dummy matmuls keep the PE busy (and its p-state
        # ramping) through the DMA-gated startup.
        nc.gpsimd.memset(warm_src[:], 0.0)
        warm_ps = ps.tile([128, QC], FP32, tag="proj", bufs=2, name="warm_ps")
        for i in range(NWARM):
            nc.tensor.matmul(
                warm_ps[:, 0:128], lhsT=warm_src[:], rhs=warm_src[:],
                start=True, stop=True)

        # ---- input DMA: one batched transfer per tensor/chunk ----
        wqt_3d = wqt.rearrange("p (e m) -> p e m", m=M)
        wkt_3d = wkt.rearrange("p (e m) -> p e m", m=M)
        wvt_3d = wvt.rearrange("p (e m) -> p e m", m=M)
        xT_3d = xT.rearrange("(e p) s -> p e s", p=128)
        # order: chunk-0 bf16 operands first (pqi/pki), then the fp8
        # operands for ALL chunks (small; chunks 1+ exp work can only start
        # once q8/k8 are projected + rearranged, and that work is what keeps
        # the ACT engine fed during the remaining xt transfers), then the
        # bf16 x chunks (V projections, consumed later) and wo.
        nc.sync.dma_start(wqt_3d[:], wq.rearrange("(e p) m -> p e m", p=128))
        nc.sync.dma_start(xt_3d[:, :, 0:256], xT_3d[:, :, 0:256])
        nc.sync.dma_start(wkt_3d[:], wk.rearrange("(e p) m -> p e m", p=128))
        nc.sync.dma_start(xt_3d[:, :, 256:QC], xT_3d[:, :, 256:QC])
        x8_dram = x8.rearrange("p (j t s) -> p j t s", j=4, t=2)
        nc.sync.dma_start(wq8t[:], wq8)
        nc.sync.dma_start(wk8t[:], wk8)

        def emit_x8_load(chunk):
            nc.sync.dma_start(
                x8_4d[:, :, :, chunk * QC:(chunk + 1) * QC],
                x8_dram[:, :, :, chunk * QC:(chunk + 1) * QC])

        emit_x8_load(1)
        nc.sync.dma_start(wvt_3d[:], wv.rearrange("(e p) m -> p e m", p=128))
        emit_x8_load(2)
        emit_x8_load(3)

        # xt[1..3] and wot are consumed late (V projections of later chunks,
        # outproj). Their dma_starts are deferred into the wave schedule so
        # the per-chunk qk8p rearranges (which gate each chunk's scores and
        # hence the ACT-bound steady state) aren't queued behind them on the
        # serial DMA device.
        def emit_xt_load(chunk):
            nc.sync.dma_start(
                xt_3d[:, :, chunk * QC:(chunk + 1) * QC],
                xT_3d[:, :, chunk * QC:(chunk + 1) * QC])

        def emit_wot_load():
            for i in range(2):
                nc.sync.dma_start(wot[i][:], wo_d[i * 128:(i + 1) * 128, :])

        # ---- constants ----
        # exp-table warm first: the ~2.7us table load + warm call must not
        # queue behind the big v1 memset on the Pool engine
        warm = scr.tile([1, 1], FP32, tag="warm", bufs=1, name="warm")
        nc.gpsimd.memset(warm[:], 0.0)
        nc.scalar.activation(warm[:], warm[:],
                             mybir.ActivationFunctionType.Exp)
        nc.gpsimd.memset(v1[:], 1.0)  # data columns overwritten by V proj
        # identity matrix for PE transposes (keep where q_local == partition)
        nc.gpsimd.memset(ident[:], 1.0)
        nc.gpsimd.affine_select(
            out=ident[:], in_=ident[:],
            compare_op=mybir.AluOpType.is_equal,
            fill=0.0, base=0,
            pattern=[[1, 128]],
            channel_multiplier=-1,
        )

        # ---- emission helpers ----
        def stage_f8(ci, mt, kind, pqk, scale=None):
            # fp8 classic staging into the (qk, mt) slab of chunk ci's qk8c
            s0, Q, _ = CHUNKS[ci]
            t = 0 if kind == "q" else 1
            if scale is None:
                nc.vector.tensor_copy(qk8c4s[ci][:, t, mt, 0:Q], pqk[:, 0:Q])
            else:
                nc.vector.tensor_scalar_mul(qk8c4s[ci][:, t, mt, 0:Q],
                                            pqk[:, 0:Q], scale)

        def emit_rearrange(ci, mt, k_only=False):
            # partition rearrange into the DoubleRow pair tile for chunk
            # ci's mt slab (per-mt so pair 0's scores aren't gated on the
            # mt=1 projections)
            s0, Q, _ = CHUNKS[ci]
            t0 = 1 if k_only else 0
            for hh in range(2):
                for j in range(2):
                    nc.sync.dma_start(
                        qk8p5s[ci][32 * hh:32 * hh + 32, j, t0:2,
                                   mt:mt + 1, 0:Q],
                        qk8c4s[ci][64 * hh + 32 * j: 64 * hh + 32 * j + 32,
                                   t0:2, mt:mt + 1, 0:Q])

        def stage_qk(ci, mt, kind, pqk):
            # chunk 0 queries score in bf16 (classic layout); all other
            # queries score in fp8 DoubleRow. k is needed in fp8 by every
            # fp8 chunk, and in bf16 only for chunk 0's k-blocks.
            s0, Q, _ = CHUNKS[ci]
            dstt = qTt if kind == "q" else kTt
            if ci == 0:
                nc.vector.tensor_copy(dstt[mt][:, s0:s0 + Q], pqk[:, 0:Q])
            if kind == "k" or ci >= 1:
                stage_f8(ci, mt, kind, pqk)

        def emit_proj_qk8(ci, mt, kind):
            # q/k projection for fp8-only consumers via fp8 DoubleRow over
            # host-packed e-pairs: 1/4 the PE cost of the bf16 projection
            s0, Q, _ = CHUNKS[ci]
            w8 = wq8_4d if kind == "q" else wk8_4d
            pk = ps.tile([128, QC], FP32, tag="proj", bufs=2,
                         name=f"p8{kind}_{ci}_{mt}")
            # a-piece OUTER: interleaving two DoubleRow accumulation groups
            # (j inner per region) miscomputes on hardware -- each region's
            # 4-instruction group must run contiguously
            for a in range(0, Q, 256):
                b = min(a + 256, Q)
                for j in range(4):
                    nc.tensor.matmul(
                        pk[:, a:b],
                        lhsT=w8[:, j, :, mt * 128:(mt + 1) * 128],
                        rhs=x8_4d[:, j, :, s0 + a: s0 + b],
                        start=(j == 0), stop=(j == 3),
                        perf_mode=DRMODE)
            # w8 is host-scaled by 64 (w values ~0.02 sit in e4m3's
            # subnormal range, which the PE flushes to zero); undo here
            stage_f8(ci, mt, kind, pk, scale=1.0 / 64.0)

        def emit_proj_qk_interleaved(ci, mt):
            # startup projection: q first (wave 0 needs all 512 q columns),
            # then k in two pieces so wave 0 only gates on its first k-block
            s0, Q, _ = CHUNKS[ci]
            pq = ps.tile([128, QC], FP32, tag="proj", bufs=2,
                         name=f"pqi_{ci}_{mt}")
            pk = ps.tile([128, QC], FP32, tag="proj", bufs=2,
                         name=f"pki_{ci}_{mt}")
            pieces = [(pq, wqt, 0, 256), (pk, wkt, 0, 128),
                      (pq, wqt, 256, Q), (pk, wkt, 128, Q)]
            for dst, wt, a, b in pieces:
                for e in range(8):
                    nc.tensor.matmul(
                        dst[:, a:b],
                        lhsT=wt[:, e * M + mt * 128: e * M + (mt + 1) * 128],
                        rhs=xt[e][:, s0 + a: s0 + b],
                        start=(e == 0), stop=(e == 7))
                tgt = qTt if wt is wqt else kTt
                nc.vector.tensor_copy(tgt[mt][:, s0 + a: s0 + b],
                                      dst[:, a:b])
            stage_f8(ci, mt, "k", pk)

        def emit_proj_v(sblk):
            pv = ps.tile([128, M], FP32, tag="proj", bufs=2, name=f"pv_{sblk}")
            for e in range(8):
                nc.tensor.matmul(
                    pv[:],
                    lhsT=xt[e][:, sblk * 128:(sblk + 1) * 128],
                    rhs=wvt[:, e * M:(e + 1) * M],
                    start=(e == 0), stop=(e == 7))
            nc.vector.tensor_copy(
                v1_3d[:, sblk * HPC:(sblk + 1) * HPC, 0:64],
                pv[:].rearrange("p (h c) -> p h c", c=64))

        def proj_qk_pieces(ci):
            pcs = []
            for mt in range(2):
                pcs.append(lambda mt=mt: emit_proj_qk8(ci, mt, "q"))
                pcs.append(lambda mt=mt: emit_proj_qk8(ci, mt, "k"))
                pcs.append(lambda mt=mt: emit_rearrange(ci, mt))
            return pcs

        def proj_v_pieces(blks):
            return [lambda sb=sb: emit_proj_v(sb) for sb in blks]

        ob_tiles = {}
        out_3d = out.rearrange("(q p) f -> p q f", p=128)

        def emit_outproj(ci, qq, fc, tail=False):
            q0, Q, _ = CHUNKS[ci]
            nqb = Q // 128
            qb = q0 // 128 + qq
            last = ci == NCH - 1
            # last two chunks store per-q-block so the kernel-tail store
            # isn't queued behind one big merged transfer
            perqb = ci >= NCH - 2
            if qq == 0 and fc == 0:
                ob_tiles[ci] = outb.tile([128, nqb * E], BF16, tag="ob",
                                         name=f"ob_{qb}")
            ob = ob_tiles[ci]
            # tail outprojs borrow the scores PSUM slots (attention is done
            # by then), keeping mm->copy->mm free of slot serialization
            tag = "scores" if tail else "proj"
            po = ps.tile([128, QC], FP32, tag=tag, bufs=2,
                         name=f"po_{qb}_{fc}")
            for mc in range(2):
                nc.tensor.matmul(
                    po[:],
                    lhsT=ctxT[mc][:, qb * 128:(qb + 1) * 128],
                    rhs=wot[mc][:, fc * QC:(fc + 1) * QC],
                    start=(mc == 0), stop=(mc == 1))
            if last and fc == 1:
                # final piece: stage on the (idle by now) ACT engine so the
                # two last copies run in parallel instead of serializing on
                # the DVE queue
                nc.scalar.activation(
                    ob[:, qq * E + fc * QC: qq * E + (fc + 1) * QC], po[:],
                    mybir.ActivationFunctionType.Copy)
            else:
                nc.vector.tensor_copy(
                    ob[:, qq * E + fc * QC: qq * E + (fc + 1) * QC], po[:])
            if perqb:
                if fc == 1:
                    nc.sync.dma_start(
                        out[qb * 128:(qb + 1) * 128, :],
                        ob[:, qq * E: (qq + 1) * E])
                if (qq, fc) == (nqb - 1, 1):
                    del ob_tiles[ci]
            elif (qq, fc) == (nqb - 1, 1):
                nc.sync.dma_start(
                    out_3d[:, q0 // 128: q0 // 128 + nqb, :],
                    ob.rearrange("p (q f) -> p q f", f=E))
                del ob_tiles[ci]

        def outproj_pieces(ci, tail=False):
            _, Q, _ = CHUNKS[ci]
            return [lambda qq=qq, fc=fc: emit_outproj(ci, qq, fc, tail=tail)
                    for qq in range(Q // 128) for fc in range(2)]

        # ---- attention waves (one head PAIR, grp k-blocks) ----
        def wave_scores(ci, pair, g):
            s0, Q, grp = CHUNKS[ci]
            mt = pair
            fp8 = ci >= 1
            sc_ps = ps.tile([128, 2 * QC], FP32, tag="scores", bufs=2,
                            name=f"s_{ci}_{pair}_{g}")
            kds = [(j, (g * grp + j) * 128 - s0) for j in range(grp)]
            lo_e = 0
            for hh in range(2):
                r0 = hh * 64
                off = hh * QC
                for j, kd in kds:
                    kb = g * grp + j
                    # cols [0, kd) of this k-block's region are fully
                    # masked -> skip in scores. Only for grp == 1 (where
                    # the exp also skips them); grp > 1 diagonal waves
                    # compute the ~128 masked cols (27ns) so the exp never
                    # reads unwritten PSUM.
                    lo = kd if (kd >= 128 and grp == 1) else 0
                    if hh == 0 and kd >= 128 and grp == 1:
                        lo_e = kd
                    if fp8:
                        # DoubleRow: dh 2x32 k-tiles, head at base 32*hh;
                        # moving free = 2*w caps piece width at 256
                        ck, koff = kb_loc(kb)
                        a = lo
                        while a < Q:
                            b = min(a + 256, Q)
                            nc.tensor.matmul(
                                sc_ps[:, off + j * Q + a: off + j * Q + b],
                                lhsT=qk8p5s[ck][32 * hh:32 * hh + 32, :, 1,
                                                mt, koff:koff + 128],
                                rhs=qk8p5s[ci][32 * hh:32 * hh + 32, :, 0,
                                               mt, a:b],
                                start=True, stop=True,
                                perf_mode=DRMODE)
                            a = b
                    elif (pair, g) != (0, 0):
                        nc.tensor.matmul(
                            sc_ps[:, off + j * Q + lo: off + (j + 1) * Q],
                            lhsT=kTt[mt][r0:r0 + 64, kb * 128:(kb + 1) * 128],
                            rhs=qTt[mt][r0:r0 + 64, s0 + lo: s0 + Q],
                            start=True, stop=True)
            if not fp8 and (pair, g) == (0, 0):
                # very first wave: scores in column pieces matching the
                # split startup projection, COLUMN-outer / head-inner (the
                # PE wait queue is FIFO, so a blocked later-column piece
                # must not sit in front of a ready first-column piece)
                for a, b in [(0, 256), (256, Q)]:
                    for hh in range(2):
                        r0, off = hh * 64, hh * QC
                        nc.tensor.matmul(
                            sc_ps[:, off + a: off + b],
                            lhsT=kTt[mt][r0:r0 + 64, 0:128],
                            rhs=qTt[mt][r0:r0 + 64, a:b],
                            start=True, stop=True)
            ex = expp.tile([128, 2 * QC], BF16, tag="ex",
                           name=f"e_{ci}_{pair}_{g}")
            W = grp * Q
            ex3 = ex.rearrange("p (h q) -> p h q", h=2)
            sc3 = sc_ps.rearrange("p (h q) -> p h q", h=2)
            if (ci, pair, g) == (0, 0, 0):
                # very first wave: exp per (head, column-half) in contiguous
                # slices (a strided 2-head AP flattens to a byte range that
                # would falsely depend on the later column pieces)
                for a, b in [(0, 256), (256, W)]:
                    for hh in range(2):
                        nc.scalar.activation(ex3[:, hh, a:b],
                                             sc3[:, hh, a:b],
                                             Exp, scale=SCALE)
            elif lo_e or W < QC:
                # both heads in one strided-AP call: the ACT engine charges
                # by total free size, so this halves the per-call init cost
                # vs one call per head
                nc.scalar.activation(ex3[:, :, lo_e:W], sc3[:, :, lo_e:W],
                                     Exp, scale=SCALE)
            else:
                nc.scalar.activation(ex[:], sc_ps[:], Exp, scale=SCALE)
            # stair mask on the diagonal 128-block: zero ex where
            # k_local > q_local. Pool affine_select (SBUF->SBUF), one call
            # covers both heads: keep where (q_local - k_partition) >= 0.
            for j, kd in kds:
                if kd >= 0:
                    nc.gpsimd.affine_select(
                        out=ex3[:, :, j * Q + kd: j * Q + kd + 128],
                        in_=ex3[:, :, j * Q + kd: j * Q + kd + 128],
                        compare_op=mybir.AluOpType.is_ge,
                        fill=0.0, base=0,
                        pattern=[[0, 2], [1, 128]],
                        channel_multiplier=-1,
                    )
            return ex

        def wave_ctx_flip(ci, pair, g, ex, ctx_pair, nqb):
            # flipped ctx: lhsT = ex q-window (stationary), rhs = v1 slab,
            # out = [q-part 128, 65] accumulated over kb. qb's last
            # contribution is its diagonal block.
            # start=True ONLY on the tile's first matmul: start marks the
            # whole 2KB PSUM zero-region pending-zero, so a second start
            # would corrupt sibling q-blocks' accumulations. Later q-blocks'
            # first writes zero-on-first-write via that same pending flag.
            s0, Q, grp = CHUNKS[ci]
            qb_base = s0 // 128
            for hh in range(2):
                h = 2 * pair + hh
                off = hh * QC
                for j in range(grp):
                    kb = g * grp + j
                    kd = kb * 128 - s0
                    qb0 = max(0, kd // 128)
                    for qb in range(qb0, nqb):
                        nc.tensor.matmul(
                            ctx_pair[hh][:, qb * 128: qb * 128 + 65],
                            lhsT=ex[:, off + j * Q + qb * 128:
                                    off + j * Q + qb * 128 + 128],
                            rhs=v1_3d[:, kb * HPC + h, :],
                            start=(kb == 0 and qb == 0),
                            stop=(kb == qb_base + qb),
                            skip_group_check=True)

        def flip_norm_pieces(ci, pair, items):
            # flipped-ctx norm: per head a [128, nqb] reciprocal of the
            # per-q-partition denominators (col 64 of each qb slice), then
            # per (head, qb) a tensor_scalar multiply into the ctx_qm
            # staging tile, then one XBAR dma-transpose per q-block into
            # ctxT. Spread over the next waves.
            s0, Q, _ = CHUNKS[ci]
            nqb = Q // 128
            qb_lo = s0 // 128
            state = {}
            cqm = cqm_pool.tile([128, nqb * 128], BF16, tag="cqm",
                                name=f"cqm_{ci}_{pair}")

            def p_recips():
                state["recs"] = []
                for h, ctx_ps in items:
                    rec = scr.tile([128, 4], FP32, tag="rec",
                                   name=f"r_{ci}_{h}")
                    c3 = ctx_ps.rearrange("p (qb c) -> p qb c", c=128)
                    r3 = rec.rearrange("p (a b) -> p a b", b=1)
                    nc.vector.reciprocal(r3[:, 0:nqb, :],
                                         c3[:, 0:nqb, 64:65])
                    state["recs"].append(rec)

            def p_muls(i):
                h, ctx_ps = items[i]
                hh = h % 2
                rec = state["recs"][i]
                for qb in range(nqb):
                    nc.vector.tensor_scalar_mul(
                        cqm[:, qb * 128 + hh * 64: qb * 128 + hh * 64 + 64],
                        ctx_ps[:, qb * 128: qb * 128 + 64],
                        rec[:, qb:qb + 1])

            def p_transposes(qbs):
                # PE transpose (cheap: 128 rows each) into a bf16 PSUM tile
                # riding the proj slot rotation, then a DVE copy into ctxT.
                # Avoids the SP/HWDGE queue entirely (in-order SP.SEQ would
                # head-of-line-block later rearrange DMA issues).
                for qb in qbs:
                    tp = ps.tile([128, 128], BF16, tag="proj", bufs=2,
                                 name=f"tp_{ci}_{pair}_{qb}")
                    nc.tensor.transpose(
                        tp[:], cqm[:, qb * 128:(qb + 1) * 128], ident[:])
                    nc.vector.tensor_copy(
                        ctxT[pair][:, (qb_lo + qb) * 128:
                                   (qb_lo + qb + 1) * 128], tp[:])

            cut = min(2, nqb)
            return [p_recips, lambda: p_muls(0), lambda: p_muls(1),
                    lambda: p_transposes(range(0, cut)),
                    lambda: p_transposes(range(cut, nqb))]

        # ---- main schedule ----
        emit_proj_qk_interleaved(0, 0)
        pending_norms = []
        for ci in range(NCH):
            q0, Q, grp = CHUNKS[ci]
            nkb = (q0 + Q) // 128
            nqb = Q // 128
            ngrp = nkb // grp
            waves = [(pair, g) for pair in range(2)
                     for g in range(ngrp)]
            head = []   # pieces pinned to the earliest waves, one per wave
            extra = []  # pieces distributed evenly over all waves
            pins = {}   # wave -> pieces with exact placement constraints
            if ci == 0:
                # chunk 1's fp8 prep is pinned to the earliest waves so its
                # scores (the ACT feed during the xt input transfers) start
                # the moment x8[1] lands; chunk 2's prep spreads behind it
                # chunk 1's mt0 prep FIRST (ahead of chunk 0's mt1 startup
                # proj in the 2-slot proj PSUM rotation): it gates chunk 1's
                # scores, the main ACT feed once chunk 0's thin exps end
                qk1 = proj_qk_pieces(1)
                v0 = proj_v_pieces(range(0, 4))
                pins = {0: [qk1[0], qk1[1], qk1[2], v0[0],
                            lambda: emit_xt_load(1)],
                        1: [lambda: emit_proj_qk_interleaved(0, 1),
                            lambda: emit_rearrange(0, 0, k_only=True),
                            v0[1]],
                        2: [qk1[3], qk1[4], qk1[5], v0[2],
                            lambda: emit_rearrange(0, 1, k_only=True)],
                        3: [v0[3]]}
                extra += proj_qk_pieces(2) + [lambda: emit_xt_load(2)]
            elif ci == 1:
                head += proj_v_pieces(range(4, 8))
                extra += (proj_qk_pieces(3) + [lambda: emit_xt_load(3),
                                               emit_wot_load])
            elif ci == 2:
                head += proj_v_pieces(range(8, 12))
                extra += proj_qk_pieces(4) + outproj_pieces(0)
            elif ci == 3:
                head += proj_v_pieces(range(12, 16))
                extra += outproj_pieces(1) + outproj_pieces(2)
            else:
                extra += outproj_pieces(3)
            sched = {w: [] for w in range(len(waves))}
            for w, pcs in pins.items():
                sched[w].extend(pcs)
            for j, pc in enumerate(head):
                sched[j].append(pc)
            if extra:
                if ci == NCH - 1:
                    # outproj(NCH-2) pieces: no earlier than wave 4 (the
                    # previous pair's transpose pops land at waves 3-4) and
                    # packed into waves 4-5 so their stores clear the PE
                    # FIFO before the final pair's attention work
                    w0 = 4
                    span_w = 2
                else:
                    w0 = 2 if ci == 0 else max(0, min(4, len(waves)
                                                      - len(extra)))
                    span_w = len(waves) - w0
                for j, pc in enumerate(extra):
                    sched[w0 + j * span_w // len(extra)].append(pc)

            ctx_tiles = {}
            ctx_queue = []
            for w, (pair, g) in enumerate(waves):
                if g == 0:
                    # one PSUM bank per head: [128, nqb*128-float slices],
                    # 65 floats used per qb slice
                    ctx_tiles[pair] = [
                        ps.tile([128, QC], FP32, tag="ctx", bufs=2,
                                name=f"c_{ci}_{pair}_{hh}")
                        for hh in range(2)]
                ex = wave_scores(ci, pair, g)
                if pending_norms:
                    pending_norms.pop(0)()
                last_of_pair = g == ngrp - 1
                final_pair = last_of_pair and pair == 1 and ci + 1 == NCH
                if not final_pair:
                    for pc in sched[w]:
                        pc()
                ctx_queue.append((pair, g, ex))
                # defer ctx so the PE has scores to run while exp catches
                # up; drain continuously (small lag) so the pair-end flush
                # is small and the norm reciprocal doesn't head-of-line-
                # block the in-order DVE queue.
                lag = max(0, 3 - g) if grp > 1 else 3
                while len(ctx_queue) > lag or \
                        (ctx_queue and last_of_pair):
                    qpair, qg, qex = ctx_queue.pop(0)
                    wave_ctx_flip(ci, qpair, qg, qex, ctx_tiles[qpair], nqb)
                if last_of_pair:
                    h0 = 2 * pair
                    items = [(h0 + hh, ctx_tiles[pair][hh])
                             for hh in range(2)]
                    while pending_norms:  # drain leftovers before reassign
                        pending_norms.pop(0)()
                    if final_pair:
                        # kernel tail: the final q-block's outproj mc0
                        # halves read ctxT[0] (ready since pair 0's norm),
                        # so emit them first — they run under the norm
                        # chain; only the mc1 halves wait on the final
                        # transpose. Then the norm chain ahead of this
                        # wave's filler pieces.
                        qbf = S // 128 - 1
                        po_t = []
                        for fc in range(2):
                            po = ps.tile([128, QC], FP32, tag="scores",
                                         bufs=2, name=f"pot_{fc}")
                            nc.tensor.matmul(
                                po[:],
                                lhsT=ctxT[0][:, qbf * 128:(qbf + 1) * 128],
                                rhs=wot[0][:, fc * QC:(fc + 1) * QC],
                                start=True, stop=False,
                                skip_group_check=True)
                            po_t.append(po)
                        for pc in flip_norm_pieces(ci, pair, items):
                            pc()
                        for pc in sched[w]:
                            pc()
                    else:
                        pending_norms = flip_norm_pieces(ci, pair, items)
        # ---- kernel tail: final q-block mc1 + staging + store ----
        qbf = S // 128 - 1
        ob_f = outb.tile([128, E], BF16, tag="ob", name="ob_f")
        for fc in range(2):
            nc.tensor.matmul(
                po_t[fc],
                lhsT=ctxT[1][:, qbf * 128:(qbf + 1) * 128],
                rhs=wot[1][:, fc * QC:(fc + 1) * QC],
                start=False, stop=True,
                skip_group_check=True)
        # stage the two halves on different engines so they run in parallel
        nc.vector.tensor_copy(ob_f[:, 0:QC], po_t[0][:])
        nc.scalar.activation(ob_f[:, QC:E], po_t[1][:],
                             mybir.ActivationFunctionType.Copy)
        nc.sync.dma_start(out[qbf * 128:(qbf + 1) * 128, :], ob_f[:])


def build_module():
    nc = bacc.Bacc("TRN2", target_bir_lowering=False, debug=False)
    xT = nc.dram_tensor("xT", [E, S], BF16, kind="ExternalInput").ap()
    wq = nc.dram_tensor("wq", [E, M], BF16, kind="ExternalInput").ap()
    wk = nc.dram_tensor("wk", [E, M], BF16, kind="ExternalInput").ap()
    wv = nc.dram_tensor("wv", [E, M], BF16, kind="ExternalInput").ap()
    wo = nc.dram_tensor("wo", [M, E], BF16, kind="ExternalInput").ap()
    x8 = nc.dram_tensor("x8", [128, 8 * S], F8, kind="ExternalInput").ap()
    wk8 = nc.dram_tensor("wk8", [128, 8 * M], F8, kind="ExternalInput").ap()
    wq8 = nc.dram_tensor("wq8", [128, 8 * M], F8, kind="ExternalInput").ap()
    out = nc.dram_tensor("out", [S, E], BF16, kind="ExternalOutput").ap()
    with tile.TileContext(nc) as tc:
        _emit_kernel(tc, xT, wq, wk, wv, wo, x8, wk8, wq8, out)
    nc.compile()
    return nc


def _pack_epairs(aT):
    """[E, N] -> [128, 4*2*N] fp8: e-tile pairs side by side per partition
    (DoubleRow packing: out[p, j, t, n] = aT[(2j+t)*128 + p, n])."""
    e4m3 = ml_dtypes.float8_e4m3
    E_, N = aT.shape
    a = np.asarray(aT, dtype=np.float32).reshape(4, 2, 128, N)
    a = np.ascontiguousarray(a.transpose(2, 0, 1, 3)).astype(e4m3)
    return a.reshape(128, 8 * N)


def make_in_maps(x, w_qkv):
    """Per-core input dicts (bf16/fp8, pre-transposed host-side)."""
    bf = ml_dtypes.bfloat16
    xTb = [np.ascontiguousarray(x[b].T).astype(bf) for b in range(B)]
    x8b = [_pack_epairs(x[b].T) for b in range(B)]
    in_maps = []
    for c in range(NCORES):
        b, g = c // 4, c % 4
        cols = slice(g * M, (g + 1) * M)
        wqT = np.ascontiguousarray(w_qkv[cols, :].T)
        wkT = np.ascontiguousarray(w_qkv[E:][cols, :].T)
        in_maps.append({
            "xT": xTb[b],
            "wq": wqT.astype(bf),
            "wk": wkT.astype(bf),
            "wv": np.ascontiguousarray(w_qkv[2 * E:][cols, :].T).astype(bf),
            "x8": x8b[b],
            "wk8": _pack_epairs(wkT * 64.0),
            "wq8": _pack_epairs(wqT * 64.0),
            "wo": None,  # filled in kernel(), needs w_out
        })
    return in_maps


_RUNNER = None
_SHARDED = None


def _get_runner():
    """Build the Bass module once and return a cached callable
    (in_maps) -> [NCORES, S, E] bf16 partial outputs."""
    global _RUNNER
    if _RUNNER is not None:
        return _RUNNER

    nc = build_module()

    from concourse import bass2jax
    import jax
    from jax.sharding import Mesh, PartitionSpec
    from jax.experimental.shard_map import shard_map

    bass2jax.install_neuronx_cc_hook()

    in_names = ["xT", "wq", "wk", "wv", "x8", "wk8", "wq8", "wo"]
    out_names = ["out"]
    out_avals = [jax.core.ShapedArray((S, E), ml_dtypes.bfloat16)]
    n_params = len(in_names)
    all_names = in_names + out_names
    partition_name = (nc.partition_id_tensor.name
                      if nc.partition_id_tensor is not None else None)
    if partition_name is not None:
        all_names = all_names + [partition_name]

    def _body(*args):
        operands = list(args)
        if partition_name is not None:
            operands.append(bass2jax.partition_id_tensor())
        outs = bass2jax._bass_exec_p.bind(
            *operands,
            out_avals=tuple(out_avals),
            in_names=tuple(all_names),
            out_names=tuple(out_names),
            lowering_input_output_aliases=(),
            sim_require_finite=True,
            sim_require_nnan=True,
            nc=nc,
        )
        return tuple(outs)

    devices = jax.devices()[:NCORES]
    mesh = Mesh(np.asarray(devices), ("core",))
    n_outs = len(out_names)
    in_specs = (PartitionSpec("core"),) * (n_params + n_outs)
    out_specs = (PartitionSpec("core"),) * n_outs
    sharded = jax.jit(
        shard_map(_body, mesh=mesh, in_specs=in_specs, out_specs=out_specs,
                  check_rep=False),
        donate_argnums=tuple(range(n_params, n_params + n_outs)),
        keep_unused=True,
    )
    global _SHARDED
    _SHARDED = sharded

    def run(in_maps):
        concat_in = [
            np.concatenate([np.asarray(in_maps[c][n]) for c in range(NCORES)],
                           axis=0)
            for n in in_names
        ]
        concat_zeros = [np.zeros((NCORES * S, E), ml_dtypes.bfloat16)]
        out_arrs = sharded(*concat_in, *concat_zeros)
        return np.asarray(out_arrs[0]).reshape(NCORES, S, E)

    _RUNNER = run
    return run


def kernel(x, w_qkv, w_out, b_out):
    x = np.asarray(x, dtype=np.float32)
    w_qkv = np.asarray(w_qkv, dtype=np.float32)
    w_out = np.asarray(w_out, dtype=np.float32)
    b_out = np.asarray(b_out, dtype=np.float32)

    bf = ml_dtypes.bfloat16
    in_maps = make_in_maps(x, w_qkv)
    for c in range(NCORES):
        g = c % 4
        cols = slice(g * M, (g + 1) * M)
        in_maps[c]["wo"] = np.ascontiguousarray(w_out[:, cols].T).astype(bf)

    run = _get_runner()
    partials = run(in_maps)  # [8, S, E] bf16

    out = np.empty((B, S, E), np.float32)
    for b in range(B):
        acc = partials[4 * b].astype(np.float64)
        for i in range(1, 4):
            acc += partials[4 * b + i].astype(np.float64)
        out[b] = (acc + b_out.astype(np.float64)).astype(np.float32)
    return out


# revision 78
# speedup vs baseline: 1.0603x; 1.0009x over previous
"""Multi-head causal self-attention (B=2, S=2048, E=1024, H=16, D=64) on 8
Trainium2 NeuronCores.

Sharding: batch x head-group. Core c handles batch (c // 4) and heads
[4*(c%4), 4*(c%4)+4). Each core computes QKV projection for its 4 heads,
causal flash-attention, and a partial output projection over its head
columns. Host sums the 4 partial outputs per batch and adds b_out.

v4 changes vs v3 (114.2us -> 108.1us cost-model span):
  - flipped ctx matmuls for ALL chunks: out = [q-part 128, 65] with
    lhsT = ex q-window (stationary), rhs = v1 slab. PE cost per
    (head, kb, qb) drops from `cols` to 65 rows (full 128-partition
    output): ctx 29us -> 15us. The softmax denominator lands
    per-q-PARTITION, so the norm is a cheap [128,nqb] reciprocal +
    per-partition-scalar multiplies (no Pool partition_broadcast).
    start=True only on each ctx tile's FIRST matmul: start marks the
    whole 2KB PSUM zero-region pending-zero, so per-q-block starts
    would corrupt sibling accumulations (lazy zero-on-first-write
    covers the other q-blocks).
  - ctx_qm [q, m] bf16 is transposed back to ctxT [m, q] with PE
    transposes (identity matmul, 128 rows each) + DVE copies; DMA/SP
    queues stay clear (in-order SP.SEQ head-of-line-blocks rearranges).
  - q projection for chunks 1-4 via fp8 DoubleRow (host-packed wq8,
    x8), mirroring the k8 path: 1/4 the PE cost of the bf16 proj.
  - stair masking of ex moved from DVE tensor_mul to Pool affine_select
    (SBUF->SBUF, one call covers both heads of a wave).
  - per-chunk qk8c/qk8p staging tiles: the dependency tracker flattens
    strided APs to byte ranges, so shared tiles made chunk ci's scores
    falsely wait on chunk ci+1's rearrange DMAs.
  - startup: fp8 operands + x8 load before the xt bulk; xt[1..3]/wot
    dma_starts deferred into the wave schedule (the serial DMA device
    processes in issue order, and the per-chunk rearranges gate the
    ACT-bound steady state); chunk1/2 fp8 prep pinned into chunk0's
    waves, per-mt rearranges so pair 0 isn't gated on mt1 projections.
  - tail: outproj spread so the last chunk's pieces land by wave 6;
    final q-block outproj mc0 halves pre-issued against ctxT[0] before
    the final norm; per-q-block output stores for the last two chunks.

The steady state is ACT-bound: exp processes every score element at
0.833ns/col (~58us) plus ~185ns/call init; PE sits at ~71%. Remaining
idle is the DMA-gated startup (~20us) and the ~6us drain tail.
"""

import sys

if "/opt/trn_rl_repo" not in sys.path:
    sys.path.insert(0, "/opt/trn_rl_repo")

import numpy as np
import ml_dtypes

import concourse.bacc as bacc
import concourse.mybir as mybir
import concourse.tile as tile

BF16 = mybir.dt.bfloat16
FP32 = mybir.dt.float32
F8 = mybir.dt.float8e4
DRMODE = mybir.MatmulPerfMode.DoubleRow

B, S, E = 2, 2048, 1024
H, DH = 16, 64
NCORES = 8
HPC = 4            # heads per core
M = HPC * DH       # 256 ctx columns per core
QC = 512           # q chunk (max wave width; also PSUM head stride)
KB = 128           # k block
SCALE = 1.0 / np.sqrt(DH)
NWARM = 64         # warmup dummy matmuls (128 cols each)
# q-chunks (q0, Q, grp). The last 512 splits 384+128 so the final
# norm/outproj tail is 4x smaller. grp = k-blocks per wave for the narrow
# final chunk (shares one exp call across 4 k-blocks).
CHUNKS = [(0, 512, 1), (512, 512, 1), (1024, 512, 1),
          (1536, 384, 1), (1920, 128, 4)]
NCH = len(CHUNKS)


def _emit_kernel(tc, xT, wq, wk, wv, wo_d, x8, wk8, wq8, out):
    nc = tc.nc
    Exp = mybir.ActivationFunctionType.Exp

    with tc.tile_pool(name="res", bufs=1) as res, \
         tc.tile_pool(name="ps", bufs=1, space="PSUM") as ps, \
         tc.tile_pool(name="expp", bufs=16) as expp, \
         tc.tile_pool(name="scr", bufs=4) as scr, \
         tc.tile_pool(name="cqm", bufs=2) as cqm_pool, \
         tc.tile_pool(name="outb", bufs=2) as outb:

        # ---- resident SBUF tiles ----
        xt_all = res.tile([128, 8 * S], BF16, name="xt_all")
        xt = [xt_all[:, e * S:(e + 1) * S] for e in range(8)]
        xt_3d = xt_all.rearrange("p (e s) -> p e s", s=S)
        wqt = res.tile([128, 8 * M], BF16, name="wqt")
        wkt = res.tile([128, 8 * M], BF16, name="wkt")
        wvt = res.tile([128, 8 * M], BF16, name="wvt")
        wot = [res.tile([128, E], BF16, name=f"wot{i}") for i in range(2)]
        qTt = [res.tile([128, S], BF16, name=f"qTt{i}") for i in range(2)]
        kTt = [res.tile([128, S], BF16, name=f"kTt{i}") for i in range(2)]
        ctxT = [res.tile([128, S], BF16, name=f"ctxT{i}") for i in range(2)]
        # fp8 scores path (queries >= 512): per chunk one classic-layout fp8
        # staging tile (free dims qk x mt x s) and one DoubleRow "pair" tile
        # [64, j x qk x mt x s] with head parity on partition halves {0,32}
        # and the two dh-32 k-tiles (j) in the free dim. PER-CHUNK tiles:
        # the dependency tracker flattens strided APs to byte ranges, so a
        # single shared tile makes chunk ci's scores falsely wait on chunk
        # ci+1's rearrange DMAs.
        qk8cs = [res.tile([128, 4 * CH[1]], F8, name=f"qk8c{i}")
                 for i, CH in enumerate(CHUNKS)]
        qk8c4s = [t.rearrange("p (t m s) -> p t m s", t=2, m=2)
                  for t in qk8cs]
        qk8ps = [res.tile([64, 8 * CH[1]], F8, name=f"qk8p{i}")
                 for i, CH in enumerate(CHUNKS)]
        qk8p5s = [t.rearrange("p (j t m s) -> p j t m s", j=2, t=2, m=2)
                  for t in qk8ps]

        def kb_loc(kb):
            # global k-block -> (chunk index, local column offset)
            for i in range(NCH - 1, -1, -1):
                if kb * 128 >= CHUNKS[i][0]:
                    return i, kb * 128 - CHUNKS[i][0]
            raise AssertionError
        # fp8 projection operands (host-packed e-pair layout): q/k columns
        # that are only ever consumed by the fp8 score path are projected
        # with fp8 DoubleRow matmuls at 1/4 the PE cost
        x8t = res.tile([128, 8 * S], F8, name="x8t")
        x8_4d = x8t.rearrange("p (j t s) -> p j t s", j=4, t=2)
        wk8t = res.tile([128, 8 * M], F8, name="wk8t")
        wk8_4d = wk8t.rearrange("p (j t m) -> p j t m", j=4, t=2)
        wq8t = res.tile([128, 8 * M], F8, name="wq8t")
        wq8_4d = wq8t.rearrange("p (j t m) -> p j t m", j=4, t=2)
        # V with ones column: per (k-block kb, head h) a [128, 65] slab
        v1 = res.tile([128, (S // KB) * HPC * 65], BF16, name="v1")
        v1_3d = v1.rearrange("p (n c) -> p n c", c=65)
        warm_src = res.tile([128, 128], BF16, name="warm_src")
        ident = res.tile([128, 128], BF16, name="ident")

        # ---- warmup: dummy matmuls keep the PE busy (and its p-state
        # ramping) through the DMA-gated startup.
        nc.gpsimd.memset(warm_src[:], 0.0)
        warm_ps = ps.tile([128, QC], FP32, tag="proj", bufs=2, name="warm_ps")
        for i in range(NWARM):
            nc.tensor.matmul(
                warm_ps[:, 0:128], lhsT=warm_src[:], rhs=warm_src[:],
                start=True, stop=True)

        # ---- input DMA: one batched transfer per tensor/chunk ----
        wqt_3d = wqt.rearrange("p (e m) -> p e m", m=M)
        wkt_3d = wkt.rearrange("p (e m) -> p e m", m=M)
        wvt_3d = wvt.rearrange("p (e m) -> p e m", m=M)
        xT_3d = xT.rearrange("(e p) s -> p e s", p=128)
        # order: chunk-0 bf16 operands first (pqi/pki), then the fp8
        # operands for ALL chunks (small; chunks 1+ exp work can only start
        # once q8/k8 are projected + rearranged, and that work is what keeps
        # the ACT engine fed during the remaining xt transfers), then the
        # bf16 x chunks (V projections, consumed later) and wo.
        nc.sync.dma_start(wqt_3d[:], wq.rearrange("(e p) m -> p e m", p=128))
        nc.sync.dma_start(xt_3d[:, :, 0:256], xT_3d[:, :, 0:256])
        nc.sync.dma_start(wkt_3d[:], wk.rearrange("(e p) m -> p e m", p=128))
        nc.sync.dma_start(xt_3d[:, :, 256:QC], xT_3d[:, :, 256:QC])
        x8_dram = x8.rearrange("p (j t s) -> p j t s", j=4, t=2)
        nc.sync.dma_start(wq8t[:], wq8)
        nc.sync.dma_start(wk8t[:], wk8)

        def emit_x8_load(chunk):
            nc.sync.dma_start(
                x8_4d[:, :, :, chunk * QC:(chunk + 1) * QC],
                x8_dram[:, :, :, chunk * QC:(chunk + 1) * QC])

        emit_x8_load(1)
        nc.sync.dma_start(wvt_3d[:], wv.rearrange("(e p) m -> p e m", p=128))
        emit_x8_load(2)
        emit_x8_load(3)

        # xt[1..3] and wot are consumed late (V projections of later chunks,
        # outproj). Their dma_starts are deferred into the wave schedule so
        # the per-chunk qk8p rearranges (which gate each chunk's scores and
        # hence the ACT-bound steady state) aren't queued behind them on the
        # serial DMA device.
        def emit_xt_load(chunk):
            nc.sync.dma_start(
                xt_3d[:, :, chunk * QC:(chunk + 1) * QC],
                xT_3d[:, :, chunk * QC:(chunk + 1) * QC])

        def emit_wot_load():
            for i in range(2):
                nc.sync.dma_start(wot[i][:], wo_d[i * 128:(i + 1) * 128, :])

        # ---- constants ----
        # exp-table warm first: the ~2.7us table load + warm call must not
        # queue behind the big v1 memset on the Pool engine
        warm = scr.tile([1, 1], FP32, tag="warm", bufs=1, name="warm")
        nc.gpsimd.memset(warm[:], 0.0)
        nc.scalar.activation(warm[:], warm[:],
                             mybir.ActivationFunctionType.Exp)
        nc.gpsimd.memset(v1[:], 1.0)  # data columns overwritten by V proj
        # binary stair mask for the FINAL wave's diagonal block: a DVE
        # multiply (127ns) replaces the Pool affine_select (451ns + sem
        # hops) on the kernel-tail critical chain
        mask = res.tile([128, 128], BF16, name="mask")
        nc.gpsimd.memset(mask[:], 1.0)
        nc.gpsimd.affine_select(
            out=mask[:], in_=mask[:],
            compare_op=mybir.AluOpType.is_ge,
            fill=0.0, base=0,
            pattern=[[1, 128]],
            channel_multiplier=-1,
        )
        # identity matrix for PE transposes (keep where q_local == partition)
        nc.gpsimd.memset(ident[:], 1.0)
        nc.gpsimd.affine_select(
            out=ident[:], in_=ident[:],
            compare_op=mybir.AluOpType.is_equal,
            fill=0.0, base=0,
            pattern=[[1, 128]],
            channel_multiplier=-1,
        )

        # ---- emission helpers ----
        def stage_f8(ci, mt, kind, pqk, scale=None):
            # fp8 classic staging into the (qk, mt) slab of chunk ci's qk8c
            s0, Q, _ = CHUNKS[ci]
            t = 0 if kind == "q" else 1
            if scale is None:
                nc.vector.tensor_copy(qk8c4s[ci][:, t, mt, 0:Q], pqk[:, 0:Q])
            else:
                nc.vector.tensor_scalar_mul(qk8c4s[ci][:, t, mt, 0:Q],
                                            pqk[:, 0:Q], scale)

        def emit_rearrange(ci, mt, k_only=False):
            # partition rearrange into the DoubleRow pair tile for chunk
            # ci's mt slab (per-mt so pair 0's scores aren't gated on the
            # mt=1 projections)
            s0, Q, _ = CHUNKS[ci]
            t0 = 1 if k_only else 0
            for hh in range(2):
                for j in range(2):
                    nc.sync.dma_start(
                        qk8p5s[ci][32 * hh:32 * hh + 32, j, t0:2,
                                   mt:mt + 1, 0:Q],
                        qk8c4s[ci][64 * hh + 32 * j: 64 * hh + 32 * j + 32,
                                   t0:2, mt:mt + 1, 0:Q])

        def stage_qk(ci, mt, kind, pqk):
            # chunk 0 queries score in bf16 (classic layout); all other
            # queries score in fp8 DoubleRow. k is needed in fp8 by every
            # fp8 chunk, and in bf16 only for chunk 0's k-blocks.
            s0, Q, _ = CHUNKS[ci]
            dstt = qTt if kind == "q" else kTt
            if ci == 0:
                nc.vector.tensor_copy(dstt[mt][:, s0:s0 + Q], pqk[:, 0:Q])
            if kind == "k" or ci >= 1:
                stage_f8(ci, mt, kind, pqk)

        def emit_proj_qk8(ci, mt, kind):
            # q/k projection for fp8-only consumers via fp8 DoubleRow over
            # host-packed e-pairs: 1/4 the PE cost of the bf16 projection
            s0, Q, _ = CHUNKS[ci]
            w8 = wq8_4d if kind == "q" else wk8_4d
            pk = ps.tile([128, QC], FP32, tag="proj", bufs=2,
                         name=f"p8{kind}_{ci}_{mt}")
            # a-piece OUTER: interleaving two DoubleRow accumulation groups
            # (j inner per region) miscomputes on hardware -- each region's
            # 4-instruction group must run contiguously
            for a in range(0, Q, 256):
                b = min(a + 256, Q)
                for j in range(4):
                    nc.tensor.matmul(
                        pk[:, a:b],
                        lhsT=w8[:, j, :, mt * 128:(mt + 1) * 128],
                        rhs=x8_4d[:, j, :, s0 + a: s0 + b],
                        start=(j == 0), stop=(j == 3),
                        perf_mode=DRMODE)
            # w8 is host-scaled by 64 (w values ~0.02 sit in e4m3's
            # subnormal range, which the PE flushes to zero); undo here
            stage_f8(ci, mt, kind, pk, scale=1.0 / 64.0)

        def emit_proj_qk_interleaved(ci, mt):
            # startup projection: q first (wave 0 needs all 512 q columns),
            # then k in two pieces so wave 0 only gates on its first k-block
            s0, Q, _ = CHUNKS[ci]
            pq = ps.tile([128, QC], FP32, tag="proj", bufs=2,
                         name=f"pqi_{ci}_{mt}")
            pk = ps.tile([128, QC], FP32, tag="proj", bufs=2,
                         name=f"pki_{ci}_{mt}")
            pieces = [(pq, wqt, 0, 256), (pk, wkt, 0, 128),
                      (pq, wqt, 256, Q), (pk, wkt, 128, Q)]
            for dst, wt, a, b in pieces:
                for e in range(8):
                    nc.tensor.matmul(
                        dst[:, a:b],
                        lhsT=wt[:, e * M + mt * 128: e * M + (mt + 1) * 128],
                        rhs=xt[e][:, s0 + a: s0 + b],
                        start=(e == 0), stop=(e == 7))
                tgt = qTt if wt is wqt else kTt
                nc.vector.tensor_copy(tgt[mt][:, s0 + a: s0 + b],
                                      dst[:, a:b])
            stage_f8(ci, mt, "k", pk)

        def emit_proj_v(sblk):
            pv = ps.tile([128, M], FP32, tag="proj", bufs=2, name=f"pv_{sblk}")
            for e in range(8):
                nc.tensor.matmul(
                    pv[:],
                    lhsT=xt[e][:, sblk * 128:(sblk + 1) * 128],
                    rhs=wvt[:, e * M:(e + 1) * M],
                    start=(e == 0), stop=(e == 7))
            nc.vector.tensor_copy(
                v1_3d[:, sblk * HPC:(sblk + 1) * HPC, 0:64],
                pv[:].rearrange("p (h c) -> p h c", c=64))

        def proj_qk_pieces(ci):
            pcs = []
            for mt in range(2):
                pcs.append(lambda mt=mt: emit_proj_qk8(ci, mt, "q"))
                pcs.append(lambda mt=mt: emit_proj_qk8(ci, mt, "k"))
                pcs.append(lambda mt=mt: emit_rearrange(ci, mt))
            return pcs

        def proj_v_pieces(blks):
            return [lambda sb=sb: emit_proj_v(sb) for sb in blks]

        ob_tiles = {}
        out_3d = out.rearrange("(q p) f -> p q f", p=128)

        def emit_outproj(ci, qq, fc, tail=False):
            q0, Q, _ = CHUNKS[ci]
            nqb = Q // 128
            qb = q0 // 128 + qq
            last = ci == NCH - 1
            # last two chunks store per-q-block so the kernel-tail store
            # isn't queued behind one big merged transfer
            perqb = ci >= NCH - 2
            if qq == 0 and fc == 0:
                ob_tiles[ci] = outb.tile([128, nqb * E], BF16, tag="ob",
                                         name=f"ob_{qb}")
            ob = ob_tiles[ci]
            # tail outprojs borrow the scores PSUM slots (attention is done
            # by then), keeping mm->copy->mm free of slot serialization
            tag = "scores" if tail else "proj"
            po = ps.tile([128, QC], FP32, tag=tag, bufs=2,
                         name=f"po_{qb}_{fc}")
            for mc in range(2):
                nc.tensor.matmul(
                    po[:],
                    lhsT=ctxT[mc][:, qb * 128:(qb + 1) * 128],
                    rhs=wot[mc][:, fc * QC:(fc + 1) * QC],
                    start=(mc == 0), stop=(mc == 1))
            if last and fc == 1:
                # final piece: stage on the (idle by now) ACT engine so the
                # two last copies run in parallel instead of serializing on
                # the DVE queue
                nc.scalar.activation(
                    ob[:, qq * E + fc * QC: qq * E + (fc + 1) * QC], po[:],
                    mybir.ActivationFunctionType.Copy)
            else:
                nc.vector.tensor_copy(
                    ob[:, qq * E + fc * QC: qq * E + (fc + 1) * QC], po[:])
            if perqb:
                if fc == 1:
                    nc.sync.dma_start(
                        out[qb * 128:(qb + 1) * 128, :],
                        ob[:, qq * E: (qq + 1) * E])
                if (qq, fc) == (nqb - 1, 1):
                    del ob_tiles[ci]
            elif (qq, fc) == (nqb - 1, 1):
                nc.sync.dma_start(
                    out_3d[:, q0 // 128: q0 // 128 + nqb, :],
                    ob.rearrange("p (q f) -> p q f", f=E))
                del ob_tiles[ci]

        def outproj_pieces(ci, tail=False):
            _, Q, _ = CHUNKS[ci]
            return [lambda qq=qq, fc=fc: emit_outproj(ci, qq, fc, tail=tail)
                    for qq in range(Q // 128) for fc in range(2)]

        # ---- attention waves (one head PAIR, grp k-blocks) ----
        def wave_scores(ci, pair, g):
            s0, Q, grp = CHUNKS[ci]
            mt = pair
            fp8 = ci >= 1
            sc_ps = ps.tile([128, 2 * QC], FP32, tag="scores", bufs=2,
                            name=f"s_{ci}_{pair}_{g}")
            kds = [(j, (g * grp + j) * 128 - s0) for j in range(grp)]
            lo_e = 0
            for hh in range(2):
                r0 = hh * 64
                off = hh * QC
                for j, kd in kds:
                    kb = g * grp + j
                    # cols [0, kd) of this k-block's region are fully
                    # masked -> skip in scores. Only for grp == 1 (where
                    # the exp also skips them); grp > 1 diagonal waves
                    # compute the ~128 masked cols (27ns) so the exp never
                    # reads unwritten PSUM.
                    lo = kd if (kd >= 128 and grp == 1) else 0
                    if hh == 0 and kd >= 128 and grp == 1:
                        lo_e = kd
                    if fp8:
                        # DoubleRow: dh 2x32 k-tiles, head at base 32*hh;
                        # moving free = 2*w caps piece width at 256
                        ck, koff = kb_loc(kb)
                        a = lo
                        while a < Q:
                            b = min(a + 256, Q)
                            nc.tensor.matmul(
                                sc_ps[:, off + j * Q + a: off + j * Q + b],
                                lhsT=qk8p5s[ck][32 * hh:32 * hh + 32, :, 1,
                                                mt, koff:koff + 128],
                                rhs=qk8p5s[ci][32 * hh:32 * hh + 32, :, 0,
                                               mt, a:b],
                                start=True, stop=True,
                                perf_mode=DRMODE)
                            a = b
                    elif (pair, g) != (0, 0):
                        nc.tensor.matmul(
                            sc_ps[:, off + j * Q + lo: off + (j + 1) * Q],
                            lhsT=kTt[mt][r0:r0 + 64, kb * 128:(kb + 1) * 128],
                            rhs=qTt[mt][r0:r0 + 64, s0 + lo: s0 + Q],
                            start=True, stop=True)
            if not fp8 and (pair, g) == (0, 0):
                # very first wave: scores in column pieces matching the
                # split startup projection, COLUMN-outer / head-inner (the
                # PE wait queue is FIFO, so a blocked later-column piece
                # must not sit in front of a ready first-column piece)
                for a, b in [(0, 256), (256, Q)]:
                    for hh in range(2):
                        r0, off = hh * 64, hh * QC
                        nc.tensor.matmul(
                            sc_ps[:, off + a: off + b],
                            lhsT=kTt[mt][r0:r0 + 64, 0:128],
                            rhs=qTt[mt][r0:r0 + 64, a:b],
                            start=True, stop=True)
            ex = expp.tile([128, 2 * QC], BF16, tag="ex",
                           name=f"e_{ci}_{pair}_{g}")
            W = grp * Q
            ex3 = ex.rearrange("p (h q) -> p h q", h=2)
            sc3 = sc_ps.rearrange("p (h q) -> p h q", h=2)
            if (ci, pair, g) == (0, 0, 0):
                # very first wave: exp per (head, column-half) in contiguous
                # slices (a strided 2-head AP flattens to a byte range that
                # would falsely depend on the later column pieces)
                for a, b in [(0, 256), (256, W)]:
                    for hh in range(2):
                        nc.scalar.activation(ex3[:, hh, a:b],
                                             sc3[:, hh, a:b],
                                             Exp, scale=SCALE)
            elif lo_e or W < QC:
                # both heads in one strided-AP call: the ACT engine charges
                # by total free size, so this halves the per-call init cost
                # vs one call per head
                nc.scalar.activation(ex3[:, :, lo_e:W], sc3[:, :, lo_e:W],
                                     Exp, scale=SCALE)
            else:
                nc.scalar.activation(ex[:], sc_ps[:], Exp, scale=SCALE)
            # stair mask on the diagonal 128-block: zero ex where
            # k_local > q_local. Pool affine_select (SBUF->SBUF), one call
            # covers both heads: keep where (q_local - k_partition) >= 0.
            final_wave = (ci == NCH - 1 and pair == 1
                          and g == (s0 + Q) // 128 // grp - 1)
            for j, kd in kds:
                if kd >= 0:
                    if final_wave:
                        # kernel tail: DVE multiply, one call per head
                        for hh in range(2):
                            sl = ex3[:, hh, j * Q + kd: j * Q + kd + 128]
                            nc.vector.tensor_mul(sl, sl, mask[:])
                    else:
                        nc.gpsimd.affine_select(
                            out=ex3[:, :, j * Q + kd: j * Q + kd + 128],
                            in_=ex3[:, :, j * Q + kd: j * Q + kd + 128],
                            compare_op=mybir.AluOpType.is_ge,
                            fill=0.0, base=0,
                            pattern=[[0, 2], [1, 128]],
                            channel_multiplier=-1,
                        )
            return ex

        def wave_ctx_flip(ci, pair, g, ex, ctx_pair, nqb):
            # flipped ctx: lhsT = ex q-window (stationary), rhs = v1 slab,
            # out = [q-part 128, 65] accumulated over kb. qb's last
            # contribution is its diagonal block.
            # start=True ONLY on the tile's first matmul: start marks the
            # whole 2KB PSUM zero-region pending-zero, so a second start
            # would corrupt sibling q-blocks' accumulations. Later q-blocks'
            # first writes zero-on-first-write via that same pending flag.
            s0, Q, grp = CHUNKS[ci]
            qb_base = s0 // 128
            for hh in range(2):
                h = 2 * pair + hh
                off = hh * QC
                for j in range(grp):
                    kb = g * grp + j
                    kd = kb * 128 - s0
                    qb0 = max(0, kd // 128)
                    for qb in range(qb0, nqb):
                        nc.tensor.matmul(
                            ctx_pair[hh][:, qb * 128: qb * 128 + 65],
                            lhsT=ex[:, off + j * Q + qb * 128:
                                    off + j * Q + qb * 128 + 128],
                            rhs=v1_3d[:, kb * HPC + h, :],
                            start=(kb == 0 and qb == 0),
                            stop=(kb == qb_base + qb),
                            skip_group_check=True)

        def flip_norm_pieces(ci, pair, items):
            # flipped-ctx norm: per head a [128, nqb] reciprocal of the
            # per-q-partition denominators (col 64 of each qb slice), then
            # per (head, qb) a tensor_scalar multiply into the ctx_qm
            # staging tile, then one XBAR dma-transpose per q-block into
            # ctxT. Spread over the next waves.
            s0, Q, _ = CHUNKS[ci]
            nqb = Q // 128
            qb_lo = s0 // 128
            state = {}
            cqm = cqm_pool.tile([128, nqb * 128], BF16, tag="cqm",
                                name=f"cqm_{ci}_{pair}")

            def p_recips():
                state["recs"] = []
                for h, ctx_ps in items:
                    rec = scr.tile([128, 4], FP32, tag="rec",
                                   name=f"r_{ci}_{h}")
                    c3 = ctx_ps.rearrange("p (qb c) -> p qb c", c=128)
                    r3 = rec.rearrange("p (a b) -> p a b", b=1)
                    nc.vector.reciprocal(r3[:, 0:nqb, :],
                                         c3[:, 0:nqb, 64:65])
                    state["recs"].append(rec)

            def p_muls(i):
                h, ctx_ps = items[i]
                hh = h % 2
                rec = state["recs"][i]
                for qb in range(nqb):
                    nc.vector.tensor_scalar_mul(
                        cqm[:, qb * 128 + hh * 64: qb * 128 + hh * 64 + 64],
                        ctx_ps[:, qb * 128: qb * 128 + 64],
                        rec[:, qb:qb + 1])

            def p_transposes(qbs):
                # PE transpose (cheap: 128 rows each) into a bf16 PSUM tile
                # riding the proj slot rotation, then a DVE copy into ctxT.
                # Avoids the SP/HWDGE queue entirely (in-order SP.SEQ would
                # head-of-line-block later rearrange DMA issues).
                for qb in qbs:
                    tp = ps.tile([128, 128], BF16, tag="proj", bufs=2,
                                 name=f"tp_{ci}_{pair}_{qb}")
                    nc.tensor.transpose(
                        tp[:], cqm[:, qb * 128:(qb + 1) * 128], ident[:])
                    nc.vector.tensor_copy(
                        ctxT[pair][:, (qb_lo + qb) * 128:
                                   (qb_lo + qb + 1) * 128], tp[:])

            cut = min(2, nqb)
            return [p_recips, lambda: p_muls(0), lambda: p_muls(1),
                    lambda: p_transposes(range(0, cut)),
                    lambda: p_transposes(range(cut, nqb))]

        # ---- main schedule ----
        emit_proj_qk_interleaved(0, 0)
        pending_norms = []
        for ci in range(NCH):
            q0, Q, grp = CHUNKS[ci]
            nkb = (q0 + Q) // 128
            nqb = Q // 128
            ngrp = nkb // grp
            waves = [(pair, g) for pair in range(2)
                     for g in range(ngrp)]
            head = []   # pieces pinned to the earliest waves, one per wave
            extra = []  # pieces distributed evenly over all waves
            pins = {}   # wave -> pieces with exact placement constraints
            if ci == 0:
                # chunk 1's fp8 prep is pinned to the earliest waves so its
                # scores (the ACT feed during the xt input transfers) start
                # the moment x8[1] lands; chunk 2's prep spreads behind it
                # chunk 1's mt0 prep FIRST (ahead of chunk 0's mt1 startup
                # proj in the 2-slot proj PSUM rotation): it gates chunk 1's
                # scores, the main ACT feed once chunk 0's thin exps end
                qk1 = proj_qk_pieces(1)
                v0 = proj_v_pieces(range(0, 4))
                pins = {0: [qk1[0], qk1[1], qk1[2], v0[0],
                            lambda: emit_xt_load(1)],
                        1: [lambda: emit_proj_qk_interleaved(0, 1),
                            lambda: emit_rearrange(0, 0, k_only=True),
                            v0[1]],
                        2: [qk1[3], qk1[4], qk1[5], v0[2],
                            lambda: emit_rearrange(0, 1, k_only=True)],
                        3: [v0[3]]}
                extra += proj_qk_pieces(2) + [lambda: emit_xt_load(2)]
            elif ci == 1:
                head += proj_v_pieces(range(4, 8))
                extra += (proj_qk_pieces(3) + [lambda: emit_xt_load(3),
                                               emit_wot_load])
            elif ci == 2:
                head += proj_v_pieces(range(8, 12))
                extra += proj_qk_pieces(4) + outproj_pieces(0)
            elif ci == 3:
                head += proj_v_pieces(range(12, 16))
                extra += outproj_pieces(1) + outproj_pieces(2)
            else:
                extra += outproj_pieces(3)
            sched = {w: [] for w in range(len(waves))}
            for w, pcs in pins.items():
                sched[w].extend(pcs)
            for j, pc in enumerate(head):
                sched[j].append(pc)
            if extra:
                if ci == NCH - 1:
                    # outproj(NCH-2) pieces: no earlier than wave 4 (the
                    # previous pair's transpose pops land at waves 3-4) and
                    # packed into waves 4-5 so their stores clear the PE
                    # FIFO before the final pair's attention work
                    w0 = 4
                    span_w = 2
                else:
                    w0 = 2 if ci == 0 else max(0, min(4, len(waves)
                                                      - len(extra)))
                    span_w = len(waves) - w0
                for j, pc in enumerate(extra):
                    sched[w0 + j * span_w // len(extra)].append(pc)

            ctx_tiles = {}
            ctx_queue = []
            for w, (pair, g) in enumerate(waves):
                if g == 0:
                    # one PSUM bank per head: [128, nqb*128-float slices],
                    # 65 floats used per qb slice
                    ctx_tiles[pair] = [
                        ps.tile([128, QC], FP32, tag="ctx", bufs=2,
                                name=f"c_{ci}_{pair}_{hh}")
                        for hh in range(2)]
                ex = wave_scores(ci, pair, g)
                if pending_norms:
                    pending_norms.pop(0)()
                last_of_pair = g == ngrp - 1
                final_pair = last_of_pair and pair == 1 and ci + 1 == NCH
                if not final_pair:
                    for pc in sched[w]:
                        pc()
                ctx_queue.append((pair, g, ex))
                # defer ctx so the PE has scores to run while exp catches
                # up; drain continuously (small lag) so the pair-end flush
                # is small and the norm reciprocal doesn't head-of-line-
                # block the in-order DVE queue.
                lag = max(0, 3 - g) if grp > 1 else 3
                while len(ctx_queue) > lag or \
                        (ctx_queue and last_of_pair):
                    qpair, qg, qex = ctx_queue.pop(0)
                    wave_ctx_flip(ci, qpair, qg, qex, ctx_tiles[qpair], nqb)
                if last_of_pair:
                    h0 = 2 * pair
                    items = [(h0 + hh, ctx_tiles[pair][hh])
                             for hh in range(2)]
                    while pending_norms:  # drain leftovers before reassign
                        pending_norms.pop(0)()
                    if final_pair:
                        # kernel tail: the final q-block's outproj mc0
                        # halves read ctxT[0] (ready since pair 0's norm),
                        # so emit them first — they run under the norm
                        # chain; only the mc1 halves wait on the final
                        # transpose. Then the norm chain ahead of this
                        # wave's filler pieces.
                        qbf = S // 128 - 1
                        po_t = []
                        for fc in range(2):
                            po = ps.tile([128, QC], FP32, tag="scores",
                                         bufs=2, name=f"pot_{fc}")
                            nc.tensor.matmul(
                                po[:],
                                lhsT=ctxT[0][:, qbf * 128:(qbf + 1) * 128],
                                rhs=wot[0][:, fc * QC:(fc + 1) * QC],
                                start=True, stop=False,
                                skip_group_check=True)
                            po_t.append(po)
                        for pc in flip_norm_pieces(ci, pair, items):
                            pc()
                        for pc in sched[w]:
                            pc()
                    else:
                        pending_norms = flip_norm_pieces(ci, pair, items)
        # ---- kernel tail: final q-block mc1 + staging + store ----
        qbf = S // 128 - 1
        ob_f = outb.tile([128, E], BF16, tag="ob", name="ob_f")
        for fc in range(2):
            nc.tensor.matmul(
                po_t[fc],
                lhsT=ctxT[1][:, qbf * 128:(qbf + 1) * 128],
                rhs=wot[1][:, fc * QC:(fc + 1) * QC],
                start=False, stop=True,
                skip_group_check=True)
        # stage the two halves on different engines so they run in parallel
        nc.vector.tensor_copy(ob_f[:, 0:QC], po_t[0][:])
        nc.scalar.activation(ob_f[:, QC:E], po_t[1][:],
                             mybir.ActivationFunctionType.Copy)
        nc.sync.dma_start(out[qbf * 128:(qbf + 1) * 128, :], ob_f[:])


def build_module():
    nc = bacc.Bacc("TRN2", target_bir_lowering=False, debug=False)
    xT = nc.dram_tensor("xT", [E, S], BF16, kind="ExternalInput").ap()
    wq = nc.dram_tensor("wq", [E, M], BF16, kind="ExternalInput").ap()
    wk = nc.dram_tensor("wk", [E, M], BF16, kind="ExternalInput").ap()
    wv = nc.dram_tensor("wv", [E, M], BF16, kind="ExternalInput").ap()
    wo = nc.dram_tensor("wo", [M, E], BF16, kind="ExternalInput").ap()
    x8 = nc.dram_tensor("x8", [128, 8 * S], F8, kind="ExternalInput").ap()
    wk8 = nc.dram_tensor("wk8", [128, 8 * M], F8, kind="ExternalInput").ap()
    wq8 = nc.dram_tensor("wq8", [128, 8 * M], F8, kind="ExternalInput").ap()
    out = nc.dram_tensor("out", [S, E], BF16, kind="ExternalOutput").ap()
    with tile.TileContext(nc) as tc:
        _emit_kernel(tc, xT, wq, wk, wv, wo, x8, wk8, wq8, out)
    nc.compile()
    return nc


def _pack_epairs(aT):
    """[E, N] -> [128, 4*2*N] fp8: e-tile pairs side by side per partition
    (DoubleRow packing: out[p, j, t, n] = aT[(2j+t)*128 + p, n])."""
    e4m3 = ml_dtypes.float8_e4m3
    E_, N = aT.shape
    a = np.asarray(aT, dtype=np.float32).reshape(4, 2, 128, N)
    a = np.ascontiguousarray(a.transpose(2, 0, 1, 3)).astype(e4m3)
    return a.reshape(128, 8 * N)


def make_in_maps(x, w_qkv):
    """Per-core input dicts (bf16/fp8, pre-transposed host-side)."""
    bf = ml_dtypes.bfloat16
    xTb = [np.ascontiguousarray(x[b].T).astype(bf) for b in range(B)]
    x8b = [_pack_epairs(x[b].T) for b in range(B)]
    in_maps = []
    for c in range(NCORES):
        b, g = c // 4, c % 4
        cols = slice(g * M, (g + 1) * M)
        wqT = np.ascontiguousarray(w_qkv[cols, :].T)
        wkT = np.ascontiguousarray(w_qkv[E:][cols, :].T)
        in_maps.append({
            "xT": xTb[b],
            "wq": wqT.astype(bf),
            "wk": wkT.astype(bf),
            "wv": np.ascontiguousarray(w_qkv[2 * E:][cols, :].T).astype(bf),
            "x8": x8b[b],
            "wk8": _pack_epairs(wkT * 64.0),
            "wq8": _pack_epairs(wqT * 64.0),
            "wo": None,  # filled in kernel(), needs w_out
        })
    return in_maps


_RUNNER = None
_SHARDED = None


def _get_runner():
    """Build the Bass module once and return a cached callable
    (in_maps) -> [NCORES, S, E] bf16 partial outputs."""
    global _RUNNER
    if _RUNNER is not None:
        return _RUNNER

    nc = build_module()

    from concourse import bass2jax
    import jax
    from jax.sharding import Mesh, PartitionSpec
    from jax.experimental.shard_map import shard_map

    bass2jax.install_neuronx_cc_hook()

    in_names = ["xT", "wq", "wk", "wv", "x8", "wk8", "wq8", "wo"]
    out_names = ["out"]
    out_avals = [jax.core.ShapedArray((S, E), ml_dtypes.bfloat16)]
    n_params = len(in_names)
    all_names = in_names + out_names
    partition_name = (nc.partition_id_tensor.name
                      if nc.partition_id_tensor is not None else None)
    if partition_name is not None:
        all_names = all_names + [partition_name]

    def _body(*args):
        operands = list(args)
        if partition_name is not None:
            operands.append(bass2jax.partition_id_tensor())
        outs = bass2jax._bass_exec_p.bind(
            *operands,
            out_avals=tuple(out_avals),
            in_names=tuple(all_names),
            out_names=tuple(out_names),
            lowering_input_output_aliases=(),
            sim_require_finite=True,
            sim_require_nnan=True,
            nc=nc,
        )
        return tuple(outs)

    devices = jax.devices()[:NCORES]
    mesh = Mesh(np.asarray(devices), ("core",))
    n_outs = len(out_names)
    in_specs = (PartitionSpec("core"),) * (n_params + n_outs)
    out_specs = (PartitionSpec("core"),) * n_outs
    sharded = jax.jit(
        shard_map(_body, mesh=mesh, in_specs=in_specs, out_specs=out_specs,
                  check_rep=False),
        donate_argnums=tuple(range(n_params, n_params + n_outs)),
        keep_unused=True,
    )
    global _SHARDED
    _SHARDED = sharded

    def run(in_maps):
        concat_in = [
            np.concatenate([np.asarray(in_maps[c][n]) for c in range(NCORES)],
                           axis=0)
            for n in in_names
        ]
        concat_zeros = [np.zeros((NCORES * S, E), ml_dtypes.bfloat16)]
        out_arrs = sharded(*concat_in, *concat_zeros)
        return np.asarray(out_arrs[0]).reshape(NCORES, S, E)

    _RUNNER = run
    return run


def kernel(x, w_qkv, w_out, b_out):
    x = np.asarray(x, dtype=np.float32)
    w_qkv = np.asarray(w_qkv, dtype=np.float32)
    w_out = np.asarray(w_out, dtype=np.float32)
    b_out = np.asarray(b_out, dtype=np.float32)

    bf = ml_dtypes.bfloat16
    in_maps = make_in_maps(x, w_qkv)
    for c in range(NCORES):
        g = c % 4
        cols = slice(g * M, (g + 1) * M)
        in_maps[c]["wo"] = np.ascontiguousarray(w_out[:, cols].T).astype(bf)

    run = _get_runner()
    partials = run(in_maps)  # [8, S, E] bf16

    out = np.empty((B, S, E), np.float32)
    for b in range(B):
        acc = partials[4 * b].astype(np.float64)
        for i in range(1, 4):
            acc += partials[4 * b + i].astype(np.float64)
        out[b] = (acc + b_out.astype(np.float64)).astype(np.float32)
    return out
